# revision 1
# baseline (speedup 1.0000x reference)
"""T5-style encoder layer (pre-LN, RMSNorm, relative-position bias) on 8 trn2
NeuronCores, data-parallel over the batch dimension (B=8 -> one batch element
per core). Each core runs the full layer for its [S, D] slice; weights and the
relative-bias diagonal blocks are replicated.

Self-contained: hardcodes all shapes; only depends on the runtime at
/opt/trn_rl_repo.
"""

import sys

if "/opt/trn_rl_repo" not in sys.path:
    sys.path.insert(0, "/opt/trn_rl_repo")

import numpy as np
import ml_dtypes

import concourse.bass as bass
import concourse.tile as tile
from concourse import bacc
from concourse import mybir
from concourse.bass_utils import run_bass_kernel_spmd
from concourse.masks import make_identity

# ---- problem constants -----------------------------------------------------
B, S, D = 8, 1024, 1024
H, HD = 16, 64
MLP = 4096
NUM_BUCKETS, MAX_DIST = 32, 128
EPS = 1e-6
NCORES = 8
P = 128
NS = S // P        # 8 token tiles
ND = D // P        # 8 feature tiles
NM = MLP // P      # 32 mlp tiles
NDIAG = 2 * NS - 1  # 15 distinct 128x128 tile-diagonals of the bias

F32 = mybir.dt.float32
F32R = mybir.dt.float32r
BF16 = mybir.dt.bfloat16
BF16NP = ml_dtypes.bfloat16


# ---- host-side relative position bias --------------------------------------
def _rel_pos_bucket_np(rel):
    # mirrors t5x _relative_position_bucket (bidirectional), numpy fp32
    n = -rel
    num_buckets = NUM_BUCKETS // 2          # 16
    ret = (n < 0).astype(np.int32) * num_buckets
    n = np.abs(n)
    max_exact = num_buckets // 2            # 8
    is_small = n < max_exact
    val_if_large = max_exact + (
        np.log(n.astype(np.float32) / max_exact + np.finfo(np.float32).eps)
        / np.log(MAX_DIST / max_exact)
        * (num_buckets - max_exact)
    ).astype(np.int32)
    val_if_large = np.minimum(val_if_large, num_buckets - 1)
    return ret + np.where(is_small, n, val_if_large)


def _bias_blocks(rel_emb):
    """[H, 128, NDIAG, 128] f32 blocks of the transposed bias.

    Block d' (=7-m, m = k_tile - q_tile) at [p, c] = bias^T[k, q] for
    k = k_tile*128 + p, q = q_tile*128 + c, i.e. table[1023 + m*128 + p - c].
    """
    rel = np.arange(-(S - 1), S, dtype=np.int32)          # k - q in [-1023, 1023]
    buckets = _rel_pos_bucket_np(rel)                     # [2047]
    table = rel_emb[buckets, :].astype(np.float32)        # [2047, H]
    pp = np.arange(P)[:, None, None]
    dd = np.arange(NDIAG)[None, :, None]
    cc = np.arange(P)[None, None, :]
    idx = 1023 + (NS - 1 - dd) * P + pp - cc              # [128, NDIAG, 128]
    blocks = np.exp(table[idx])                           # [128, NDIAG, 128, H]
    return np.ascontiguousarray(blocks.transpose(3, 0, 1, 2)).astype(BF16NP)


# ---- device kernel ---------------------------------------------------------
def build_nc():
    nc = bacc.Bacc(None, target_bir_lowering=False)

    x_d = nc.declare_dram_parameter("x", [S, D], F32, isOutput=False)
    wq_d = nc.declare_dram_parameter("wq", [D, H * HD], F32R, isOutput=False)
    wk_d = nc.declare_dram_parameter("wk", [D, H * HD], F32R, isOutput=False)
    wv_d = nc.declare_dram_parameter("wv", [D, H * HD], F32R, isOutput=False)
    wo_d = nc.declare_dram_parameter("wo", [H * HD, D], BF16, isOutput=False)
    wi_d = nc.declare_dram_parameter("wi", [D, MLP], BF16, isOutput=False)
    wm_d = nc.declare_dram_parameter("womlp", [MLP, D], BF16, isOutput=False)
    bias_d = nc.declare_dram_parameter("biasb", [H, P, NDIAG, P], BF16, isOutput=False)
    out_d = nc.declare_dram_parameter("out", [S, D], F32, isOutput=True)
    rden_scr = nc.dram_tensor("rden_scr", [H, S], F32)

    wo_t = wo_d.ap().rearrange("(hp p) d -> p hp d", p=P)
    wq_t = wq_d.ap().rearrange("(di p) m -> p di m", p=P)
    wk_t = wk_d.ap().rearrange("(di p) m -> p di m", p=P)
    wv_t = wv_d.ap().rearrange("(di p) m -> p di m", p=P)
    wi_t = wi_d.ap().rearrange("(di p) m -> p di m", p=P)
    wm_t = wm_d.ap().rearrange("(ci p) d -> p ci d", p=P)

    with tile.TileContext(nc) as tc:
        _body(nc, tc, x_d, wq_t, wk_t, wv_t, wo_t, wi_t, wm_t, bias_d, out_d, rden_scr)
    nc.finalize()
    return nc


def _rmsnorm(nc, pools, src_ap, dst_tile, eps_t):
    """dst = src * rsqrt(mean(src^2) + eps); src [128, D] f32, dst any dtype.

    dst is also used as scratch for the squared values before the final write.
    """
    var = pools["nrm"].tile([P, 1], F32, tag="var")
    nc.vector.tensor_mul(out=dst_tile, in0=src_ap, in1=src_ap)
    nc.vector.reduce_sum(out=var, in_=dst_tile[:, :], axis=mybir.AxisListType.X)
    sd = pools["nrm"].tile([P, 1], F32, tag="sd")
    nc.scalar.activation(out=sd, in_=var, func=mybir.ActivationFunctionType.Sqrt,
                         bias=eps_t[:, :], scale=1.0 / D)
    rstd = pools["nrm"].tile([P, 1], F32, tag="rstd")
    nc.vector.reciprocal(out=rstd, in_=sd)
    nc.scalar.activation(out=dst_tile, in_=src_ap,
                         func=mybir.ActivationFunctionType.Copy,
                         bias=0.0, scale=rstd[:, :])


def _transpose_into(nc, psum_pool, src_tile, dst, si, ident):
    """PE-transpose [128, D] f32/bf16 src into dst[:, di, si*128:...]."""
    for di in range(ND):
        ps = psum_pool.tile([P, P], F32, space="PSUM", tag="tp")
        nc.tensor.transpose(ps[:, :], src_tile[:, di * P:(di + 1) * P], ident[:, :])
        nc.scalar.copy(out=dst[:, di, si * P:(si + 1) * P], in_=ps[:, :])


def _body(nc, tc, x_d, wq_t, wk_t, wv_t, wo_t, wi_t, wm_t, bias_d, out_d, rden_scr):
    fp = {}  # pools

    def pool(name, bufs, space="SBUF"):
        p = tc.alloc_tile_pool(name=name, bufs=bufs, space=space)
        fp[name] = p
        return p

    AF = mybir.ActivationFunctionType
    ALU = mybir.AluOpType

    singles = pool("singles", 1)
    ident32 = singles.tile([P, P], F32)
    make_identity(nc, ident32)
    ident16 = singles.tile([P, P], BF16)
    make_identity(nc, ident16)
    eps_t = singles.tile([P, 1], F32)
    nc.vector.memset(eps_t, EPS)

    pool("sc", 2)      # [128, D] scratch
    pool("nrm", 8)     # [128, 1] norm scalars
    pool("xs", 2)      # x stream tiles

    # activations that live through the attention block
    qkv_act = tc.alloc_tile_pool(name="qkv_act", bufs=1)
    qT = qkv_act.tile([P, ND, S], F32R)     # q^T  [hhd, s]
    kT = qkv_act.tile([P, ND, S], F32R)     # k^T  [hhd, s]
    v_ext = qkv_act.tile([P, NS, H, HD + 1], BF16)  # [tok, stile, h, hd|1]

    nc.vector.memset(v_ext[:, :, :, HD:HD + 1], 1.0)

    # ---- stage 1: rmsnorm(x) -> hT (feature-major) -------------------------
    with tc.tile_pool(name="hT_pool", bufs=1) as hT_pool:
        hT = hT_pool.tile([P, ND, S], F32R)
        with tc.tile_pool(name="tp1", bufs=4, space="PSUM") as tp1:
            for si in range(NS):
                xt = fp["xs"].tile([P, D], F32, tag="x")
                nc.sync.dma_start(out=xt, in_=x_d.ap()[si * P:(si + 1) * P, :])
                ht = fp["sc"].tile([P, D], F32, tag="h")
                _rmsnorm(nc, fp, xt[:, :], ht, eps_t)
                _transpose_into(nc, tp1, ht, hT, si, ident32)

        # ---- stage 2: QKV projections (fp32r) -------------------------------
        with tc.tile_pool(name="wqkv", bufs=2) as wqkv, \
             tc.tile_pool(name="psqkv", bufs=2, space="PSUM") as psqkv, \
             tc.tile_pool(name="psv", bufs=2, space="PSUM") as psv:
            for (w_ap, dstT) in ((wq_t, qT), (wk_t, kT)):
                for half in range(2):
                    w_sb = wqkv.tile([P, ND, 512], F32R, tag="w")
                    nc.sync.dma_start(out=w_sb, in_=w_ap[:, :, half * 512:(half + 1) * 512])
                    for mj in range(4):
                        m0 = half * 4 + mj
                        ps = psqkv.tile([P, S], F32, space="PSUM", tag="qkv")
                        for di in range(ND):
                            for sh in range(2):
                                nc.tensor.matmul(
                                    ps[:, sh * 512:(sh + 1) * 512],
                                    w_sb[:, di, mj * P:(mj + 1) * P],
                                    hT[:, di, sh * 512:(sh + 1) * 512],
                                    start=(di == 0), stop=(di == ND - 1),
                                )
                        nc.vector.tensor_copy(out=dstT[:, m0, :], in_=ps[:, :])
            # v: token-major, written into v_ext with the ones column gap
            for half in range(2):
                w_sb = wqkv.tile([P, ND, 512], F32R, tag="w")
                nc.sync.dma_start(out=w_sb, in_=wv_t[:, :, half * 512:(half + 1) * 512])
                for ci in range(NS):
                    ps = psv.tile([P, 512], F32, space="PSUM", tag="vps")
                    for di in range(ND):
                        nc.tensor.matmul(
                            ps[:, :],
                            hT[:, di, ci * P:(ci + 1) * P],
                            w_sb[:, di, :],
                            start=(di == 0), stop=(di == ND - 1),
                        )
                    nc.scalar.copy(
                        out=v_ext[:, ci, half * 8:half * 8 + 8, 0:HD],
                        in_=ps[:, :].rearrange("p (h e) -> p h e", e=HD),
                    )

    # ---- stage 3: attention per head ---------------------------------------
    attnT_pool = tc.alloc_tile_pool(name="attnT_pool", bufs=1)
    # attn^T packed: head 2i on partitions 0-63, head 2i+1 on 64-127
    attnT = attnT_pool.tile([P, H // 2, S], BF16)
    with (
        tc.tile_pool(name="biasp", bufs=2) as biasp,
        tc.tile_pool(name="wexpp", bufs=6) as wexpp,
        tc.tile_pool(name="lgp", bufs=2, space="PSUM") as lgp,
        tc.tile_pool(name="aup", bufs=2, space="PSUM") as aup,
        tc.tile_pool(name="rp", bufs=2) as rp,
    ):
        for h in range(H):
            hb = HD * (h % 2)           # partition base of this head in qT/kT
            hm = h // 2
            bias_sb = biasp.tile([P, NDIAG, P], BF16, tag="bias")
            nc.sync.dma_start(out=bias_sb, in_=bias_d.ap()[h])
            au = aup.tile([HD + 1, S], F32, tag="au")
            for ki in range(NS):
                lg = lgp.tile([P, S], F32, tag="lg")
                for qh in range(2):
                    nc.tensor.matmul(
                        lg[:, qh * 512:(qh + 1) * 512],
                        kT[hb:hb + HD, hm, ki * P:(ki + 1) * P],
                        qT[hb:hb + HD, hm, qh * 512:(qh + 1) * 512],
                        start=True, stop=True,
                    )
                # w = exp(l) * exp(bias): exp on ACT straight from PSUM,
                # then an all-bf16 SBUF multiply on DVE (2x mode)
                ex = wexpp.tile([P, S], BF16, tag="ex")
                nc.scalar.activation(out=ex, in_=lg[:, :], func=AF.Exp)
                wexp = wexpp.tile([P, S], BF16, tag="wexp")
                nc.vector.tensor_mul(
                    out=wexp[:, :].rearrange("p (c w) -> p c w", w=P),
                    in0=ex[:, :].rearrange("p (c w) -> p c w", w=P),
                    in1=bias_sb[:, NS - 1 - ki:2 * NS - 1 - ki, :],
                )
                for qh in range(2):
                    nc.tensor.matmul(
                        au[:, qh * 512:(qh + 1) * 512],
                        v_ext[:, ki, h, :],
                        wexp[:, qh * 512:(qh + 1) * 512],
                        start=(ki == 0), stop=(ki == NS - 1),
                    )
            rden = rp.tile([HD + 1, S], F32, tag="rden")
            nc.vector.reciprocal(out=rden[HD:HD + 1, :], in_=au[HD:HD + 1, :])
            # broadcast 1/denom to all 64 hd partitions via a DRAM bounce
            nc.sync.dma_start(out=rden_scr.ap()[h:h + 1, :], in_=rden[HD:HD + 1, :])
            rbc = rp.tile([HD, S], F32, tag="rbc")
            nc.sync.dma_start(out=rbc[:, :],
                              in_=rden_scr.ap()[h:h + 1, :].broadcast_to((HD, S)))
            hb2 = HD * (h % 2)
            nc.vector.tensor_mul(
                out=attnT[hb2:hb2 + HD, h // 2, :], in0=au[0:HD, :], in1=rbc[:, :],
            )

    # ---- stage 4: attn @ wo + residual -------------------------------------
    out1_pool = tc.alloc_tile_pool(name="out1_pool", bufs=1, side="right")
    out1 = out1_pool.tile([P, NS, D], F32)    # x + attn_out, token-major
    with tc.tile_pool(name="wop", bufs=1) as wop, \
         tc.tile_pool(name="ops", bufs=2, space="PSUM") as ops:
        wo_sb = wop.tile([P, H // 2, D], BF16)
        nc.sync.dma_start(out=wo_sb, in_=wo_t[:, :, :])
        for si in range(NS):
            ps = ops.tile([P, D], F32, tag="wo")
            for hp in range(H // 2):
                for dh in range(2):
                    nc.tensor.matmul(
                        ps[:, dh * 512:(dh + 1) * 512],
                        attnT[:, hp, si * P:(si + 1) * P],
                        wo_sb[:, hp, dh * 512:(dh + 1) * 512],
                        start=(hp == 0), stop=(hp == H // 2 - 1),
                    )
            xt = fp["xs"].tile([P, D], F32, tag="x")
            nc.sync.dma_start(out=xt, in_=x_d.ap()[si * P:(si + 1) * P, :])
            nc.vector.tensor_add(out=out1[:, si, :], in0=ps[:, :], in1=xt[:, :])
    attnT_pool.release()
    qkv_act.release()

    # ---- stage 5: rmsnorm(out1) -> h2T (bf16, feature-major) ---------------
    with tc.tile_pool(name="h2T_pool", bufs=1) as h2T_pool, \
         tc.tile_pool(name="yT_pool", bufs=1) as yT_pool:
        h2T = h2T_pool.tile([P, ND, S], BF16)
        with tc.tile_pool(name="tp5", bufs=4, space="PSUM") as tp5:
            for si in range(NS):
                h2 = fp["sc"].tile([P, D], BF16, tag="h2")
                _rmsnorm(nc, fp, out1[:, si, :], h2, eps_t)
                for di in range(ND):
                    ps = tp5.tile([P, P], BF16, space="PSUM", tag="tp16")
                    nc.tensor.transpose(ps[:, :], h2[:, di * P:(di + 1) * P], ident16[:, :])
                    nc.scalar.copy(out=h2T[:, di, si * P:(si + 1) * P], in_=ps[:, :])

        # ---- stage 6: y^T = relu(wi^T @ h2^T) (bf16) ------------------------
        yT = yT_pool.tile([P, NM, S], BF16)
        with tc.tile_pool(name="wip", bufs=2) as wip, \
             tc.tile_pool(name="psy", bufs=2, space="PSUM") as psy:
            for eighth in range(8):
                wi_sb = wip.tile([P, ND, MLP // 8], BF16, tag="wi")
                nc.sync.dma_start(out=wi_sb, in_=wi_t[:, :, eighth * (MLP // 8):(eighth + 1) * (MLP // 8)])
                for mj in range(NM // 8):
                    m0 = eighth * (NM // 8) + mj
                    ps = psy.tile([P, S], F32, space="PSUM", tag="y")
                    for di in range(ND):
                        for sh in range(2):
                            nc.tensor.matmul(
                                ps[:, sh * 512:(sh + 1) * 512],
                                wi_sb[:, di, mj * P:(mj + 1) * P],
                                h2T[:, di, sh * 512:(sh + 1) * 512],
                                start=(di == 0), stop=(di == ND - 1),
                            )
                    nc.scalar.activation(out=yT[:, m0, :], in_=ps[:, :], func=AF.Relu)

        # ---- stage 7: out = out1 + y^T.T @ womlp ----------------------------
        # womlp is streamed per 128-row chunk; four output tiles accumulate
        # concurrently (8 PSUM banks), so womlp is read twice overall.
        with tc.tile_pool(name="wmp", bufs=3) as wmp, \
             tc.tile_pool(name="o2ps", bufs=4, space="PSUM") as o2ps:
            for sg in range(2):
                pss = [o2ps.tile([P, D], F32, tag="o2", name=f"o2_{sg}_{i}") for i in range(4)]
                for ci in range(NM):
                    wmc = wmp.tile([P, D], BF16, tag="wm")
                    nc.sync.dma_start(out=wmc, in_=wm_t[:, ci, :])
                    for i4 in range(4):
                        si = sg * 4 + i4
                        for dh in range(2):
                            nc.tensor.matmul(
                                pss[i4][:, dh * 512:(dh + 1) * 512],
                                yT[:, ci, si * P:(si + 1) * P],
                                wmc[:, dh * 512:(dh + 1) * 512],
                                start=(ci == 0), stop=(ci == NM - 1),
                            )
                for i4 in range(4):
                    si = sg * 4 + i4
                    oo = fp["sc"].tile([P, D], F32, tag="oo")
                    nc.vector.tensor_add(out=oo, in0=pss[i4][:, :], in1=out1[:, si, :])
                    nc.sync.dma_start(out=out_d.ap()[si * P:(si + 1) * P, :], in_=oo)

    out1_pool.release()
    for name in ("xs", "nrm", "sc", "singles"):
        fp[name].release()


# ---- host wrapper ----------------------------------------------------------
_NC_CACHE = {}


def _get_nc():
    if "nc" not in _NC_CACHE:
        _NC_CACHE["nc"] = build_nc()
    return _NC_CACHE["nc"]


def _get_exec():
    """Compile once: a sharded PJRT executable over the 8 NeuronCores."""
    if "exec" in _NC_CACHE:
        return _NC_CACHE["exec"]
    import jax
    from jax.sharding import Mesh, PartitionSpec, NamedSharding
    from jax.experimental.shard_map import shard_map
    from concourse.bass2jax import (
        _bass_exec_p, install_neuronx_cc_hook, partition_id_tensor,
    )

    nc = _get_nc()
    install_neuronx_cc_hook()
    pname = nc.partition_id_tensor.name if nc.partition_id_tensor else None
    in_names, out_names, out_avals, zero_outs = [], [], [], []
    for alloc in nc.m.functions[0].allocations:
        if not isinstance(alloc, mybir.MemoryLocationSet):
            continue
        name = alloc.memorylocations[0].name
        if alloc.kind == "ExternalInput":
            if name != pname:
                in_names.append(name)
        elif alloc.kind == "ExternalOutput":
            out_names.append(name)
            shape = tuple(alloc.tensor_shape)
            dtype = mybir.dt.np(alloc.dtype)
            out_avals.append(jax.core.ShapedArray(shape, dtype))
            zero_outs.append(np.zeros(shape, dtype))
    n_params = len(in_names)
    all_in_names = in_names + out_names + ([pname] if pname else [])

    def _body(*args):
        operands = list(args)
        if pname is not None:
            operands.append(partition_id_tensor())
        outs = _bass_exec_p.bind(
            *operands,
            out_avals=tuple(out_avals),
            in_names=tuple(all_in_names),
            out_names=tuple(out_names),
            lowering_input_output_aliases=(),
            sim_require_finite=True,
            sim_require_nnan=True,
            nc=nc,
        )
        return tuple(outs)

    n_outs = len(out_avals)
    devices = jax.devices()[:NCORES]
    mesh = Mesh(np.asarray(devices), ("core",))
    sharded = jax.jit(
        shard_map(_body, mesh=mesh,
                  in_specs=(PartitionSpec("core"),) * (n_params + n_outs),
                  out_specs=(PartitionSpec("core"),) * n_outs,
                  check_rep=False),
        donate_argnums=tuple(range(n_params, n_params + n_outs)),
        keep_unused=True,
    )
    sh = NamedSharding(mesh, PartitionSpec("core"))
    _NC_CACHE["exec"] = (sharded, in_names, out_names, zero_outs, sh)
    return _NC_CACHE["exec"]


def _prep_inputs(x, ln1_scale, wq, wk, wv, wo_attn, ln2_scale, wi, wo_mlp, rel_emb):
    x = np.asarray(x, np.float32)
    ln1 = np.asarray(ln1_scale, np.float32)[:, None]
    ln2 = np.asarray(ln2_scale, np.float32)[:, None]
    wq_h = (np.asarray(wq, np.float32) * ln1).astype(np.float32)
    wk_h = (np.asarray(wk, np.float32) * ln1).astype(np.float32)
    wv_h = (np.asarray(wv, np.float32) * ln1).astype(np.float32)
    wo_h = np.asarray(wo_attn, np.float32).astype(BF16NP)
    wi_h = (np.asarray(wi, np.float32) * ln2).astype(BF16NP)
    wm_h = np.asarray(wo_mlp, np.float32).astype(BF16NP)
    biasb = _bias_blocks(np.asarray(rel_emb, np.float32))
    shared = {
        "wq": wq_h, "wk": wk_h, "wv": wv_h, "wo": wo_h,
        "wi": wi_h, "womlp": wm_h, "biasb": biasb,
    }
    in_maps = [dict(shared, x=np.ascontiguousarray(x[b])) for b in range(NCORES)]
    return in_maps


def kernel(x, ln1_scale, wq, wk, wv, wo_attn, ln2_scale, wi, wo_mlp, rel_emb):
    import jax
    in_maps = _prep_inputs(x, ln1_scale, wq, wk, wv, wo_attn, ln2_scale,
                           wi, wo_mlp, rel_emb)
    sharded, in_names, out_names, zero_outs, sh = _get_exec()
    concat_in = [
        jax.device_put(
            np.concatenate([in_maps[c][n] for c in range(NCORES)], axis=0), sh)
        for n in in_names
    ]
    czero = [
        jax.device_put(np.zeros((NCORES * z.shape[0], *z.shape[1:]), z.dtype), sh)
        for z in zero_outs
    ]
    outs = sharded(*concat_in, *czero)
    oidx = out_names.index("out")
    full = np.asarray(outs[oidx]).reshape(NCORES, S, D)
    return full.astype(np.float32)



# revision 17
# speedup vs baseline: 1.0960x; 1.0960x over previous
"""T5-style encoder layer (pre-LN, RMSNorm, relative-position bias) on 8 trn2
NeuronCores, data-parallel over the batch dimension (B=8 -> one batch element
per core). Each core runs the full layer for its [S, D] slice; weights and the
relative-bias data are replicated.

Self-contained: hardcodes all shapes; only depends on the runtime at
/opt/trn_rl_repo.
"""

import sys

if "/opt/trn_rl_repo" not in sys.path:
    sys.path.insert(0, "/opt/trn_rl_repo")

import numpy as np
import ml_dtypes

import concourse.bass as bass
import concourse.tile as tile
from concourse import bacc
from concourse import mybir
from concourse.masks import make_identity

# ---- problem constants -----------------------------------------------------
B, S, D = 8, 1024, 1024
H, HD = 16, 64
MLP = 4096
NUM_BUCKETS, MAX_DIST = 32, 128
EPS = 1e-6
NCORES = 8
P = 128
NS = S // P        # 8 token tiles
ND = D // P        # 8 feature tiles
NM = MLP // P      # 32 mlp tiles

F32 = mybir.dt.float32
F32R = mybir.dt.float32r
BF16 = mybir.dt.bfloat16
F8 = mybir.dt.float8e4
BF16NP = ml_dtypes.bfloat16
F8NP = ml_dtypes.float8_e4m3

import os
WI_FP8 = os.environ.get("K_WI_FP8", "1") == "1"   # 2-term fp8 DoubleRow wi
SEG_GPSIMD = os.environ.get("K_SEG_GPSIMD", "1") == "1"
ACT_DMA = os.environ.get("K_ACT_DMA", "1") == "1"
TP_SHIFT = os.environ.get("K_TP_SHIFT", "1") == "1"

AF = mybir.ActivationFunctionType
DR = mybir.MatmulPerfMode.DoubleRow


# ---- host-side relative position bias --------------------------------------
def _rel_pos_bucket_np(rel):
    # mirrors t5x _relative_position_bucket (bidirectional), numpy fp32
    n = -rel
    num_buckets = NUM_BUCKETS // 2          # 16
    ret = (n < 0).astype(np.int32) * num_buckets
    n = np.abs(n)
    max_exact = num_buckets // 2            # 8
    is_small = n < max_exact
    val_if_large = max_exact + (
        np.log(n.astype(np.float32) / max_exact + np.finfo(np.float32).eps)
        / np.log(MAX_DIST / max_exact)
        * (num_buckets - max_exact)
    ).astype(np.int32)
    val_if_large = np.minimum(val_if_large, num_buckets - 1)
    return ret + np.where(is_small, n, val_if_large)


def _bias_data(rel_emb):
    """Compressed exp(bias) data.

    Returns (bias3, cexp):
      bias3 [H, 128, 3, 128] bf16: block d (m = 1-d = k_tile - q_tile) at
        [p, c] = exp(bias[k, q]) for k = k_tile*128 + p, q = q_tile*128 + c,
        i.e. exp(table[k - q]) with k - q = m*128 + p - c.
      cexp  [1, 2H] f32: per head, exp of the two saturated buckets:
        [2h]   = exp(emb[31, h])  (k - q >= 128, q_tile <= k_tile - 2)
        [2h+1] = exp(emb[15, h])  (k - q <= -128, q_tile >= k_tile + 2)
    """
    rel = np.arange(-(S - 1), S, dtype=np.int32)          # k - q in [-1023, 1023]
    buckets = _rel_pos_bucket_np(rel)                     # [2047]
    table = rel_emb[buckets, :].astype(np.float32)        # [2047, H]
    pp = np.arange(P)[:, None, None]
    dd = np.arange(3)[None, :, None]
    cc = np.arange(P)[None, None, :]
    idx = 1023 + (1 - dd) * P + pp - cc                   # [128, 3, 128]
    blocks = np.exp(table[idx])                           # [128, 3, 128, H]
    bias3 = np.ascontiguousarray(blocks.transpose(3, 0, 1, 2)).astype(BF16NP)
    cexp = np.empty((1, 2 * H), np.float32)
    cexp[0, 0::2] = np.exp(rel_emb[31, :].astype(np.float32))
    cexp[0, 1::2] = np.exp(rel_emb[15, :].astype(np.float32))
    return bias3, cexp


# ---- device kernel ---------------------------------------------------------
def build_nc():
    nc = bacc.Bacc(None, target_bir_lowering=False)

    x_d = nc.declare_dram_parameter("x", [S, D], F32, isOutput=False)
    wq_d = nc.declare_dram_parameter("wq", [D, H * HD], F32R, isOutput=False)
    wk_d = nc.declare_dram_parameter("wk", [D, H * HD], F32R, isOutput=False)
    wv_d = nc.declare_dram_parameter("wv", [D, H * HD], F32R, isOutput=False)
    wo_d = nc.declare_dram_parameter("wo", [H * HD, D], BF16, isOutput=False)
    if WI_FP8:
        wi8_d = nc.declare_dram_parameter("wi8", [D, MLP], F8, isOutput=False)
        wil_d = nc.declare_dram_parameter("wil", [D, MLP], F8, isOutput=False)
    else:
        wi8_d = nc.declare_dram_parameter("wi8", [D, MLP], BF16, isOutput=False)
        wil_d = None
    wm_d = nc.declare_dram_parameter("womlp", [MLP, D], BF16, isOutput=False)
    bias_d = nc.declare_dram_parameter("bias3", [H, P, 3, P], BF16, isOutput=False)
    cexp_d = nc.declare_dram_parameter("cexp", [1, 2 * H], F32, isOutput=False)
    out_d = nc.declare_dram_parameter("out", [S, D], F32, isOutput=True)

    wo_t = wo_d.ap().rearrange("(hp p) d -> p hp d", p=P)
    wq_t = wq_d.ap().rearrange("(di p) m -> p di m", p=P)
    wk_t = wk_d.ap().rearrange("(di p) m -> p di m", p=P)
    wv_t = wv_d.ap().rearrange("(di p) m -> p di m", p=P)
    wi8_t = wi8_d.ap().rearrange("(di p) m -> p di m", p=P)
    wil_t = wil_d.ap().rearrange("(di p) m -> p di m", p=P) if WI_FP8 else None
    wm_t = wm_d.ap().rearrange("(ci p) d -> p ci d", p=P)
    bias_t = bias_d.ap().rearrange("h p d c -> p h d c")

    with tile.TileContext(nc) as tc:
        _body(nc, tc, x_d, wq_t, wk_t, wv_t, wo_t, wi8_t, wil_t, wm_t,
              bias_t, cexp_d, out_d)
    nc.finalize()
    return nc


def _rms_factor(nc, nrm, src_ap, sq_tile, eps_t):
    """rstd [P,1] = rsqrt(mean(src^2) + eps); sq_tile is scratch."""
    var = nrm.tile([P, 1], F32, tag="var")
    nc.vector.tensor_mul(out=sq_tile, in0=src_ap, in1=src_ap)
    nc.vector.reduce_sum(out=var, in_=sq_tile, axis=mybir.AxisListType.X)
    sd = nrm.tile([P, 1], F32, tag="sd")
    nc.scalar.activation(out=sd, in_=var, func=AF.Sqrt,
                         bias=eps_t[:, :], scale=1.0 / D)
    rstd = nrm.tile([P, 1], F32, tag="rstd")
    nc.vector.reciprocal(out=rstd, in_=sd)
    return rstd


def _body(nc, tc, x_d, wq_t, wk_t, wv_t, wo_t, wi8_t, wil_t, wm_t,
          bias_t, cexp_d, out_d):
    ALU = mybir.AluOpType

    # ---- persistent small data ---------------------------------------------
    singles = tc.alloc_tile_pool(name="singles", bufs=1)
    ident16 = singles.tile([P, P], BF16)
    make_identity(nc, ident16)
    ident32 = singles.tile([P, P], F32)
    make_identity(nc, ident32)
    ident32r = singles.tile([P, P], F32R)
    nc.vector.tensor_copy(out=ident32r, in_=ident32)
    eps_t = singles.tile([P, 1], F32)
    nc.vector.memset(eps_t, EPS)
    cexp_sb = singles.tile([P, 2 * H], F32)
    nc.sync.dma_start(out=cexp_sb, in_=cexp_d.ap()[0:1, :].broadcast_to((P, 2 * H)))
    bias_sb = singles.tile([P, H, 3, P], BF16)
    nc.sync.dma_start(out=bias_sb, in_=bias_t)

    # activations that live through the attention block
    qkv_act = tc.alloc_tile_pool(name="qkv_act", bufs=1)
    hT = qkv_act.tile([P, ND, S], F32R)      # rmsnorm(x)^T  [d, s]
    qT = qkv_act.tile([P, ND, S], F32R)      # q^T  [m, s] per head-pair tile
    kT = qkv_act.tile([P, ND, S], F32R)
    v_ext = qkv_act.tile([P, NS, H, HD + 1], BF16)  # [tok, stile, h, hd|1]
    nc.vector.memset(v_ext[:, :, :, HD:HD + 1], 1.0)
    # right-side stack: out1 (lives to the end) below attnT (dies after wo)
    out1_pool = tc.alloc_tile_pool(name="out1_pool", bufs=1, side="right")
    out1 = out1_pool.tile([P, NS, D], F32)    # x + attn_out, token-major
    attnT_pool = tc.alloc_tile_pool(name="attnT_pool", bufs=1, side="right")
    attnT = attnT_pool.tile([P, H // 2, S], BF16)   # heads packed 2/tile

    # ---- stage 1: rmsnorm(x) -> hT (feature-major) -------------------------
    with tc.tile_pool(name="xs1", bufs=3) as xs1, \
         tc.tile_pool(name="sq1", bufs=2) as sq1, \
         tc.tile_pool(name="nrm1", bufs=4) as nrm1, \
         tc.tile_pool(name="hts", bufs=2) as hts, \
         tc.tile_pool(name="st1ps", bufs=2, space="PSUM") as st1ps:
        for si in range(NS):
            xt = xs1.tile([P, D], F32, tag="x")
            nc.sync.dma_start(out=xt, in_=x_d.ap()[si * P:(si + 1) * P, :])
            sq = sq1.tile([P, D], F32, tag="sq")
            rstd = _rms_factor(nc, nrm1, xt[:, :], sq, eps_t)
            ht = hts.tile([P, D], F32R, tag="ht")
            nc.scalar.activation(out=ht, in_=xt, func=AF.Copy,
                                 bias=0.0, scale=rstd[:, :])
            tp = st1ps.tile([P, ND, P], F32R, space="PSUM", tag="tp1")
            for di in range(ND):
                nc.tensor.transpose(tp[:, di, :], ht[:, di * P:(di + 1) * P],
                                    ident32r[:, :])
            nc.scalar.copy(out=hT[:, :, si * P:(si + 1) * P], in_=tp[:, :, :])

    # ---- stage 2a: V projection (token-major, all heads) -------------------
    with tc.tile_pool(name="wvp", bufs=2) as wvp, \
         tc.tile_pool(name="vps", bufs=2, space="PSUM") as vps:
        for half in range(2):
            w_sb = wvp.tile([P, ND, 512], F32R, tag="wv")
            nc.sync.dma_start(out=w_sb, in_=wv_t[:, :, half * 512:(half + 1) * 512])
            for ci in range(NS):
                ps = vps.tile([P, 512], F32, space="PSUM", tag="vps")
                for di in range(ND):
                    nc.tensor.matmul(
                        ps[:, :],
                        hT[:, di, ci * P:(ci + 1) * P],
                        w_sb[:, di, :],
                        start=(di == 0), stop=(di == ND - 1),
                    )
                nc.scalar.copy(
                    out=v_ext[:, ci, half * 8:half * 8 + 8, 0:HD],
                    in_=ps[:, :].rearrange("p (h e) -> p h e", e=HD),
                )

    # ---- stage 2b/3: QK projection interleaved with per-head attention -----
    with tc.tile_pool(name="wqkp", bufs=4) as wqkp, \
         tc.tile_pool(name="qkp", bufs=1, space="PSUM") as qkp, \
         tc.tile_pool(name="lgp", bufs=2, space="PSUM") as lgp, \
         tc.tile_pool(name="avp", bufs=1, space="PSUM") as avp, \
         tc.tile_pool(name="tpp", bufs=1, space="PSUM") as tpp, \
         tc.tile_pool(name="exq", bufs=2) as exq, \
         tc.tile_pool(name="wexpp", bufs=3) as wexpp, \
         tc.tile_pool(name="rdp", bufs=4) as rdp, \
         tc.tile_pool(name="avsbp", bufs=2) as avsbp:

        def qk_project(hm):
            for (w_ap, dstT, nm) in ((wq_t, qT, "q"), (wk_t, kT, "k")):
                w_sb = wqkp.tile([P, ND, P], F32R, tag="w", name=f"w{nm}{hm}")
                nc.sync.dma_start(out=w_sb, in_=w_ap[:, :, hm * P:(hm + 1) * P])
                ps = qkp.tile([P, 2, 256], F32, space="PSUM", tag="qk",
                              name=f"qk{nm}{hm}")
                for qt in range(4):
                    sl = ps[:, qt % 2, :]
                    for di in range(ND):
                        nc.tensor.matmul(
                            sl,
                            w_sb[:, di, :],
                            hT[:, di, qt * 256:(qt + 1) * 256],
                            start=(di == 0), stop=(di == ND - 1),
                        )
                    nc.vector.tensor_copy(out=dstT[:, hm, qt * 256:(qt + 1) * 256],
                                          in_=sl)

        def attn_head(h):
            hm, hb = h // 2, HD * (h % 2)
            av = avp.tile([P, NS, P], F32, space="PSUM", tag="av")
            for ki in range(NS):
                lg = lgp.tile([P, S], F32, space="PSUM", tag="lg")
                for qh in range(2):
                    nc.tensor.matmul(
                        lg[:, qh * 512:(qh + 1) * 512],
                        kT[hb:hb + HD, hm, ki * P:(ki + 1) * P],
                        qT[hb:hb + HD, hm, qh * 512:(qh + 1) * 512],
                        start=True, stop=True,
                    )
                ex = exq.tile([P, S], BF16, tag="ex")
                nc.scalar.activation(out=ex, in_=lg[:, :], func=AF.Exp)
                wexp = wexpp.tile([P, S], BF16, tag="wexp")
                # middle (varying-bias) window: q tiles [ki-1, ki+1]
                q0, q1 = max(ki - 1, 0), min(ki + 2, NS)
                d0 = q0 - (ki - 1)
                nc.vector.tensor_mul(
                    out=wexp[:, q0 * P:q1 * P].rearrange("p (c w) -> p c w", w=P),
                    in0=ex[:, q0 * P:q1 * P].rearrange("p (c w) -> p c w", w=P),
                    in1=bias_sb[:, h, d0:d0 + (q1 - q0), :],
                )
                # constant-bias segments (saturated buckets)
                segs = []
                if q0 > 0:
                    segs.append((0, q0 * P, cexp_sb[:, 2 * h:2 * h + 1]))
                if q1 < NS:
                    segs.append((q1 * P, S, cexp_sb[:, 2 * h + 1:2 * h + 2]))
                segs.sort(key=lambda t: t[1] - t[0])
                for i, (a, b, c_ap) in enumerate(segs):
                    eng = nc.gpsimd if (SEG_GPSIMD and i == len(segs) - 1) else nc.vector
                    eng.tensor_scalar_mul(out=wexp[:, a:b], in0=ex[:, a:b],
                                          scalar1=c_ap)
                for qi in range(NS):
                    # one accumulation group per PSUM bank (4 qi slots/bank):
                    # start pending-zeroes the whole bank, so only the first
                    # matmul in each bank starts and the last stops.
                    nc.tensor.matmul(
                        av[:, qi, 0:HD + 1],
                        wexp[:, qi * P:(qi + 1) * P],
                        v_ext[:, ki, h, :],
                        start=(ki == 0 and qi % 4 == 0),
                        stop=(ki == NS - 1 and qi % 4 == 3),
                    )
            # drain: per q tile normalize (fp32 psum * rden) -> bf16, transpose
            av_sb = avsbp.tile([P, NS, HD], BF16, tag="avsb")
            tp = tpp.tile([P, NS, P], BF16, space="PSUM", tag="tp")
            for qi in range(NS):
                rden = rdp.tile([P, 1], F32, tag="rd")
                nc.vector.reciprocal(out=rden, in_=av[:, qi, HD:HD + 1])
                nc.vector.tensor_scalar_mul(out=av_sb[:, qi, :],
                                            in0=av[:, qi, 0:HD],
                                            scalar1=rden)
                tb = hb if TP_SHIFT else 0
                nc.tensor.transpose(tp[tb:tb + HD, qi, :], av_sb[:, qi, :],
                                    ident16[:, :])
            tb = hb if TP_SHIFT else 0
            nc.vector.tensor_copy(out=attnT[hb:hb + HD, hm, :],
                                  in_=tp[tb:tb + HD, :, :])

        for hm in range(ND):
            qk_project(hm)
            attn_head(2 * hm)
            attn_head(2 * hm + 1)

    # ---- stage 4/5: attn @ wo + residual, rmsnorm -> h2T --------------------
    qkv_act.release()
    yT_pool = tc.alloc_tile_pool(name="yT_pool", bufs=1)
    yT = yT_pool.tile([P, NM, S], BF16)
    h2T_pool = tc.alloc_tile_pool(name="h2T_pool", bufs=1)
    h2T = h2T_pool.tile([P, ND, S], F8 if WI_FP8 else BF16)

    with tc.tile_pool(name="wop", bufs=1) as wop, \
         tc.tile_pool(name="xs4", bufs=3) as xs4, \
         tc.tile_pool(name="ops", bufs=2, space="PSUM") as ops, \
         tc.tile_pool(name="tp5", bufs=2, space="PSUM") as tp5, \
         tc.tile_pool(name="sq5", bufs=2) as sq5, \
         tc.tile_pool(name="nrm5", bufs=4) as nrm5, \
         tc.tile_pool(name="h2p", bufs=2) as h2p:
        wo_sb = wop.tile([P, H // 2, D], BF16)
        for hq in range(4):
            nc.sync.dma_start(out=wo_sb[:, 2 * hq:2 * hq + 2, :],
                              in_=wo_t[:, 2 * hq:2 * hq + 2, :])
        for si in range(NS):
            xt = xs4.tile([P, D], F32, tag="x")
            nc.sync.dma_start(out=xt, in_=x_d.ap()[si * P:(si + 1) * P, :])
            ps = ops.tile([P, D], F32, space="PSUM", tag="wo")
            for hp in range(H // 2):
                for dh in range(2):
                    nc.tensor.matmul(
                        ps[:, dh * 512:(dh + 1) * 512],
                        attnT[:, hp, si * P:(si + 1) * P],
                        wo_sb[:, hp, dh * 512:(dh + 1) * 512],
                        start=(hp == 0), stop=(hp == H // 2 - 1),
                    )
            nc.vector.tensor_add(out=out1[:, si, :], in0=ps[:, :], in1=xt[:, :])
            # stage 5 per si: rmsnorm -> h2 (bf16) -> transpose -> h2T
            sq = sq5.tile([P, D], F32, tag="sq")
            rstd = _rms_factor(nc, nrm5, out1[:, si, :], sq, eps_t)
            h2 = h2p.tile([P, D], BF16, tag="h2")
            nc.scalar.activation(out=h2, in_=out1[:, si, :], func=AF.Copy,
                                 bias=0.0, scale=rstd[:, :])
            tp = tp5.tile([P, ND, P], BF16, space="PSUM", tag="tp5")
            for di in range(ND):
                nc.tensor.transpose(tp[:, di, :], h2[:, di * P:(di + 1) * P],
                                    ident16[:, :])
            nc.vector.tensor_copy(out=h2T[:, :, si * P:(si + 1) * P], in_=tp[:, :, :])
    attnT_pool.release()

    # ---- stage 6: y^T = relu(wi^T @ h2^T) ----------------------------------
    with tc.tile_pool(name="wip", bufs=2) as wip, \
         tc.tile_pool(name="psy", bufs=2, space="PSUM") as psy:
        for eighth in range(8):
            sl = slice(eighth * (MLP // 8), (eighth + 1) * (MLP // 8))
            wi_sb = wip.tile([P, ND, MLP // 8], F8 if WI_FP8 else BF16, tag="wi8")
            nc.sync.dma_start(out=wi_sb, in_=wi8_t[:, :, sl])
            if WI_FP8:
                wil_sb = wip.tile([P, ND, MLP // 8], F8, tag="wil")
                nc.sync.dma_start(out=wil_sb, in_=wil_t[:, :, sl])
            for mj in range(NM // 8):
                m0 = eighth * (NM // 8) + mj
                ps = psy.tile([P, S], F32, space="PSUM", tag="y")
                if WI_FP8:
                    for sh in range(2):
                        for ti, term in enumerate((wi_sb, wil_sb)):
                            for pr in range(4):
                                nc.tensor.matmul(
                                    ps[:, sh * 512:(sh + 1) * 512],
                                    term[:, 2 * pr:2 * pr + 2, mj * P:(mj + 1) * P],
                                    h2T[:, 2 * pr:2 * pr + 2, sh * 512:(sh + 1) * 512],
                                    start=(ti == 0 and pr == 0),
                                    stop=(ti == 1 and pr == 3),
                                    perf_mode=DR,
                                )
                else:
                    for sh in range(2):
                        for di in range(ND):
                            nc.tensor.matmul(
                                ps[:, sh * 512:(sh + 1) * 512],
                                wi_sb[:, di, mj * P:(mj + 1) * P],
                                h2T[:, di, sh * 512:(sh + 1) * 512],
                                start=(di == 0), stop=(di == ND - 1),
                            )
                nc.scalar.activation(out=yT[:, m0, :], in_=ps[:, :], func=AF.Relu)
    h2T_pool.release()

    # ---- stage 7: out = out1 + y^T.T @ womlp -------------------------------
    # 4 groups (dh half of D x sg half of tokens); womlp streamed per group,
    # two groups of PSUM banks so group g+1 computes while g drains.
    with tc.tile_pool(name="wmp", bufs=3) as wmp, \
         tc.tile_pool(name="oop", bufs=4) as oop, \
         tc.tile_pool(name="o2ps", bufs=8, space="PSUM") as o2ps:
        for dh in range(2):
            for sg in range(2):
                pss = [o2ps.tile([P, 512], F32, space="PSUM", tag="o2",
                                 name=f"o2_{dh}_{sg}_{i}") for i in range(4)]
                for ci in range(NM):
                    wmc = wmp.tile([P, 512], BF16, tag="wm")
                    dma_eng = nc.scalar if ACT_DMA else nc.sync
                    dma_eng.dma_start(out=wmc,
                                      in_=wm_t[:, ci, dh * 512:(dh + 1) * 512])
                    for i4 in range(4):
                        si = sg * 4 + i4
                        nc.tensor.matmul(
                            pss[i4][:, :],
                            yT[:, ci, si * P:(si + 1) * P],
                            wmc[:, :],
                            start=(ci == 0), stop=(ci == NM - 1),
                        )
                for i4 in range(4):
                    si = sg * 4 + i4
                    oo = oop.tile([P, 512], F32, tag="oo")
                    nc.vector.tensor_add(out=oo, in0=pss[i4][:, :],
                                         in1=out1[:, si, dh * 512:(dh + 1) * 512])
                    nc.sync.dma_start(
                        out=out_d.ap()[si * P:(si + 1) * P, dh * 512:(dh + 1) * 512],
                        in_=oo)

    yT_pool.release()
    out1_pool.release()
    singles.release()


# ---- host wrapper ----------------------------------------------------------
_NC_CACHE = {}


def _get_nc():
    if "nc" not in _NC_CACHE:
        _NC_CACHE["nc"] = build_nc()
    return _NC_CACHE["nc"]


def _get_exec():
    """Compile once: a sharded PJRT executable over the 8 NeuronCores."""
    if "exec" in _NC_CACHE:
        return _NC_CACHE["exec"]
    import jax
    from jax.sharding import Mesh, PartitionSpec, NamedSharding
    from jax.experimental.shard_map import shard_map
    from concourse.bass2jax import (
        _bass_exec_p, install_neuronx_cc_hook, partition_id_tensor,
    )

    nc = _get_nc()
    install_neuronx_cc_hook()
    pname = nc.partition_id_tensor.name if nc.partition_id_tensor else None
    in_names, out_names, out_avals, zero_outs = [], [], [], []
    for alloc in nc.m.functions[0].allocations:
        if not isinstance(alloc, mybir.MemoryLocationSet):
            continue
        name = alloc.memorylocations[0].name
        if alloc.kind == "ExternalInput":
            if name != pname:
                in_names.append(name)
        elif alloc.kind == "ExternalOutput":
            out_names.append(name)
            shape = tuple(alloc.tensor_shape)
            dtype = mybir.dt.np(alloc.dtype)
            out_avals.append(jax.core.ShapedArray(shape, dtype))
            zero_outs.append(np.zeros(shape, dtype))
    n_params = len(in_names)
    all_in_names = in_names + out_names + ([pname] if pname else [])

    def _body(*args):
        operands = list(args)
        if pname is not None:
            operands.append(partition_id_tensor())
        outs = _bass_exec_p.bind(
            *operands,
            out_avals=tuple(out_avals),
            in_names=tuple(all_in_names),
            out_names=tuple(out_names),
            lowering_input_output_aliases=(),
            sim_require_finite=True,
            sim_require_nnan=True,
            nc=nc,
        )
        return tuple(outs)

    n_outs = len(out_avals)
    devices = jax.devices()[:NCORES]
    mesh = Mesh(np.asarray(devices), ("core",))
    sharded = jax.jit(
        shard_map(_body, mesh=mesh,
                  in_specs=(PartitionSpec("core"),) * (n_params + n_outs),
                  out_specs=(PartitionSpec("core"),) * n_outs,
                  check_rep=False),
        donate_argnums=tuple(range(n_params, n_params + n_outs)),
        keep_unused=True,
    )
    sh = NamedSharding(mesh, PartitionSpec("core"))
    _NC_CACHE["exec"] = (sharded, in_names, out_names, zero_outs, sh)
    return _NC_CACHE["exec"]


def _prep_inputs(x, ln1_scale, wq, wk, wv, wo_attn, ln2_scale, wi, wo_mlp, rel_emb):
    x = np.asarray(x, np.float32)
    ln1 = np.asarray(ln1_scale, np.float32)[:, None]
    ln2 = np.asarray(ln2_scale, np.float32)[:, None]
    wq_h = (np.asarray(wq, np.float32) * ln1).astype(np.float32)
    wk_h = (np.asarray(wk, np.float32) * ln1).astype(np.float32)
    wv_h = (np.asarray(wv, np.float32) * ln1).astype(np.float32)
    wo_h = np.asarray(wo_attn, np.float32).astype(BF16NP)
    wi_eff = np.asarray(wi, np.float32) * ln2
    if WI_FP8:
        wi8_h = wi_eff.astype(F8NP)
        wil_h = (wi_eff - wi8_h.astype(np.float32)).astype(F8NP)
    else:
        wi8_h = wi_eff.astype(BF16NP)
        wil_h = None
    wm_h = np.asarray(wo_mlp, np.float32).astype(BF16NP)
    bias3, cexp = _bias_data(np.asarray(rel_emb, np.float32))
    shared = {
        "wq": wq_h, "wk": wk_h, "wv": wv_h, "wo": wo_h,
        "wi8": wi8_h, "womlp": wm_h, "bias3": bias3, "cexp": cexp,
    }
    if WI_FP8:
        shared["wil"] = wil_h
    in_maps = [dict(shared, x=np.ascontiguousarray(x[b])) for b in range(NCORES)]
    return in_maps


def kernel(x, ln1_scale, wq, wk, wv, wo_attn, ln2_scale, wi, wo_mlp, rel_emb):
    import jax
    in_maps = _prep_inputs(x, ln1_scale, wq, wk, wv, wo_attn, ln2_scale,
                           wi, wo_mlp, rel_emb)
    sharded, in_names, out_names, zero_outs, sh = _get_exec()
    concat_in = [
        jax.device_put(
            np.concatenate([in_maps[c][n] for c in range(NCORES)], axis=0), sh)
        for n in in_names
    ]
    czero = [
        jax.device_put(np.zeros((NCORES * z.shape[0], *z.shape[1:]), z.dtype), sh)
        for z in zero_outs
    ]
    outs = sharded(*concat_in, *czero)
    oidx = out_names.index("out")
    full = np.asarray(outs[oidx]).reshape(NCORES, S, D)
    return full.astype(np.float32)


# revision 21
# speedup vs baseline: 1.2369x; 1.1286x over previous
"""T5-style encoder layer (pre-LN, RMSNorm, relative-position bias) on 8 trn2
NeuronCores, data-parallel over the batch dimension (B=8 -> one batch element
per core). Each core runs the full layer for its [S, D] slice; weights and the
relative-bias data are replicated.

Self-contained: hardcodes all shapes; only depends on the runtime at
/opt/trn_rl_repo.
"""

import sys

if "/opt/trn_rl_repo" not in sys.path:
    sys.path.insert(0, "/opt/trn_rl_repo")

import numpy as np
import ml_dtypes

import concourse.bass as bass
import concourse.tile as tile
from concourse import bacc
from concourse import mybir
from concourse.masks import make_identity

# ---- problem constants -----------------------------------------------------
B, S, D = 8, 1024, 1024
H, HD = 16, 64
MLP = 4096
NUM_BUCKETS, MAX_DIST = 32, 128
EPS = 1e-6
NCORES = 8
P = 128
NS = S // P        # 8 token tiles
ND = D // P        # 8 feature tiles
NM = MLP // P      # 32 mlp tiles

F32 = mybir.dt.float32
F32R = mybir.dt.float32r
BF16 = mybir.dt.bfloat16
F8 = mybir.dt.float8e4
BF16NP = ml_dtypes.bfloat16
F8NP = ml_dtypes.float8_e4m3

import os
WI_FP8 = os.environ.get("K_WI_FP8", "1") == "1"   # 2-term fp8 DoubleRow wi
SEG_GPSIMD = os.environ.get("K_SEG_GPSIMD", "1") == "1"
ACT_DMA = os.environ.get("K_ACT_DMA", "1") == "1"
TP_SHIFT = os.environ.get("K_TP_SHIFT", "1") == "1"

AF = mybir.ActivationFunctionType
DR = mybir.MatmulPerfMode.DoubleRow


# ---- host-side relative position bias --------------------------------------
def _rel_pos_bucket_np(rel):
    # mirrors t5x _relative_position_bucket (bidirectional), numpy fp32
    n = -rel
    num_buckets = NUM_BUCKETS // 2          # 16
    ret = (n < 0).astype(np.int32) * num_buckets
    n = np.abs(n)
    max_exact = num_buckets // 2            # 8
    is_small = n < max_exact
    val_if_large = max_exact + (
        np.log(n.astype(np.float32) / max_exact + np.finfo(np.float32).eps)
        / np.log(MAX_DIST / max_exact)
        * (num_buckets - max_exact)
    ).astype(np.int32)
    val_if_large = np.minimum(val_if_large, num_buckets - 1)
    return ret + np.where(is_small, n, val_if_large)


def _bias_data(rel_emb):
    """Compressed exp(bias) data.

    Returns (bias3, cexp):
      bias3 [H, 128, 3, 128] bf16: block d (m = 1-d = k_tile - q_tile) at
        [p, c] = exp(bias[k, q]) for k = k_tile*128 + p, q = q_tile*128 + c,
        i.e. exp(table[k - q]) with k - q = m*128 + p - c.
      cexp  [1, 2H] f32: per head, exp of the two saturated buckets:
        [2h]   = exp(emb[31, h])  (k - q >= 128, q_tile <= k_tile - 2)
        [2h+1] = exp(emb[15, h])  (k - q <= -128, q_tile >= k_tile + 2)
    """
    rel = np.arange(-(S - 1), S, dtype=np.int32)          # k - q in [-1023, 1023]
    buckets = _rel_pos_bucket_np(rel)                     # [2047]
    table = rel_emb[buckets, :].astype(np.float32)        # [2047, H]
    pp = np.arange(P)[:, None, None]
    dd = np.arange(3)[None, :, None]
    cc = np.arange(P)[None, None, :]
    idx = 1023 + (1 - dd) * P + pp - cc                   # [128, 3, 128]
    blocks = np.exp(table[idx])                           # [128, 3, 128, H]
    bias3 = np.ascontiguousarray(blocks.transpose(3, 0, 1, 2)).astype(BF16NP)
    cexp = np.empty((1, 2 * H), np.float32)
    cexp[0, 0::2] = np.exp(rel_emb[31, :].astype(np.float32))
    cexp[0, 1::2] = np.exp(rel_emb[15, :].astype(np.float32))
    return bias3, cexp


# ---- device kernel ---------------------------------------------------------
def build_nc():
    nc = bacc.Bacc(None, target_bir_lowering=False)

    x_d = nc.declare_dram_parameter("x", [S, D], F32, isOutput=False)
    wq_d = nc.declare_dram_parameter("wq", [D, H * HD], F32R, isOutput=False)
    wk_d = nc.declare_dram_parameter("wk", [D, H * HD], F32R, isOutput=False)
    wv_d = nc.declare_dram_parameter("wv", [D, H * HD], F32R, isOutput=False)
    wo_d = nc.declare_dram_parameter("wo", [H * HD, D], BF16, isOutput=False)
    if WI_FP8:
        wi8_d = nc.declare_dram_parameter("wi8", [D, MLP], F8, isOutput=False)
        wil_d = nc.declare_dram_parameter("wil", [D, MLP], F8, isOutput=False)
    else:
        wi8_d = nc.declare_dram_parameter("wi8", [D, MLP], BF16, isOutput=False)
        wil_d = None
    wm_d = nc.declare_dram_parameter("womlp", [MLP, D], BF16, isOutput=False)
    bias_d = nc.declare_dram_parameter("bias3", [H, P, 3, P], BF16, isOutput=False)
    cexp_d = nc.declare_dram_parameter("cexp", [1, 2 * H], F32, isOutput=False)
    out_d = nc.declare_dram_parameter("out", [S, D], F32, isOutput=True)

    wo_t = wo_d.ap().rearrange("(hp p) d -> p hp d", p=P)
    wq_t = wq_d.ap().rearrange("(di p) m -> p di m", p=P)
    wk_t = wk_d.ap().rearrange("(di p) m -> p di m", p=P)
    wv_t = wv_d.ap().rearrange("(di p) m -> p di m", p=P)
    wi8_t = wi8_d.ap().rearrange("(di p) m -> p di m", p=P)
    wil_t = wil_d.ap().rearrange("(di p) m -> p di m", p=P) if WI_FP8 else None
    wm_t = wm_d.ap().rearrange("(ci p) d -> p ci d", p=P)
    bias_t = bias_d.ap().rearrange("h p d c -> p h d c")

    with tile.TileContext(nc) as tc:
        _body(nc, tc, x_d, wq_t, wk_t, wv_t, wo_t, wi8_t, wil_t, wm_t,
              bias_t, cexp_d, out_d)
    nc.finalize()
    return nc


def _rms_factor(nc, nrm, src_ap, sq_tile, eps_t):
    """rstd [P,1] = rsqrt(mean(src^2) + eps); sq_tile is scratch."""
    var = nrm.tile([P, 1], F32, tag="var")
    nc.vector.tensor_mul(out=sq_tile, in0=src_ap, in1=src_ap)
    nc.vector.reduce_sum(out=var, in_=sq_tile, axis=mybir.AxisListType.X)
    sd = nrm.tile([P, 1], F32, tag="sd")
    nc.scalar.activation(out=sd, in_=var, func=AF.Sqrt,
                         bias=eps_t[:, :], scale=1.0 / D)
    rstd = nrm.tile([P, 1], F32, tag="rstd")
    nc.vector.reciprocal(out=rstd, in_=sd)
    return rstd


def _body(nc, tc, x_d, wq_t, wk_t, wv_t, wo_t, wi8_t, wil_t, wm_t,
          bias_t, cexp_d, out_d):
    ALU = mybir.AluOpType

    # ---- persistent small data ---------------------------------------------
    singles = tc.alloc_tile_pool(name="singles", bufs=1)
    ident16 = singles.tile([P, P], BF16)
    make_identity(nc, ident16)
    ident32 = singles.tile([P, P], F32)
    make_identity(nc, ident32)
    ident32r = singles.tile([P, P], F32R)
    nc.vector.tensor_copy(out=ident32r, in_=ident32)
    eps_t = singles.tile([P, 1], F32)
    nc.vector.memset(eps_t, EPS)
    cexp_sb = singles.tile([P, 2 * H], F32)
    bias_sb = singles.tile([P, H, 3, P], BF16)

    # activations that live through the attention block
    qkv_act = tc.alloc_tile_pool(name="qkv_act", bufs=1)
    hT = qkv_act.tile([P, ND, S], F32R)      # rmsnorm(x)^T  [d, s]
    qT = qkv_act.tile([P, ND, S], F32R)      # q^T  [m, s] per head-pair tile
    kT = qkv_act.tile([P, ND, S], F32R)
    v_ext = qkv_act.tile([P, NS, H, HD + 1], BF16)  # [tok, stile, h, hd|1]
    nc.vector.memset(v_ext[:, :, :, HD:HD + 1], 1.0)
    # right-side stack: out1 (lives to the end) below attnT (dies after wo)
    out1_pool = tc.alloc_tile_pool(name="out1_pool", bufs=1, side="right")
    out1 = out1_pool.tile([P, NS, D], F32)    # x + attn_out, token-major
    attnT_pool = tc.alloc_tile_pool(name="attnT_pool", bufs=1, side="right")
    attnT = attnT_pool.tile([P, H // 2, S], BF16)   # heads packed 2/tile

    # ---- stage 1 + 2a interleaved: rmsnorm/transpose + V projection --------
    # emission order puts x tiles first on the SP queue, then wv chunks (split
    # per di pair for fast first-use), then bias/cexp (needed only at attn).
    with tc.tile_pool(name="xs1", bufs=2) as xs1, \
         tc.tile_pool(name="sq1", bufs=3) as sq1, \
         tc.tile_pool(name="nrm1", bufs=4) as nrm1, \
         tc.tile_pool(name="wvp", bufs=3) as wvp, \
         tc.tile_pool(name="st1ps", bufs=2, space="PSUM") as st1ps, \
         tc.tile_pool(name="vps", bufs=2, space="PSUM") as vps:
        w_sbs = []

        def emit_wv_dma(quarter):
            # wv columns for heads [4q, 4q+4), split in two [P, 4, 256] tiles
            pair = []
            for dh_ in range(2):
                w_sb = wvp.tile([P, 4, 256], F32R, tag="wv",
                                name=f"wv{quarter}_{dh_}")
                nc.sync.dma_start(
                    out=w_sb,
                    in_=wv_t[:, 4 * dh_:4 * dh_ + 4,
                             quarter * 256:(quarter + 1) * 256])
                pair.append(w_sb)
            w_sbs.append(pair)

        def emit_v(quarter, ci):
            ps = vps.tile([P, 256], F32, space="PSUM", tag="vps")
            for di in range(ND):
                nc.tensor.matmul(
                    ps[:, :],
                    hT[:, di, ci * P:(ci + 1) * P],
                    w_sbs[quarter][di // 4][:, di % 4, :],
                    start=(di == 0), stop=(di == ND - 1),
                )
            nc.scalar.copy(
                out=v_ext[:, ci, quarter * 4:quarter * 4 + 4, 0:HD],
                in_=ps[:, :].rearrange("p (h e) -> p h e", e=HD),
            )

        for si in range(NS):
            xt = xs1.tile([P, D], F32, tag="x")
            nc.sync.dma_start(out=xt, in_=x_d.ap()[si * P:(si + 1) * P, :])
            if si == 2:
                emit_wv_dma(0)
            if si == 4:
                emit_wv_dma(1)
            if si == 6:
                nc.sync.dma_start(out=cexp_sb,
                                  in_=cexp_d.ap()[0:1, :].broadcast_to((P, 2 * H)))
                nc.sync.dma_start(out=bias_sb, in_=bias_t)
            sq = sq1.tile([P, D], F32R, tag="sq")
            rstd = _rms_factor(nc, nrm1, xt[:, :], sq, eps_t)
            ht = sq1.tile([P, D], F32R, tag="sq", name=f"ht{si}")
            nc.scalar.activation(out=ht, in_=xt, func=AF.Copy,
                                 bias=0.0, scale=rstd[:, :])
            tp = st1ps.tile([P, ND, P], F32R, space="PSUM", tag="tp1")
            for di in range(ND):
                nc.tensor.transpose(tp[:, di, :], ht[:, di * P:(di + 1) * P],
                                    ident32r[:, :])
            nc.scalar.copy(out=hT[:, :, si * P:(si + 1) * P], in_=tp[:, :, :])
            # v for already-transposed token tiles fills PE while rms runs
            if si >= 4:
                emit_v(0, si - 4)
        for ci in range(4, NS):
            emit_v(0, ci)
        for quarter in range(1, 4):
            if quarter >= 2:
                emit_wv_dma(quarter)
            for ci in range(NS):
                emit_v(quarter, ci)

    # ---- stage 2b/3: QK projection interleaved with per-head attention -----
    with tc.tile_pool(name="wqkp", bufs=4) as wqkp, \
         tc.tile_pool(name="qkp", bufs=1, space="PSUM") as qkp, \
         tc.tile_pool(name="lgp", bufs=2, space="PSUM") as lgp, \
         tc.tile_pool(name="avp", bufs=1, space="PSUM") as avp, \
         tc.tile_pool(name="tpp", bufs=1, space="PSUM") as tpp, \
         tc.tile_pool(name="exq", bufs=2) as exq, \
         tc.tile_pool(name="wexpp", bufs=3) as wexpp, \
         tc.tile_pool(name="rdp", bufs=4) as rdp, \
         tc.tile_pool(name="avsbp", bufs=2) as avsbp:

        def qk_project(hm):
            for (w_ap, dstT, nm) in ((wq_t, qT, "q"), (wk_t, kT, "k")):
                w_sb = wqkp.tile([P, ND, P], F32R, tag="w", name=f"w{nm}{hm}")
                nc.sync.dma_start(out=w_sb, in_=w_ap[:, :, hm * P:(hm + 1) * P])
                ps = qkp.tile([P, 2, 256], F32, space="PSUM", tag="qk",
                              name=f"qk{nm}{hm}")
                for qt in range(4):
                    sl = ps[:, qt % 2, :]
                    for di in range(ND):
                        nc.tensor.matmul(
                            sl,
                            w_sb[:, di, :],
                            hT[:, di, qt * 256:(qt + 1) * 256],
                            start=(di == 0), stop=(di == ND - 1),
                        )
                    if qt % 2 == 1:
                        nc.vector.tensor_copy(
                            out=dstT[:, hm, (qt - 1) * 256:(qt + 1) * 256],
                            in_=ps[:, :, :])

        def attn_head(h):
            hm, hb = h // 2, HD * (h % 2)
            av = avp.tile([P, NS, P], F32, space="PSUM", tag="av")
            for ki in range(NS):
                lg = lgp.tile([P, S], F32, space="PSUM", tag="lg")
                for qh in range(2):
                    nc.tensor.matmul(
                        lg[:, qh * 512:(qh + 1) * 512],
                        kT[hb:hb + HD, hm, ki * P:(ki + 1) * P],
                        qT[hb:hb + HD, hm, qh * 512:(qh + 1) * 512],
                        start=True, stop=True,
                    )
                ex = exq.tile([P, S], BF16, tag="ex")
                nc.scalar.activation(out=ex, in_=lg[:, :], func=AF.Exp)
                wexp = wexpp.tile([P, S], BF16, tag="wexp")
                # middle (varying-bias) window: q tiles [ki-1, ki+1]
                q0, q1 = max(ki - 1, 0), min(ki + 2, NS)
                d0 = q0 - (ki - 1)
                nc.vector.tensor_mul(
                    out=wexp[:, q0 * P:q1 * P].rearrange("p (c w) -> p c w", w=P),
                    in0=ex[:, q0 * P:q1 * P].rearrange("p (c w) -> p c w", w=P),
                    in1=bias_sb[:, h, d0:d0 + (q1 - q0), :],
                )
                # constant-bias segments (saturated buckets)
                segs = []
                if q0 > 0:
                    segs.append((0, q0 * P, cexp_sb[:, 2 * h:2 * h + 1]))
                if q1 < NS:
                    segs.append((q1 * P, S, cexp_sb[:, 2 * h + 1:2 * h + 2]))
                segs.sort(key=lambda t: t[1] - t[0])
                for i, (a, b, c_ap) in enumerate(segs):
                    eng = nc.gpsimd if (SEG_GPSIMD and i == len(segs) - 1) else nc.vector
                    eng.tensor_scalar_mul(out=wexp[:, a:b], in0=ex[:, a:b],
                                          scalar1=c_ap)
                for qi in range(NS):
                    # one accumulation group per PSUM bank (4 qi slots/bank):
                    # start pending-zeroes the whole bank, so only the first
                    # matmul in each bank starts and the last stops.
                    nc.tensor.matmul(
                        av[:, qi, 0:HD + 1],
                        wexp[:, qi * P:(qi + 1) * P],
                        v_ext[:, ki, h, :],
                        start=(ki == 0 and qi % 4 == 0),
                        stop=(ki == NS - 1 and qi % 4 == 3),
                    )
            # drain: per q tile normalize (fp32 psum * rden) -> bf16, transpose
            av_sb = avsbp.tile([P, NS, HD], BF16, tag="avsb")
            tp = tpp.tile([P, NS, P], BF16, space="PSUM", tag="tp")
            for qi in range(NS):
                rden = rdp.tile([P, 1], F32, tag="rd")
                nc.vector.reciprocal(out=rden, in_=av[:, qi, HD:HD + 1])
                nc.vector.tensor_scalar_mul(out=av_sb[:, qi, :],
                                            in0=av[:, qi, 0:HD],
                                            scalar1=rden)
                tb = hb if TP_SHIFT else 0
                nc.tensor.transpose(tp[tb:tb + HD, qi, :], av_sb[:, qi, :],
                                    ident16[:, :])
            tb = hb if TP_SHIFT else 0
            nc.vector.tensor_copy(out=attnT[hb:hb + HD, hm, :],
                                  in_=tp[tb:tb + HD, :, :])

        for hm in range(ND):
            qk_project(hm)
            attn_head(2 * hm)
            attn_head(2 * hm + 1)

    # ---- stage 4/5: attn @ wo + residual, rmsnorm -> h2T --------------------
    qkv_act.release()
    yT_pool = tc.alloc_tile_pool(name="yT_pool", bufs=1)
    yT = yT_pool.tile([P, NM, S], BF16)
    h2T_pool = tc.alloc_tile_pool(name="h2T_pool", bufs=1)
    h2T = h2T_pool.tile([P, ND, S], F8 if WI_FP8 else BF16)

    with tc.tile_pool(name="wop", bufs=1) as wop, \
         tc.tile_pool(name="xs4", bufs=3) as xs4, \
         tc.tile_pool(name="ops", bufs=2, space="PSUM") as ops, \
         tc.tile_pool(name="tp5", bufs=2, space="PSUM") as tp5, \
         tc.tile_pool(name="sq5", bufs=2) as sq5, \
         tc.tile_pool(name="nrm5", bufs=4) as nrm5, \
         tc.tile_pool(name="h2p", bufs=2) as h2p:
        wo_sb = wop.tile([P, H // 2, D], BF16)
        for si in range(NS):
            xt = xs4.tile([P, D], F32, tag="x")
            nc.sync.dma_start(out=xt, in_=x_d.ap()[si * P:(si + 1) * P, :])
            if si == 0:
                for hq in range(4):
                    nc.sync.dma_start(out=wo_sb[:, 2 * hq:2 * hq + 2, :],
                                      in_=wo_t[:, 2 * hq:2 * hq + 2, :])
            ps = ops.tile([P, D], F32, space="PSUM", tag="wo")
            for hp in range(H // 2):
                for dh in range(2):
                    nc.tensor.matmul(
                        ps[:, dh * 512:(dh + 1) * 512],
                        attnT[:, hp, si * P:(si + 1) * P],
                        wo_sb[:, hp, dh * 512:(dh + 1) * 512],
                        start=(hp == 0), stop=(hp == H // 2 - 1),
                    )
            nc.vector.tensor_add(out=out1[:, si, :], in0=ps[:, :], in1=xt[:, :])
            # stage 5 per si: rmsnorm -> h2 (bf16) -> transpose -> h2T
            sq = sq5.tile([P, D], F32, tag="sq")
            rstd = _rms_factor(nc, nrm5, out1[:, si, :], sq, eps_t)
            h2 = h2p.tile([P, D], BF16, tag="h2")
            nc.scalar.activation(out=h2, in_=out1[:, si, :], func=AF.Copy,
                                 bias=0.0, scale=rstd[:, :])
            tp = tp5.tile([P, ND, P], BF16, space="PSUM", tag="tp5")
            for di in range(ND):
                nc.tensor.transpose(tp[:, di, :], h2[:, di * P:(di + 1) * P],
                                    ident16[:, :])
            nc.vector.tensor_copy(out=h2T[:, :, si * P:(si + 1) * P], in_=tp[:, :, :])
    attnT_pool.release()

    # ---- stage 6: y^T = relu(wi^T @ h2^T) ----------------------------------
    with tc.tile_pool(name="wip", bufs=2) as wip, \
         tc.tile_pool(name="psy", bufs=2, space="PSUM") as psy:
        for eighth in range(8):
            sl = slice(eighth * (MLP // 8), (eighth + 1) * (MLP // 8))
            wi_sb = wip.tile([P, ND, MLP // 8], F8 if WI_FP8 else BF16, tag="wi8")
            wi_eng = nc.scalar if ACT_DMA else nc.sync
            wi_eng.dma_start(out=wi_sb, in_=wi8_t[:, :, sl])
            if WI_FP8:
                wil_sb = wip.tile([P, ND, MLP // 8], F8, tag="wil")
                wi_eng.dma_start(out=wil_sb, in_=wil_t[:, :, sl])
            for mj in range(NM // 8):
                m0 = eighth * (NM // 8) + mj
                ps = psy.tile([P, S], F32, space="PSUM", tag="y")
                if WI_FP8:
                    for sh in range(2):
                        for ti, term in enumerate((wi_sb, wil_sb)):
                            for pr in range(4):
                                nc.tensor.matmul(
                                    ps[:, sh * 512:(sh + 1) * 512],
                                    term[:, 2 * pr:2 * pr + 2, mj * P:(mj + 1) * P],
                                    h2T[:, 2 * pr:2 * pr + 2, sh * 512:(sh + 1) * 512],
                                    start=(ti == 0 and pr == 0),
                                    stop=(ti == 1 and pr == 3),
                                    perf_mode=DR,
                                )
                else:
                    for sh in range(2):
                        for di in range(ND):
                            nc.tensor.matmul(
                                ps[:, sh * 512:(sh + 1) * 512],
                                wi_sb[:, di, mj * P:(mj + 1) * P],
                                h2T[:, di, sh * 512:(sh + 1) * 512],
                                start=(di == 0), stop=(di == ND - 1),
                            )
                nc.scalar.activation(out=yT[:, m0, :], in_=ps[:, :], func=AF.Relu)
    h2T_pool.release()

    # ---- stage 7: out = out1 + y^T.T @ womlp -------------------------------
    # 4 groups (dh half of D x sg half of tokens); womlp streamed per group,
    # two groups of PSUM banks so group g+1 computes while g drains.
    with tc.tile_pool(name="wmp", bufs=3) as wmp, \
         tc.tile_pool(name="oop", bufs=4) as oop, \
         tc.tile_pool(name="o2ps", bufs=8, space="PSUM") as o2ps:
        for dh in range(2):
            for sg in range(2):
                pss = [o2ps.tile([P, 512], F32, space="PSUM", tag="o2",
                                 name=f"o2_{dh}_{sg}_{i}") for i in range(4)]
                for cg in range(NM // 4):
                    wmc = wmp.tile([P, 4, 512], BF16, tag="wm")
                    dma_eng = nc.scalar if ACT_DMA else nc.sync
                    dma_eng.dma_start(
                        out=wmc,
                        in_=wm_t[:, 4 * cg:4 * cg + 4, dh * 512:(dh + 1) * 512])
                    for cj in range(4):
                        ci = 4 * cg + cj
                        for i4 in range(4):
                            si = sg * 4 + i4
                            nc.tensor.matmul(
                                pss[i4][:, :],
                                yT[:, ci, si * P:(si + 1) * P],
                                wmc[:, cj, :],
                                start=(ci == 0), stop=(ci == NM - 1),
                            )
                for i4 in range(4):
                    si = sg * 4 + i4
                    oo = oop.tile([P, 512], F32, tag="oo")
                    nc.vector.tensor_add(out=oo, in0=pss[i4][:, :],
                                         in1=out1[:, si, dh * 512:(dh + 1) * 512])
                    nc.sync.dma_start(
                        out=out_d.ap()[si * P:(si + 1) * P, dh * 512:(dh + 1) * 512],
                        in_=oo)

    yT_pool.release()
    out1_pool.release()
    singles.release()


# ---- host wrapper ----------------------------------------------------------
_NC_CACHE = {}


def _get_nc():
    if "nc" not in _NC_CACHE:
        _NC_CACHE["nc"] = build_nc()
    return _NC_CACHE["nc"]


def _get_exec():
    """Compile once: a sharded PJRT executable over the 8 NeuronCores."""
    if "exec" in _NC_CACHE:
        return _NC_CACHE["exec"]
    import jax
    from jax.sharding import Mesh, PartitionSpec, NamedSharding
    from jax.experimental.shard_map import shard_map
    from concourse.bass2jax import (
        _bass_exec_p, install_neuronx_cc_hook, partition_id_tensor,
    )

    nc = _get_nc()
    install_neuronx_cc_hook()
    pname = nc.partition_id_tensor.name if nc.partition_id_tensor else None
    in_names, out_names, out_avals, zero_outs = [], [], [], []
    for alloc in nc.m.functions[0].allocations:
        if not isinstance(alloc, mybir.MemoryLocationSet):
            continue
        name = alloc.memorylocations[0].name
        if alloc.kind == "ExternalInput":
            if name != pname:
                in_names.append(name)
        elif alloc.kind == "ExternalOutput":
            out_names.append(name)
            shape = tuple(alloc.tensor_shape)
            dtype = mybir.dt.np(alloc.dtype)
            out_avals.append(jax.core.ShapedArray(shape, dtype))
            zero_outs.append(np.zeros(shape, dtype))
    n_params = len(in_names)
    all_in_names = in_names + out_names + ([pname] if pname else [])

    def _body(*args):
        operands = list(args)
        if pname is not None:
            operands.append(partition_id_tensor())
        outs = _bass_exec_p.bind(
            *operands,
            out_avals=tuple(out_avals),
            in_names=tuple(all_in_names),
            out_names=tuple(out_names),
            lowering_input_output_aliases=(),
            sim_require_finite=True,
            sim_require_nnan=True,
            nc=nc,
        )
        return tuple(outs)

    n_outs = len(out_avals)
    devices = jax.devices()[:NCORES]
    mesh = Mesh(np.asarray(devices), ("core",))
    sharded = jax.jit(
        shard_map(_body, mesh=mesh,
                  in_specs=(PartitionSpec("core"),) * (n_params + n_outs),
                  out_specs=(PartitionSpec("core"),) * n_outs,
                  check_rep=False),
        donate_argnums=tuple(range(n_params, n_params + n_outs)),
        keep_unused=True,
    )
    sh = NamedSharding(mesh, PartitionSpec("core"))
    _NC_CACHE["exec"] = (sharded, in_names, out_names, zero_outs, sh)
    return _NC_CACHE["exec"]


def _prep_inputs(x, ln1_scale, wq, wk, wv, wo_attn, ln2_scale, wi, wo_mlp, rel_emb):
    x = np.asarray(x, np.float32)
    ln1 = np.asarray(ln1_scale, np.float32)[:, None]
    ln2 = np.asarray(ln2_scale, np.float32)[:, None]
    wq_h = (np.asarray(wq, np.float32) * ln1).astype(np.float32)
    wk_h = (np.asarray(wk, np.float32) * ln1).astype(np.float32)
    wv_h = (np.asarray(wv, np.float32) * ln1).astype(np.float32)
    wo_h = np.asarray(wo_attn, np.float32).astype(BF16NP)
    wi_eff = np.asarray(wi, np.float32) * ln2
    if WI_FP8:
        wi8_h = wi_eff.astype(F8NP)
        wil_h = (wi_eff - wi8_h.astype(np.float32)).astype(F8NP)
    else:
        wi8_h = wi_eff.astype(BF16NP)
        wil_h = None
    wm_h = np.asarray(wo_mlp, np.float32).astype(BF16NP)
    bias3, cexp = _bias_data(np.asarray(rel_emb, np.float32))
    shared = {
        "wq": wq_h, "wk": wk_h, "wv": wv_h, "wo": wo_h,
        "wi8": wi8_h, "womlp": wm_h, "bias3": bias3, "cexp": cexp,
    }
    if WI_FP8:
        shared["wil"] = wil_h
    in_maps = [dict(shared, x=np.ascontiguousarray(x[b])) for b in range(NCORES)]
    return in_maps


def kernel(x, ln1_scale, wq, wk, wv, wo_attn, ln2_scale, wi, wo_mlp, rel_emb):
    import jax
    in_maps = _prep_inputs(x, ln1_scale, wq, wk, wv, wo_attn, ln2_scale,
                           wi, wo_mlp, rel_emb)
    sharded, in_names, out_names, zero_outs, sh = _get_exec()
    concat_in = [
        jax.device_put(
            np.concatenate([in_maps[c][n] for c in range(NCORES)], axis=0), sh)
        for n in in_names
    ]
    czero = [
        jax.device_put(np.zeros((NCORES * z.shape[0], *z.shape[1:]), z.dtype), sh)
        for z in zero_outs
    ]
    outs = sharded(*concat_in, *czero)
    oidx = out_names.index("out")
    full = np.asarray(outs[oidx]).reshape(NCORES, S, D)
    return full.astype(np.float32)


# revision 25
# speedup vs baseline: 1.2759x; 1.0316x over previous
"""T5-style encoder layer (pre-LN, RMSNorm, relative-position bias) on 8 trn2
NeuronCores, data-parallel over the batch dimension (B=8 -> one batch element
per core). Each core runs the full layer for its [S, D] slice; weights and the
relative-bias data are replicated.

Self-contained: hardcodes all shapes; only depends on the runtime at
/opt/trn_rl_repo.
"""

import sys

if "/opt/trn_rl_repo" not in sys.path:
    sys.path.insert(0, "/opt/trn_rl_repo")

import numpy as np
import ml_dtypes

import concourse.bass as bass
import concourse.tile as tile
from concourse import bacc
from concourse import mybir
from concourse.masks import make_identity

# ---- problem constants -----------------------------------------------------
B, S, D = 8, 1024, 1024
H, HD = 16, 64
MLP = 4096
NUM_BUCKETS, MAX_DIST = 32, 128
EPS = 1e-6
NCORES = 8
P = 128
NS = S // P        # 8 token tiles
ND = D // P        # 8 feature tiles
NM = MLP // P      # 32 mlp tiles

F32 = mybir.dt.float32
F32R = mybir.dt.float32r
BF16 = mybir.dt.bfloat16
F8 = mybir.dt.float8e4
BF16NP = ml_dtypes.bfloat16
F8NP = ml_dtypes.float8_e4m3

import os
WI_FP8 = os.environ.get("K_WI_FP8", "1") == "1"   # 2-term fp8 DoubleRow wi
SEG_GPSIMD = os.environ.get("K_SEG_GPSIMD", "1") == "1"
ACT_DMA = os.environ.get("K_ACT_DMA", "1") == "1"
TP_SHIFT = os.environ.get("K_TP_SHIFT", "1") == "1"

AF = mybir.ActivationFunctionType
DR = mybir.MatmulPerfMode.DoubleRow


# ---- host-side relative position bias --------------------------------------
def _rel_pos_bucket_np(rel):
    # mirrors t5x _relative_position_bucket (bidirectional), numpy fp32
    n = -rel
    num_buckets = NUM_BUCKETS // 2          # 16
    ret = (n < 0).astype(np.int32) * num_buckets
    n = np.abs(n)
    max_exact = num_buckets // 2            # 8
    is_small = n < max_exact
    val_if_large = max_exact + (
        np.log(n.astype(np.float32) / max_exact + np.finfo(np.float32).eps)
        / np.log(MAX_DIST / max_exact)
        * (num_buckets - max_exact)
    ).astype(np.int32)
    val_if_large = np.minimum(val_if_large, num_buckets - 1)
    return ret + np.where(is_small, n, val_if_large)


def _bias_data(rel_emb):
    """Compressed exp(bias) data.

    Returns (bias3, cexp):
      bias3 [H, 128, 3, 128] bf16: block d (m = 1-d = k_tile - q_tile) at
        [p, c] = exp(bias[k, q]) for k = k_tile*128 + p, q = q_tile*128 + c,
        i.e. exp(table[k - q]) with k - q = m*128 + p - c.
      cexp  [1, 2H] f32: per head, exp of the two saturated buckets:
        [2h]   = exp(emb[31, h])  (k - q >= 128, q_tile <= k_tile - 2)
        [2h+1] = exp(emb[15, h])  (k - q <= -128, q_tile >= k_tile + 2)
    """
    rel = np.arange(-(S - 1), S, dtype=np.int32)          # k - q in [-1023, 1023]
    buckets = _rel_pos_bucket_np(rel)                     # [2047]
    table = rel_emb[buckets, :].astype(np.float32)        # [2047, H]
    pp = np.arange(P)[:, None, None]
    dd = np.arange(3)[None, :, None]
    cc = np.arange(P)[None, None, :]
    idx = 1023 + (1 - dd) * P + pp - cc                   # [128, 3, 128]
    blocks = np.exp(table[idx])                           # [128, 3, 128, H]
    bias3 = np.ascontiguousarray(blocks.transpose(3, 0, 1, 2)).astype(BF16NP)
    cexp = np.empty((1, 2 * H), np.float32)
    cexp[0, 0::2] = np.exp(rel_emb[31, :].astype(np.float32))
    cexp[0, 1::2] = np.exp(rel_emb[15, :].astype(np.float32))
    return bias3, cexp


# ---- device kernel ---------------------------------------------------------
def build_nc():
    nc = bacc.Bacc(None, target_bir_lowering=False)

    x_d = nc.declare_dram_parameter("x", [S, D], F32, isOutput=False)
    wq_d = nc.declare_dram_parameter("wq", [D, H * HD], F32R, isOutput=False)
    wk_d = nc.declare_dram_parameter("wk", [D, H * HD], F32R, isOutput=False)
    wv_d = nc.declare_dram_parameter("wv", [D, H * HD], F32R, isOutput=False)
    wo_d = nc.declare_dram_parameter("wo", [H * HD, D], BF16, isOutput=False)
    if WI_FP8:
        wi8_d = nc.declare_dram_parameter("wi8", [D, MLP], F8, isOutput=False)
        wil_d = nc.declare_dram_parameter("wil", [D, MLP], F8, isOutput=False)
    else:
        wi8_d = nc.declare_dram_parameter("wi8", [D, MLP], BF16, isOutput=False)
        wil_d = None
    wm_d = nc.declare_dram_parameter("womlp", [MLP, D], BF16, isOutput=False)
    bias_d = nc.declare_dram_parameter("bias3", [H, P, 3, P], BF16, isOutput=False)
    cexp_d = nc.declare_dram_parameter("cexp", [1, 2 * H], F32, isOutput=False)
    out_d = nc.declare_dram_parameter("out", [S, D], F32, isOutput=True)

    wo_t = wo_d.ap().rearrange("(hp p) d -> p hp d", p=P)
    wq_t = wq_d.ap().rearrange("(di p) m -> p di m", p=P)
    wk_t = wk_d.ap().rearrange("(di p) m -> p di m", p=P)
    wv_t = wv_d.ap().rearrange("(di p) m -> p di m", p=P)
    wi8_t = wi8_d.ap().rearrange("(di p) m -> p di m", p=P)
    wil_t = wil_d.ap().rearrange("(di p) m -> p di m", p=P) if WI_FP8 else None
    wm_t = wm_d.ap().rearrange("(ci p) d -> p ci d", p=P)
    bias_t = bias_d.ap().rearrange("h p d c -> p h d c")

    with tile.TileContext(nc) as tc:
        _body(nc, tc, x_d, wq_t, wk_t, wv_t, wo_t, wi8_t, wil_t, wm_t,
              bias_t, cexp_d, out_d)
    nc.finalize()
    return nc


def _rms_factor(nc, nrm, src_ap, sq_tile, eps_t):
    """rstd [P,1] = rsqrt(mean(src^2) + eps); sq_tile is scratch.

    Square+accum on ACT does the elementwise square and the free-axis sum
    in one pass; the rsqrt split (sqrt on ACT, reciprocal on DVE) follows
    the bass guidance (ACT Rsqrt is inaccurate).
    """
    var = nrm.tile([P, 1], F32, tag="var")
    nc.scalar.activation(out=sq_tile, in_=src_ap, func=AF.Square,
                         accum_out=var)
    sd = nrm.tile([P, 1], F32, tag="sd")
    nc.scalar.activation(out=sd, in_=var, func=AF.Sqrt,
                         bias=eps_t[:, :], scale=1.0 / D)
    rstd = nrm.tile([P, 1], F32, tag="rstd")
    nc.vector.reciprocal(out=rstd, in_=sd)
    return rstd


def _body(nc, tc, x_d, wq_t, wk_t, wv_t, wo_t, wi8_t, wil_t, wm_t,
          bias_t, cexp_d, out_d):
    ALU = mybir.AluOpType

    # ---- persistent small data ---------------------------------------------
    singles = tc.alloc_tile_pool(name="singles", bufs=1)
    ident16 = singles.tile([P, P], BF16)
    make_identity(nc, ident16)
    ident32 = singles.tile([P, P], F32)
    make_identity(nc, ident32)
    ident32r = singles.tile([P, P], F32R)
    nc.vector.tensor_copy(out=ident32r, in_=ident32)
    eps_t = singles.tile([P, 1], F32)
    nc.vector.memset(eps_t, EPS)
    cexp_sb = singles.tile([P, 2 * H], F32)
    bias_sb = singles.tile([P, H, 3, P], BF16)

    # activations that live through the attention block
    qkv_act = tc.alloc_tile_pool(name="qkv_act", bufs=1)
    hT = qkv_act.tile([P, ND, S], F32R)      # rmsnorm(x)^T  [d, s]
    qT = qkv_act.tile([P, ND, S], F32R)      # q^T  [m, s] per head-pair tile
    kT = qkv_act.tile([P, ND, S], F32R)
    v_ext = qkv_act.tile([P, NS, H, HD + 1], BF16)  # [tok, stile, h, hd|1]
    nc.vector.memset(v_ext[:, :, :, HD:HD + 1], 1.0)
    # right-side stack: out1 (lives to the end) below attnT (dies after wo)
    out1_pool = tc.alloc_tile_pool(name="out1_pool", bufs=1, side="right")
    out1 = out1_pool.tile([P, NS, D], F32)    # x + attn_out, token-major
    attnT_pool = tc.alloc_tile_pool(name="attnT_pool", bufs=1, side="right")
    attnT = attnT_pool.tile([P, H // 2, S], BF16)   # heads packed 2/tile

    # ---- stage 1 + 2a interleaved: rmsnorm/transpose + V projection --------
    # emission order puts x tiles first on the SP queue, then wv chunks (split
    # per di pair for fast first-use), then bias/cexp (needed only at attn).
    with tc.tile_pool(name="xs1", bufs=2) as xs1, \
         tc.tile_pool(name="sq1", bufs=2) as sq1, \
         tc.tile_pool(name="nrm1", bufs=4) as nrm1, \
         tc.tile_pool(name="wvp", bufs=4) as wvp, \
         tc.tile_pool(name="st1ps", bufs=2, space="PSUM") as st1ps, \
         tc.tile_pool(name="vps", bufs=2, space="PSUM") as vps:
        w_sbs = []

        def emit_wv_dma(quarter):
            # wv columns for heads [4q, 4q+4), split in two [P, 4, 256] tiles
            pair = []
            for dh_ in range(2):
                w_sb = wvp.tile([P, 4, 256], F32R, tag="wv",
                                name=f"wv{quarter}_{dh_}")
                nc.sync.dma_start(
                    out=w_sb,
                    in_=wv_t[:, 4 * dh_:4 * dh_ + 4,
                             quarter * 256:(quarter + 1) * 256])
                pair.append(w_sb)
            w_sbs.append(pair)

        def emit_v(quarter, ci):
            ps = vps.tile([P, 256], F32, space="PSUM", tag="vps")
            for di in range(ND):
                nc.tensor.matmul(
                    ps[:, :],
                    hT[:, di, ci * P:(ci + 1) * P],
                    w_sbs[quarter][di // 4][:, di % 4, :],
                    start=(di == 0), stop=(di == ND - 1),
                )
            nc.scalar.copy(
                out=v_ext[:, ci, quarter * 4:quarter * 4 + 4, 0:HD],
                in_=ps[:, :].rearrange("p (h e) -> p h e", e=HD),
            )

        for si in range(NS):
            xt = xs1.tile([P, D], F32, tag="x")
            nc.sync.dma_start(out=xt, in_=x_d.ap()[si * P:(si + 1) * P, :])
            if si == 2:
                emit_wv_dma(0)
            if si == 4:
                emit_wv_dma(1)
            if si == 6:
                nc.sync.dma_start(out=cexp_sb,
                                  in_=cexp_d.ap()[0:1, :].broadcast_to((P, 2 * H)))
                nc.sync.dma_start(out=bias_sb, in_=bias_t)
            ht = sq1.tile([P, D], F32R, tag="sq", name=f"ht{si}")
            rstd = _rms_factor(nc, nrm1, xt[:, :], ht, eps_t)
            nc.vector.tensor_scalar_mul(out=ht, in0=xt, scalar1=rstd[:, :])
            tp = st1ps.tile([P, ND, P], F32R, space="PSUM", tag="tp1")
            for di in range(ND):
                nc.tensor.transpose(tp[:, di, :], ht[:, di * P:(di + 1) * P],
                                    ident32r[:, :])
            nc.scalar.copy(out=hT[:, :, si * P:(si + 1) * P], in_=tp[:, :, :])
            # v for already-transposed token tiles fills PE while rms runs
            if si >= 4:
                emit_v(0, si - 4)
                emit_v(1, si - 4)
        for ci in range(4, NS):
            emit_v(0, ci)
            emit_v(1, ci)
        for quarter in range(2, 4):
            emit_wv_dma(quarter)
            for ci in range(NS):
                emit_v(quarter, ci)

    # ---- stage 2b/3: QK projection interleaved with per-head attention -----
    with tc.tile_pool(name="wqkp", bufs=4) as wqkp, \
         tc.tile_pool(name="qkp", bufs=1, space="PSUM") as qkp, \
         tc.tile_pool(name="lgp", bufs=2, space="PSUM") as lgp, \
         tc.tile_pool(name="avp", bufs=1, space="PSUM") as avp, \
         tc.tile_pool(name="tpp", bufs=1, space="PSUM") as tpp, \
         tc.tile_pool(name="exq", bufs=2) as exq, \
         tc.tile_pool(name="wexpp", bufs=3) as wexpp, \
         tc.tile_pool(name="rdp", bufs=4) as rdp, \
         tc.tile_pool(name="avsbp", bufs=2) as avsbp:

        def qk_project(hm):
            for (w_ap, dstT, nm) in ((wq_t, qT, "q"), (wk_t, kT, "k")):
                w_sb = wqkp.tile([P, ND, P], F32R, tag="w", name=f"w{nm}{hm}")
                nc.sync.dma_start(out=w_sb, in_=w_ap[:, :, hm * P:(hm + 1) * P])
                ps = qkp.tile([P, 2, 256], F32, space="PSUM", tag="qk",
                              name=f"qk{nm}{hm}")
                for qt in range(4):
                    sl = ps[:, qt % 2, :]
                    for di in range(ND):
                        nc.tensor.matmul(
                            sl,
                            w_sb[:, di, :],
                            hT[:, di, qt * 256:(qt + 1) * 256],
                            start=(di == 0), stop=(di == ND - 1),
                        )
                    if qt % 2 == 1:
                        nc.vector.tensor_copy(
                            out=dstT[:, hm, (qt - 1) * 256:(qt + 1) * 256],
                            in_=ps[:, :, :])

        def attn_head(h):
            hm, hb = h // 2, HD * (h % 2)
            av = avp.tile([P, NS, P], F32, space="PSUM", tag="av")
            for ki in range(NS):
                lg = lgp.tile([P, S], F32, space="PSUM", tag="lg")
                for qh in range(2):
                    nc.tensor.matmul(
                        lg[:, qh * 512:(qh + 1) * 512],
                        kT[hb:hb + HD, hm, ki * P:(ki + 1) * P],
                        qT[hb:hb + HD, hm, qh * 512:(qh + 1) * 512],
                        start=True, stop=True,
                    )
                ex = exq.tile([P, S], BF16, tag="ex")
                nc.scalar.activation(out=ex, in_=lg[:, :], func=AF.Exp)
                wexp = wexpp.tile([P, S], BF16, tag="wexp")
                # middle (varying-bias) window: q tiles [ki-1, ki+1]
                q0, q1 = max(ki - 1, 0), min(ki + 2, NS)
                d0 = q0 - (ki - 1)
                nc.vector.tensor_mul(
                    out=wexp[:, q0 * P:q1 * P].rearrange("p (c w) -> p c w", w=P),
                    in0=ex[:, q0 * P:q1 * P].rearrange("p (c w) -> p c w", w=P),
                    in1=bias_sb[:, h, d0:d0 + (q1 - q0), :],
                )
                # constant-bias segments (saturated buckets)
                segs = []
                if q0 > 0:
                    segs.append((0, q0 * P, cexp_sb[:, 2 * h:2 * h + 1]))
                if q1 < NS:
                    segs.append((q1 * P, S, cexp_sb[:, 2 * h + 1:2 * h + 2]))
                segs.sort(key=lambda t: t[1] - t[0])
                for i, (a, b, c_ap) in enumerate(segs):
                    eng = nc.gpsimd if (SEG_GPSIMD and i == len(segs) - 1) else nc.vector
                    eng.tensor_scalar_mul(out=wexp[:, a:b], in0=ex[:, a:b],
                                          scalar1=c_ap)
                for qi in range(NS):
                    # one accumulation group per PSUM bank (4 qi slots/bank):
                    # start pending-zeroes the whole bank, so only the first
                    # matmul in each bank starts and the last stops.
                    nc.tensor.matmul(
                        av[:, qi, 0:HD + 1],
                        wexp[:, qi * P:(qi + 1) * P],
                        v_ext[:, ki, h, :],
                        start=(ki == 0 and qi % 4 == 0),
                        stop=(ki == NS - 1 and qi % 4 == 3),
                    )
            # drain: per q tile normalize (fp32 psum * rden) -> bf16, transpose
            av_sb = avsbp.tile([P, NS, HD], BF16, tag="avsb")
            tp = tpp.tile([P, NS, P], BF16, space="PSUM", tag="tp")
            for qi in range(NS):
                rden = rdp.tile([P, 1], F32, tag="rd")
                nc.vector.reciprocal(out=rden, in_=av[:, qi, HD:HD + 1])
                nc.vector.tensor_scalar_mul(out=av_sb[:, qi, :],
                                            in0=av[:, qi, 0:HD],
                                            scalar1=rden)
                tb = hb if TP_SHIFT else 0
                nc.tensor.transpose(tp[tb:tb + HD, qi, :], av_sb[:, qi, :],
                                    ident16[:, :])
            tb = hb if TP_SHIFT else 0
            nc.vector.tensor_copy(out=attnT[hb:hb + HD, hm, :],
                                  in_=tp[tb:tb + HD, :, :])

        for hm in range(ND):
            qk_project(hm)
            attn_head(2 * hm)
            attn_head(2 * hm + 1)

    # ---- stage 4/5: attn @ wo + residual, rmsnorm -> h2T --------------------
    qkv_act.release()
    yT_pool = tc.alloc_tile_pool(name="yT_pool", bufs=1)
    yT = yT_pool.tile([P, NM, S], BF16)
    h2T_pool = tc.alloc_tile_pool(name="h2T_pool", bufs=1)
    h2T = h2T_pool.tile([P, ND, S], F8 if WI_FP8 else BF16)

    with tc.tile_pool(name="wop", bufs=1) as wop, \
         tc.tile_pool(name="xs4", bufs=3) as xs4, \
         tc.tile_pool(name="ops", bufs=2, space="PSUM") as ops, \
         tc.tile_pool(name="tp5", bufs=2, space="PSUM") as tp5, \
         tc.tile_pool(name="sq5", bufs=2) as sq5, \
         tc.tile_pool(name="nrm5", bufs=4) as nrm5, \
         tc.tile_pool(name="h2p", bufs=2) as h2p:
        wo_sb = wop.tile([P, H // 2, D], BF16)
        for si in range(NS):
            xt = xs4.tile([P, D], F32, tag="x")
            nc.sync.dma_start(out=xt, in_=x_d.ap()[si * P:(si + 1) * P, :])
            if si == 0:
                for hq in range(4):
                    nc.sync.dma_start(out=wo_sb[:, 2 * hq:2 * hq + 2, :],
                                      in_=wo_t[:, 2 * hq:2 * hq + 2, :])
            ps = ops.tile([P, D], F32, space="PSUM", tag="wo")
            for hp in range(H // 2):
                for dh in range(2):
                    nc.tensor.matmul(
                        ps[:, dh * 512:(dh + 1) * 512],
                        attnT[:, hp, si * P:(si + 1) * P],
                        wo_sb[:, hp, dh * 512:(dh + 1) * 512],
                        start=(hp == 0), stop=(hp == H // 2 - 1),
                    )
            nc.vector.tensor_add(out=out1[:, si, :], in0=ps[:, :], in1=xt[:, :])
            # stage 5 per si: rmsnorm -> h2 (bf16) -> transpose -> h2T
            sq = sq5.tile([P, D], F32, tag="sq")
            rstd = _rms_factor(nc, nrm5, out1[:, si, :], sq, eps_t)
            h2 = h2p.tile([P, D], BF16, tag="h2")
            nc.vector.tensor_scalar_mul(out=h2, in0=out1[:, si, :],
                                        scalar1=rstd[:, :])
            tp = tp5.tile([P, ND, P], BF16, space="PSUM", tag="tp5")
            for di in range(ND):
                nc.tensor.transpose(tp[:, di, :], h2[:, di * P:(di + 1) * P],
                                    ident16[:, :])
            nc.scalar.copy(out=h2T[:, :, si * P:(si + 1) * P], in_=tp[:, :, :])
    attnT_pool.release()

    # ---- stage 6: y^T = relu(wi^T @ h2^T) ----------------------------------
    with tc.tile_pool(name="wip", bufs=2) as wip, \
         tc.tile_pool(name="psy", bufs=2, space="PSUM") as psy:
        for eighth in range(8):
            sl = slice(eighth * (MLP // 8), (eighth + 1) * (MLP // 8))
            wi_sb = wip.tile([P, ND, MLP // 8], F8 if WI_FP8 else BF16, tag="wi8")
            wi_eng = nc.scalar if ACT_DMA else nc.sync
            wi_eng.dma_start(out=wi_sb, in_=wi8_t[:, :, sl])
            if WI_FP8:
                wil_sb = wip.tile([P, ND, MLP // 8], F8, tag="wil")
                wi_eng.dma_start(out=wil_sb, in_=wil_t[:, :, sl])
            for mj in range(NM // 8):
                m0 = eighth * (NM // 8) + mj
                ps = psy.tile([P, S], F32, space="PSUM", tag="y")
                if WI_FP8:
                    for sh in range(2):
                        for ti, term in enumerate((wi_sb, wil_sb)):
                            for pr in range(4):
                                nc.tensor.matmul(
                                    ps[:, sh * 512:(sh + 1) * 512],
                                    term[:, 2 * pr:2 * pr + 2, mj * P:(mj + 1) * P],
                                    h2T[:, 2 * pr:2 * pr + 2, sh * 512:(sh + 1) * 512],
                                    start=(ti == 0 and pr == 0),
                                    stop=(ti == 1 and pr == 3),
                                    perf_mode=DR,
                                )
                else:
                    for sh in range(2):
                        for di in range(ND):
                            nc.tensor.matmul(
                                ps[:, sh * 512:(sh + 1) * 512],
                                wi_sb[:, di, mj * P:(mj + 1) * P],
                                h2T[:, di, sh * 512:(sh + 1) * 512],
                                start=(di == 0), stop=(di == ND - 1),
                            )
                nc.scalar.activation(out=yT[:, m0, :], in_=ps[:, :], func=AF.Relu)
    h2T_pool.release()

    # ---- stage 7: out = out1 + y^T.T @ womlp -------------------------------
    # 4 groups (dh half of D x sg half of tokens); womlp streamed per group,
    # two groups of PSUM banks so group g+1 computes while g drains.
    with tc.tile_pool(name="wmp", bufs=3) as wmp, \
         tc.tile_pool(name="oop", bufs=4) as oop, \
         tc.tile_pool(name="o2ps", bufs=8, space="PSUM") as o2ps:
        # group list: (dh, si list); last group split so its drain overlaps
        groups = [(0, [0, 1, 2, 3]), (0, [4, 5, 6, 7]),
                  (1, [0, 1, 2, 3]), (1, [4, 5]), (1, [6, 7])]
        for gi, (dh, sis) in enumerate(groups):
            pss = [o2ps.tile([P, 512], F32, space="PSUM", tag="o2",
                             name=f"o2_{gi}_{i}") for i in range(len(sis))]
            for cg in range(NM // 4):
                wmc = wmp.tile([P, 4, 512], BF16, tag="wm")
                dma_eng = nc.scalar if ACT_DMA else nc.sync
                dma_eng.dma_start(
                    out=wmc,
                    in_=wm_t[:, 4 * cg:4 * cg + 4, dh * 512:(dh + 1) * 512])
                for cj in range(4):
                    ci = 4 * cg + cj
                    for i4, si in enumerate(sis):
                        nc.tensor.matmul(
                            pss[i4][:, :],
                            yT[:, ci, si * P:(si + 1) * P],
                            wmc[:, cj, :],
                            start=(ci == 0), stop=(ci == NM - 1),
                        )
            for i4, si in enumerate(sis):
                oo = oop.tile([P, 512], F32, tag="oo")
                nc.vector.tensor_add(out=oo, in0=pss[i4][:, :],
                                     in1=out1[:, si, dh * 512:(dh + 1) * 512])
                nc.sync.dma_start(
                    out=out_d.ap()[si * P:(si + 1) * P, dh * 512:(dh + 1) * 512],
                    in_=oo)

    yT_pool.release()
    out1_pool.release()
    singles.release()


# ---- host wrapper ----------------------------------------------------------
_NC_CACHE = {}


def _get_nc():
    if "nc" not in _NC_CACHE:
        _NC_CACHE["nc"] = build_nc()
    return _NC_CACHE["nc"]


def _get_exec():
    """Compile once: a sharded PJRT executable over the 8 NeuronCores."""
    if "exec" in _NC_CACHE:
        return _NC_CACHE["exec"]
    import jax
    from jax.sharding import Mesh, PartitionSpec, NamedSharding
    from jax.experimental.shard_map import shard_map
    from concourse.bass2jax import (
        _bass_exec_p, install_neuronx_cc_hook, partition_id_tensor,
    )

    nc = _get_nc()
    install_neuronx_cc_hook()
    pname = nc.partition_id_tensor.name if nc.partition_id_tensor else None
    in_names, out_names, out_avals, zero_outs = [], [], [], []
    for alloc in nc.m.functions[0].allocations:
        if not isinstance(alloc, mybir.MemoryLocationSet):
            continue
        name = alloc.memorylocations[0].name
        if alloc.kind == "ExternalInput":
            if name != pname:
                in_names.append(name)
        elif alloc.kind == "ExternalOutput":
            out_names.append(name)
            shape = tuple(alloc.tensor_shape)
            dtype = mybir.dt.np(alloc.dtype)
            out_avals.append(jax.core.ShapedArray(shape, dtype))
            zero_outs.append(np.zeros(shape, dtype))
    n_params = len(in_names)
    all_in_names = in_names + out_names + ([pname] if pname else [])

    def _body(*args):
        operands = list(args)
        if pname is not None:
            operands.append(partition_id_tensor())
        outs = _bass_exec_p.bind(
            *operands,
            out_avals=tuple(out_avals),
            in_names=tuple(all_in_names),
            out_names=tuple(out_names),
            lowering_input_output_aliases=(),
            sim_require_finite=True,
            sim_require_nnan=True,
            nc=nc,
        )
        return tuple(outs)

    n_outs = len(out_avals)
    devices = jax.devices()[:NCORES]
    mesh = Mesh(np.asarray(devices), ("core",))
    sharded = jax.jit(
        shard_map(_body, mesh=mesh,
                  in_specs=(PartitionSpec("core"),) * (n_params + n_outs),
                  out_specs=(PartitionSpec("core"),) * n_outs,
                  check_rep=False),
        donate_argnums=tuple(range(n_params, n_params + n_outs)),
        keep_unused=True,
    )
    sh = NamedSharding(mesh, PartitionSpec("core"))
    _NC_CACHE["exec"] = (sharded, in_names, out_names, zero_outs, sh)
    return _NC_CACHE["exec"]


def _prep_inputs(x, ln1_scale, wq, wk, wv, wo_attn, ln2_scale, wi, wo_mlp, rel_emb):
    x = np.asarray(x, np.float32)
    ln1 = np.asarray(ln1_scale, np.float32)[:, None]
    ln2 = np.asarray(ln2_scale, np.float32)[:, None]
    wq_h = (np.asarray(wq, np.float32) * ln1).astype(np.float32)
    wk_h = (np.asarray(wk, np.float32) * ln1).astype(np.float32)
    wv_h = (np.asarray(wv, np.float32) * ln1).astype(np.float32)
    wo_h = np.asarray(wo_attn, np.float32).astype(BF16NP)
    wi_eff = np.asarray(wi, np.float32) * ln2
    if WI_FP8:
        wi8_h = wi_eff.astype(F8NP)
        wil_h = (wi_eff - wi8_h.astype(np.float32)).astype(F8NP)
    else:
        wi8_h = wi_eff.astype(BF16NP)
        wil_h = None
    wm_h = np.asarray(wo_mlp, np.float32).astype(BF16NP)
    bias3, cexp = _bias_data(np.asarray(rel_emb, np.float32))
    shared = {
        "wq": wq_h, "wk": wk_h, "wv": wv_h, "wo": wo_h,
        "wi8": wi8_h, "womlp": wm_h, "bias3": bias3, "cexp": cexp,
    }
    if WI_FP8:
        shared["wil"] = wil_h
    in_maps = [dict(shared, x=np.ascontiguousarray(x[b])) for b in range(NCORES)]
    return in_maps


def kernel(x, ln1_scale, wq, wk, wv, wo_attn, ln2_scale, wi, wo_mlp, rel_emb):
    import jax
    in_maps = _prep_inputs(x, ln1_scale, wq, wk, wv, wo_attn, ln2_scale,
                           wi, wo_mlp, rel_emb)
    sharded, in_names, out_names, zero_outs, sh = _get_exec()
    concat_in = [
        jax.device_put(
            np.concatenate([in_maps[c][n] for c in range(NCORES)], axis=0), sh)
        for n in in_names
    ]
    czero = [
        jax.device_put(np.zeros((NCORES * z.shape[0], *z.shape[1:]), z.dtype), sh)
        for z in zero_outs
    ]
    outs = sharded(*concat_in, *czero)
    oidx = out_names.index("out")
    full = np.asarray(outs[oidx]).reshape(NCORES, S, D)
    return full.astype(np.float32)


# revision 30
# speedup vs baseline: 1.3024x; 1.0207x over previous
"""T5-style encoder layer (pre-LN, RMSNorm, relative-position bias) on 8 trn2
NeuronCores, data-parallel over the batch dimension (B=8 -> one batch element
per core). Each core runs the full layer for its [S, D] slice; weights and the
relative-bias data are replicated.

Self-contained: hardcodes all shapes; only depends on the runtime at
/opt/trn_rl_repo.
"""

import sys

if "/opt/trn_rl_repo" not in sys.path:
    sys.path.insert(0, "/opt/trn_rl_repo")

import numpy as np
import ml_dtypes

import concourse.bass as bass
import concourse.tile as tile
from concourse import bacc
from concourse import mybir
from concourse.masks import make_identity

# ---- problem constants -----------------------------------------------------
B, S, D = 8, 1024, 1024
H, HD = 16, 64
MLP = 4096
NUM_BUCKETS, MAX_DIST = 32, 128
EPS = 1e-6
NCORES = 8
P = 128
NS = S // P        # 8 token tiles
ND = D // P        # 8 feature tiles
NM = MLP // P      # 32 mlp tiles

F32 = mybir.dt.float32
F32R = mybir.dt.float32r
BF16 = mybir.dt.bfloat16
F8 = mybir.dt.float8e4
BF16NP = ml_dtypes.bfloat16
F8NP = ml_dtypes.float8_e4m3

import os
WI_FP8 = os.environ.get("K_WI_FP8", "1") == "1"   # 2-term fp8 DoubleRow wi
SEG_GPSIMD = os.environ.get("K_SEG_GPSIMD", "1") == "1"
ACT_DMA = os.environ.get("K_ACT_DMA", "1") == "1"
TP_SHIFT = os.environ.get("K_TP_SHIFT", "1") == "1"

AF = mybir.ActivationFunctionType
DR = mybir.MatmulPerfMode.DoubleRow


# ---- host-side relative position bias --------------------------------------
def _rel_pos_bucket_np(rel):
    # mirrors t5x _relative_position_bucket (bidirectional), numpy fp32
    n = -rel
    num_buckets = NUM_BUCKETS // 2          # 16
    ret = (n < 0).astype(np.int32) * num_buckets
    n = np.abs(n)
    max_exact = num_buckets // 2            # 8
    is_small = n < max_exact
    val_if_large = max_exact + (
        np.log(n.astype(np.float32) / max_exact + np.finfo(np.float32).eps)
        / np.log(MAX_DIST / max_exact)
        * (num_buckets - max_exact)
    ).astype(np.int32)
    val_if_large = np.minimum(val_if_large, num_buckets - 1)
    return ret + np.where(is_small, n, val_if_large)


def _bias_data(rel_emb):
    """Compressed exp(bias) data.

    Returns (bias3, cexp):
      bias3 [H, 128, 3, 128] bf16: block d (m = 1-d = k_tile - q_tile) at
        [p, c] = exp(bias[k, q]) for k = k_tile*128 + p, q = q_tile*128 + c,
        i.e. exp(table[k - q]) with k - q = m*128 + p - c.
      cexp  [1, 2H] f32: per head, exp of the two saturated buckets:
        [2h]   = exp(emb[31, h])  (k - q >= 128, q_tile <= k_tile - 2)
        [2h+1] = exp(emb[15, h])  (k - q <= -128, q_tile >= k_tile + 2)
    """
    rel = np.arange(-(S - 1), S, dtype=np.int32)          # k - q in [-1023, 1023]
    buckets = _rel_pos_bucket_np(rel)                     # [2047]
    table = rel_emb[buckets, :].astype(np.float32)        # [2047, H]
    pp = np.arange(P)[:, None, None]
    dd = np.arange(3)[None, :, None]
    cc = np.arange(P)[None, None, :]
    idx = 1023 + (1 - dd) * P + pp - cc                   # [128, 3, 128]
    blocks = np.exp(table[idx])                           # [128, 3, 128, H]
    bias3 = np.ascontiguousarray(blocks.transpose(3, 0, 1, 2)).astype(BF16NP)
    cexp = np.empty((1, 2 * H), np.float32)
    cexp[0, 0::2] = np.exp(rel_emb[31, :].astype(np.float32))
    cexp[0, 1::2] = np.exp(rel_emb[15, :].astype(np.float32))
    return bias3, cexp


# ---- device kernel ---------------------------------------------------------
def build_nc():
    nc = bacc.Bacc(None, target_bir_lowering=False)

    x_d = nc.declare_dram_parameter("x", [S, D], F32, isOutput=False)
    wq_d = nc.declare_dram_parameter("wq", [D, H * HD], F32R, isOutput=False)
    wk_d = nc.declare_dram_parameter("wk", [D, H * HD], F32R, isOutput=False)
    wv_d = nc.declare_dram_parameter("wv", [D, H * HD], F32R, isOutput=False)
    wo_d = nc.declare_dram_parameter("wo", [H * HD, D], BF16, isOutput=False)
    if WI_FP8:
        wi8_d = nc.declare_dram_parameter("wi8", [D, MLP], F8, isOutput=False)
        wil_d = nc.declare_dram_parameter("wil", [D, MLP], F8, isOutput=False)
    else:
        wi8_d = nc.declare_dram_parameter("wi8", [D, MLP], BF16, isOutput=False)
        wil_d = None
    wm_d = nc.declare_dram_parameter("womlp", [MLP, D], BF16, isOutput=False)
    bias_d = nc.declare_dram_parameter("bias3", [H, P, 3, P], BF16, isOutput=False)
    cexp_d = nc.declare_dram_parameter("cexp", [1, 2 * H], F32, isOutput=False)
    out_d = nc.declare_dram_parameter("out", [S, D], F32, isOutput=True)

    wo_t = wo_d.ap().rearrange("(hp p) d -> p hp d", p=P)
    wq_t = wq_d.ap().rearrange("(di p) m -> p di m", p=P)
    wk_t = wk_d.ap().rearrange("(di p) m -> p di m", p=P)
    wv_t = wv_d.ap().rearrange("(di p) m -> p di m", p=P)
    wi8_t = wi8_d.ap().rearrange("(di p) m -> p di m", p=P)
    wil_t = wil_d.ap().rearrange("(di p) m -> p di m", p=P) if WI_FP8 else None
    wm_t = wm_d.ap().rearrange("(ci p) d -> p ci d", p=P)
    bias_t = bias_d.ap().rearrange("h p d c -> p h d c")

    with tile.TileContext(nc) as tc:
        _body(nc, tc, x_d, wq_t, wk_t, wv_t, wo_t, wi8_t, wil_t, wm_t,
              bias_t, cexp_d, out_d)
    nc.finalize()
    return nc


def _rms_factor(nc, nrm, src_ap, sq_tile, eps_t):
    """rstd [P,1] = rsqrt(mean(src^2) + eps); sq_tile is scratch.

    Square+accum on ACT does the elementwise square and the free-axis sum
    in one pass; the rsqrt split (sqrt on ACT, reciprocal on DVE) follows
    the bass guidance (ACT Rsqrt is inaccurate).
    """
    var = nrm.tile([P, 1], F32, tag="var")
    nc.scalar.activation(out=sq_tile, in_=src_ap, func=AF.Square,
                         accum_out=var)
    sd = nrm.tile([P, 1], F32, tag="sd")
    nc.scalar.activation(out=sd, in_=var, func=AF.Sqrt,
                         bias=eps_t[:, :], scale=1.0 / D)
    rstd = nrm.tile([P, 1], F32, tag="rstd")
    nc.vector.reciprocal(out=rstd, in_=sd)
    return rstd


def _body(nc, tc, x_d, wq_t, wk_t, wv_t, wo_t, wi8_t, wil_t, wm_t,
          bias_t, cexp_d, out_d):
    ALU = mybir.AluOpType

    # ---- persistent small data ---------------------------------------------
    singles = tc.alloc_tile_pool(name="singles", bufs=1)
    ident16 = singles.tile([P, P], BF16)
    make_identity(nc, ident16)
    ident32 = singles.tile([P, P], F32)
    make_identity(nc, ident32)
    ident32r = singles.tile([P, P], F32R)
    nc.vector.tensor_copy(out=ident32r, in_=ident32)
    eps_t = singles.tile([P, 1], F32)
    nc.vector.memset(eps_t, EPS)
    cexp_sb = singles.tile([P, 2 * H], F32)
    bias_sb = singles.tile([P, H, 3, P], BF16)

    # activations that live through the attention block
    qkv_act = tc.alloc_tile_pool(name="qkv_act", bufs=1)
    hT = qkv_act.tile([P, ND, S], F32R)      # rmsnorm(x)^T  [d, s]
    qT = qkv_act.tile([P, ND, S], F32R)      # q^T  [m, s] per head-pair tile
    kT = qkv_act.tile([P, ND, S], F32R)
    v_ext = qkv_act.tile([P, NS, H, HD + 1], BF16)  # [tok, stile, h, hd|1]
    nc.vector.memset(v_ext[:, :, :, HD:HD + 1], 1.0)
    # right-side stack: out1 (lives to the end) below attnT (dies after wo)
    out1_pool = tc.alloc_tile_pool(name="out1_pool", bufs=1, side="right")
    out1 = out1_pool.tile([P, NS, D], F32)    # x + attn_out, token-major
    attnT_pool = tc.alloc_tile_pool(name="attnT_pool", bufs=1, side="right")
    attnT = attnT_pool.tile([P, H // 2, S], BF16)   # heads packed 2/tile

    # ---- stage 1 + 2a interleaved: rmsnorm/transpose + V projection --------
    # emission order puts x tiles first on the SP queue, then wv chunks (split
    # per di pair for fast first-use), then bias/cexp (needed only at attn).
    with tc.tile_pool(name="xs1", bufs=2) as xs1, \
         tc.tile_pool(name="sq1", bufs=2) as sq1, \
         tc.tile_pool(name="nrm1", bufs=4) as nrm1, \
         tc.tile_pool(name="wvp", bufs=4) as wvp, \
         tc.tile_pool(name="st1ps", bufs=2, space="PSUM") as st1ps, \
         tc.tile_pool(name="vps", bufs=2, space="PSUM") as vps:
        w_sbs = []

        def emit_wv_dma(quarter):
            # wv columns for heads [4q, 4q+4), split in two [P, 4, 256] tiles
            pair = []
            for dh_ in range(2):
                w_sb = wvp.tile([P, 4, 256], F32R, tag="wv",
                                name=f"wv{quarter}_{dh_}")
                nc.sync.dma_start(
                    out=w_sb,
                    in_=wv_t[:, 4 * dh_:4 * dh_ + 4,
                             quarter * 256:(quarter + 1) * 256])
                pair.append(w_sb)
            w_sbs.append(pair)

        def emit_v(quarter, ci):
            ps = vps.tile([P, 256], F32, space="PSUM", tag="vps")
            for di in range(ND):
                nc.tensor.matmul(
                    ps[:, :],
                    hT[:, di, ci * P:(ci + 1) * P],
                    w_sbs[quarter][di // 4][:, di % 4, :],
                    start=(di == 0), stop=(di == ND - 1),
                )
            nc.scalar.copy(
                out=v_ext[:, ci, quarter * 4:quarter * 4 + 4, 0:HD],
                in_=ps[:, :].rearrange("p (h e) -> p h e", e=HD),
            )

        for si in range(NS):
            xt = xs1.tile([P, D], F32, tag="x")
            nc.sync.dma_start(out=xt, in_=x_d.ap()[si * P:(si + 1) * P, :])
            if si == 2:
                emit_wv_dma(0)
            if si == 4:
                emit_wv_dma(1)
            if si == 6:
                nc.sync.dma_start(out=cexp_sb,
                                  in_=cexp_d.ap()[0:1, :].broadcast_to((P, 2 * H)))
                nc.sync.dma_start(out=bias_sb, in_=bias_t)
            ht = sq1.tile([P, D], F32R, tag="sq", name=f"ht{si}")
            rstd = _rms_factor(nc, nrm1, xt[:, :], ht, eps_t)
            nc.vector.tensor_scalar_mul(out=ht, in0=xt, scalar1=rstd[:, :])
            tp = st1ps.tile([P, ND, P], F32R, space="PSUM", tag="tp1")
            for di in range(ND):
                nc.tensor.transpose(tp[:, di, :], ht[:, di * P:(di + 1) * P],
                                    ident32r[:, :])
            nc.scalar.copy(out=hT[:, :, si * P:(si + 1) * P], in_=tp[:, :, :])
            # v for already-transposed token tiles fills PE while rms runs
            if si >= 2:
                emit_v(0, si - 2)
            if si >= 5:
                emit_v(1, si - 5)
        for ci in range(6, NS):
            emit_v(0, ci)
        for ci in range(3, NS):
            emit_v(1, ci)
        for quarter in range(2, 4):
            emit_wv_dma(quarter)
            for ci in range(NS):
                emit_v(quarter, ci)

    # ---- stage 2b/3: QK projection interleaved with per-head attention -----
    with tc.tile_pool(name="wqkp", bufs=3) as wqkp, \
         tc.tile_pool(name="qkp", bufs=1, space="PSUM") as qkp, \
         tc.tile_pool(name="lgp", bufs=2, space="PSUM") as lgp, \
         tc.tile_pool(name="avp", bufs=1, space="PSUM") as avp, \
         tc.tile_pool(name="tpp", bufs=1, space="PSUM") as tpp, \
         tc.tile_pool(name="exq", bufs=3) as exq, \
         tc.tile_pool(name="wexpp", bufs=4) as wexpp, \
         tc.tile_pool(name="rdp", bufs=4) as rdp, \
         tc.tile_pool(name="avsbp", bufs=2) as avsbp:

        def qk_project(hm):
            for (w_ap, dstT, nm) in ((wq_t, qT, "q"), (wk_t, kT, "k")):
                w_sb = wqkp.tile([P, ND, P], F32R, tag="w", name=f"w{nm}{hm}")
                nc.sync.dma_start(out=w_sb, in_=w_ap[:, :, hm * P:(hm + 1) * P])
                ps = qkp.tile([P, 2, 256], F32, space="PSUM", tag="qk",
                              name=f"qk{nm}{hm}")
                for qt in range(4):
                    sl = ps[:, qt % 2, :]
                    for di in range(ND):
                        nc.tensor.matmul(
                            sl,
                            w_sb[:, di, :],
                            hT[:, di, qt * 256:(qt + 1) * 256],
                            start=(di == 0), stop=(di == ND - 1),
                        )
                    if qt % 2 == 1:
                        nc.vector.tensor_copy(
                            out=dstT[:, hm, (qt - 1) * 256:(qt + 1) * 256],
                            in_=ps[:, :, :])

        def attn_head(h, prev_drain):
            hm, hb = h // 2, HD * (h % 2)
            av = avp.tile([P, NS, P], F32, space="PSUM", tag="av")
            lgs = []

            def emit_lg(ki):
                lg = lgp.tile([P, S], F32, space="PSUM", tag="lg")
                for qh in range(2):
                    nc.tensor.matmul(
                        lg[:, qh * 512:(qh + 1) * 512],
                        kT[hb:hb + HD, hm, ki * P:(ki + 1) * P],
                        qT[hb:hb + HD, hm, qh * 512:(qh + 1) * 512],
                        start=True, stop=True,
                    )
                lgs.append(lg)

            emit_lg(0)
            for ki in range(NS):
                lg = lgs[ki]
                ex = exq.tile([P, S], BF16, tag="ex")
                nc.scalar.activation(out=ex, in_=lg[:, :], func=AF.Exp)
                wexp = wexpp.tile([P, S], BF16, tag="wexp")
                # middle (varying-bias) window: q tiles [ki-1, ki+1]
                q0, q1 = max(ki - 1, 0), min(ki + 2, NS)
                d0 = q0 - (ki - 1)
                nc.vector.tensor_mul(
                    out=wexp[:, q0 * P:q1 * P].rearrange("p (c w) -> p c w", w=P),
                    in0=ex[:, q0 * P:q1 * P].rearrange("p (c w) -> p c w", w=P),
                    in1=bias_sb[:, h, d0:d0 + (q1 - q0), :],
                )
                # constant-bias segments (saturated buckets)
                segs = []
                if q0 > 0:
                    segs.append((0, q0 * P, cexp_sb[:, 2 * h:2 * h + 1]))
                if q1 < NS:
                    segs.append((q1 * P, S, cexp_sb[:, 2 * h + 1:2 * h + 2]))
                segs.sort(key=lambda t: t[1] - t[0])
                for i, (a, b, c_ap) in enumerate(segs):
                    eng = nc.gpsimd if (SEG_GPSIMD and i == len(segs) - 1) else nc.vector
                    eng.tensor_scalar_mul(out=wexp[:, a:b], in0=ex[:, a:b],
                                          scalar1=c_ap)
                # next logits tile goes to PE before av(ki), filling the
                # exp(ki) wait; previous head's transposes fill the first one
                if ki + 1 < NS:
                    emit_lg(ki + 1)
                if ki == 0 and prev_drain is not None:
                    prev_drain()
                for qi in range(NS):
                    # one accumulation group per PSUM bank (4 qi slots/bank):
                    # start pending-zeroes the whole bank, so only the first
                    # matmul in each bank starts and the last stops.
                    nc.tensor.matmul(
                        av[:, qi, 0:HD + 1],
                        wexp[:, qi * P:(qi + 1) * P],
                        v_ext[:, ki, h, :],
                        start=(ki == 0 and qi % 4 == 0),
                        stop=(ki == NS - 1 and qi % 4 == 3),
                    )
            # drain now (DVE, batched): one reciprocal over the 8 denominators
            # and one broadcast-multiply normalize; the PE transposes are
            # deferred into the next head's exp(0) window.
            av_sb = avsbp.tile([P, NS, HD], BF16, tag="avsb")
            for qi in range(NS):
                rden = rdp.tile([P, 1], F32, tag="rd")
                nc.vector.reciprocal(out=rden, in_=av[:, qi, HD:HD + 1])
                nc.vector.tensor_scalar_mul(out=av_sb[:, qi, :],
                                            in0=av[:, qi, 0:HD],
                                            scalar1=rden)

            def drain():
                tp = tpp.tile([P, NS, P], BF16, space="PSUM", tag="tp")
                tb = hb if TP_SHIFT else 0
                for qi in range(NS):
                    nc.tensor.transpose(tp[tb:tb + HD, qi, :], av_sb[:, qi, :],
                                        ident16[:, :])
                nc.vector.tensor_copy(out=attnT[hb:hb + HD, hm, :],
                                      in_=tp[tb:tb + HD, :, :])
            return drain

        pending = None
        for hm in range(ND):
            qk_project(hm)
            pending = attn_head(2 * hm, pending)
            pending = attn_head(2 * hm + 1, pending)
        pending()

    # ---- stage 4/5: attn @ wo + residual, rmsnorm -> h2T --------------------
    qkv_act.release()
    yT_pool = tc.alloc_tile_pool(name="yT_pool", bufs=1)
    yT = yT_pool.tile([P, NM, S], BF16)
    h2T_pool = tc.alloc_tile_pool(name="h2T_pool", bufs=1)
    h2T = h2T_pool.tile([P, ND, S], F8 if WI_FP8 else BF16)

    with tc.tile_pool(name="wop", bufs=1) as wop, \
         tc.tile_pool(name="xs4", bufs=3) as xs4, \
         tc.tile_pool(name="ops", bufs=2, space="PSUM") as ops, \
         tc.tile_pool(name="tp5", bufs=2, space="PSUM") as tp5, \
         tc.tile_pool(name="sq5", bufs=2) as sq5, \
         tc.tile_pool(name="nrm5", bufs=4) as nrm5, \
         tc.tile_pool(name="h2p", bufs=2) as h2p:
        wo_sb = wop.tile([P, H // 2, D], BF16)
        for si in range(NS):
            xt = xs4.tile([P, D], F32, tag="x")
            nc.sync.dma_start(out=xt, in_=x_d.ap()[si * P:(si + 1) * P, :])
            if si == 0:
                for hq in range(4):
                    nc.sync.dma_start(out=wo_sb[:, 2 * hq:2 * hq + 2, :],
                                      in_=wo_t[:, 2 * hq:2 * hq + 2, :])
            ps = ops.tile([P, D], F32, space="PSUM", tag="wo")
            for hp in range(H // 2):
                for dh in range(2):
                    nc.tensor.matmul(
                        ps[:, dh * 512:(dh + 1) * 512],
                        attnT[:, hp, si * P:(si + 1) * P],
                        wo_sb[:, hp, dh * 512:(dh + 1) * 512],
                        start=(hp == 0), stop=(hp == H // 2 - 1),
                    )
            nc.vector.tensor_add(out=out1[:, si, :], in0=ps[:, :], in1=xt[:, :])
            # stage 5 per si: rmsnorm -> h2 (bf16) -> transpose -> h2T
            sq = sq5.tile([P, D], F32, tag="sq")
            rstd = _rms_factor(nc, nrm5, out1[:, si, :], sq, eps_t)
            h2 = h2p.tile([P, D], BF16, tag="h2")
            nc.vector.tensor_scalar_mul(out=h2, in0=out1[:, si, :],
                                        scalar1=rstd[:, :])
            tp = tp5.tile([P, ND, P], BF16, space="PSUM", tag="tp5")
            for di in range(ND):
                nc.tensor.transpose(tp[:, di, :], h2[:, di * P:(di + 1) * P],
                                    ident16[:, :])
            nc.scalar.copy(out=h2T[:, :, si * P:(si + 1) * P], in_=tp[:, :, :])
    attnT_pool.release()

    # ---- stage 6: y^T = relu(wi^T @ h2^T) ----------------------------------
    with tc.tile_pool(name="wip", bufs=2) as wip, \
         tc.tile_pool(name="psy", bufs=2, space="PSUM") as psy:
        for eighth in range(8):
            sl = slice(eighth * (MLP // 8), (eighth + 1) * (MLP // 8))
            wi_sb = wip.tile([P, ND, MLP // 8], F8 if WI_FP8 else BF16, tag="wi8")
            wi_eng = nc.scalar if ACT_DMA else nc.sync
            wi_eng.dma_start(out=wi_sb, in_=wi8_t[:, :, sl])
            if WI_FP8:
                wil_sb = wip.tile([P, ND, MLP // 8], F8, tag="wil")
                wi_eng.dma_start(out=wil_sb, in_=wil_t[:, :, sl])
            for mj in range(NM // 8):
                m0 = eighth * (NM // 8) + mj
                ps = psy.tile([P, S], F32, space="PSUM", tag="y")
                if WI_FP8:
                    for sh in range(2):
                        for ti, term in enumerate((wi_sb, wil_sb)):
                            for pr in range(4):
                                nc.tensor.matmul(
                                    ps[:, sh * 512:(sh + 1) * 512],
                                    term[:, 2 * pr:2 * pr + 2, mj * P:(mj + 1) * P],
                                    h2T[:, 2 * pr:2 * pr + 2, sh * 512:(sh + 1) * 512],
                                    start=(ti == 0 and pr == 0),
                                    stop=(ti == 1 and pr == 3),
                                    perf_mode=DR,
                                )
                else:
                    for sh in range(2):
                        for di in range(ND):
                            nc.tensor.matmul(
                                ps[:, sh * 512:(sh + 1) * 512],
                                wi_sb[:, di, mj * P:(mj + 1) * P],
                                h2T[:, di, sh * 512:(sh + 1) * 512],
                                start=(di == 0), stop=(di == ND - 1),
                            )
                nc.scalar.activation(out=yT[:, m0, :], in_=ps[:, :], func=AF.Relu)
    h2T_pool.release()

    # ---- stage 7: out = out1 + y^T.T @ womlp -------------------------------
    # 4 groups (dh half of D x sg half of tokens); womlp streamed per group,
    # two groups of PSUM banks so group g+1 computes while g drains.
    with tc.tile_pool(name="wmp", bufs=4) as wmp, \
         tc.tile_pool(name="oop", bufs=4) as oop, \
         tc.tile_pool(name="o2ps", bufs=8, space="PSUM") as o2ps:
        # group list: (dh, si list); last group split so its drain overlaps
        groups = [(0, [0, 1, 2, 3]), (0, [4, 5, 6, 7]),
                  (1, [0, 1, 2, 3]), (1, [4, 5]), (1, [6, 7])]
        for gi, (dh, sis) in enumerate(groups):
            pss = [o2ps.tile([P, 512], F32, space="PSUM", tag="o2",
                             name=f"o2_{gi}_{i}") for i in range(len(sis))]
            for cg in range(NM // 4):
                wmc = wmp.tile([P, 4, 512], BF16, tag="wm")
                dma_eng = nc.scalar if ACT_DMA else nc.sync
                dma_eng.dma_start(
                    out=wmc,
                    in_=wm_t[:, 4 * cg:4 * cg + 4, dh * 512:(dh + 1) * 512])
                for cj in range(4):
                    ci = 4 * cg + cj
                    for i4, si in enumerate(sis):
                        nc.tensor.matmul(
                            pss[i4][:, :],
                            yT[:, ci, si * P:(si + 1) * P],
                            wmc[:, cj, :],
                            start=(ci == 0), stop=(ci == NM - 1),
                        )
            for i4, si in enumerate(sis):
                oo = oop.tile([P, 512], F32, tag="oo")
                nc.vector.tensor_add(out=oo, in0=pss[i4][:, :],
                                     in1=out1[:, si, dh * 512:(dh + 1) * 512])
                nc.sync.dma_start(
                    out=out_d.ap()[si * P:(si + 1) * P, dh * 512:(dh + 1) * 512],
                    in_=oo)

    yT_pool.release()
    out1_pool.release()
    singles.release()


# ---- host wrapper ----------------------------------------------------------
_NC_CACHE = {}


def _get_nc():
    if "nc" not in _NC_CACHE:
        _NC_CACHE["nc"] = build_nc()
    return _NC_CACHE["nc"]


def _get_exec():
    """Compile once: a sharded PJRT executable over the 8 NeuronCores."""
    if "exec" in _NC_CACHE:
        return _NC_CACHE["exec"]
    import jax
    from jax.sharding import Mesh, PartitionSpec, NamedSharding
    from jax.experimental.shard_map import shard_map
    from concourse.bass2jax import (
        _bass_exec_p, install_neuronx_cc_hook, partition_id_tensor,
    )

    nc = _get_nc()
    install_neuronx_cc_hook()
    pname = nc.partition_id_tensor.name if nc.partition_id_tensor else None
    in_names, out_names, out_avals, zero_outs = [], [], [], []
    for alloc in nc.m.functions[0].allocations:
        if not isinstance(alloc, mybir.MemoryLocationSet):
            continue
        name = alloc.memorylocations[0].name
        if alloc.kind == "ExternalInput":
            if name != pname:
                in_names.append(name)
        elif alloc.kind == "ExternalOutput":
            out_names.append(name)
            shape = tuple(alloc.tensor_shape)
            dtype = mybir.dt.np(alloc.dtype)
            out_avals.append(jax.core.ShapedArray(shape, dtype))
            zero_outs.append(np.zeros(shape, dtype))
    n_params = len(in_names)
    all_in_names = in_names + out_names + ([pname] if pname else [])

    def _body(*args):
        operands = list(args)
        if pname is not None:
            operands.append(partition_id_tensor())
        outs = _bass_exec_p.bind(
            *operands,
            out_avals=tuple(out_avals),
            in_names=tuple(all_in_names),
            out_names=tuple(out_names),
            lowering_input_output_aliases=(),
            sim_require_finite=True,
            sim_require_nnan=True,
            nc=nc,
        )
        return tuple(outs)

    n_outs = len(out_avals)
    devices = jax.devices()[:NCORES]
    mesh = Mesh(np.asarray(devices), ("core",))
    sharded = jax.jit(
        shard_map(_body, mesh=mesh,
                  in_specs=(PartitionSpec("core"),) * (n_params + n_outs),
                  out_specs=(PartitionSpec("core"),) * n_outs,
                  check_rep=False),
        donate_argnums=tuple(range(n_params, n_params + n_outs)),
        keep_unused=True,
    )
    sh = NamedSharding(mesh, PartitionSpec("core"))
    _NC_CACHE["exec"] = (sharded, in_names, out_names, zero_outs, sh)
    return _NC_CACHE["exec"]


def _prep_inputs(x, ln1_scale, wq, wk, wv, wo_attn, ln2_scale, wi, wo_mlp, rel_emb):
    x = np.asarray(x, np.float32)
    ln1 = np.asarray(ln1_scale, np.float32)[:, None]
    ln2 = np.asarray(ln2_scale, np.float32)[:, None]
    wq_h = (np.asarray(wq, np.float32) * ln1).astype(np.float32)
    wk_h = (np.asarray(wk, np.float32) * ln1).astype(np.float32)
    wv_h = (np.asarray(wv, np.float32) * ln1).astype(np.float32)
    wo_h = np.asarray(wo_attn, np.float32).astype(BF16NP)
    wi_eff = np.asarray(wi, np.float32) * ln2
    if WI_FP8:
        wi8_h = wi_eff.astype(F8NP)
        wil_h = (wi_eff - wi8_h.astype(np.float32)).astype(F8NP)
    else:
        wi8_h = wi_eff.astype(BF16NP)
        wil_h = None
    wm_h = np.asarray(wo_mlp, np.float32).astype(BF16NP)
    bias3, cexp = _bias_data(np.asarray(rel_emb, np.float32))
    shared = {
        "wq": wq_h, "wk": wk_h, "wv": wv_h, "wo": wo_h,
        "wi8": wi8_h, "womlp": wm_h, "bias3": bias3, "cexp": cexp,
    }
    if WI_FP8:
        shared["wil"] = wil_h
    in_maps = [dict(shared, x=np.ascontiguousarray(x[b])) for b in range(NCORES)]
    return in_maps


def kernel(x, ln1_scale, wq, wk, wv, wo_attn, ln2_scale, wi, wo_mlp, rel_emb):
    import jax
    in_maps = _prep_inputs(x, ln1_scale, wq, wk, wv, wo_attn, ln2_scale,
                           wi, wo_mlp, rel_emb)
    sharded, in_names, out_names, zero_outs, sh = _get_exec()
    concat_in = [
        jax.device_put(
            np.concatenate([in_maps[c][n] for c in range(NCORES)], axis=0), sh)
        for n in in_names
    ]
    czero = [
        jax.device_put(np.zeros((NCORES * z.shape[0], *z.shape[1:]), z.dtype), sh)
        for z in zero_outs
    ]
    outs = sharded(*concat_in, *czero)
    oidx = out_names.index("out")
    full = np.asarray(outs[oidx]).reshape(NCORES, S, D)
    return full.astype(np.float32)


# revision 35
# speedup vs baseline: 1.3174x; 1.0115x over previous
"""T5-style encoder layer (pre-LN, RMSNorm, relative-position bias) on 8 trn2
NeuronCores, data-parallel over the batch dimension (B=8 -> one batch element
per core). Each core runs the full layer for its [S, D] slice; weights and the
relative-bias data are replicated.

Self-contained: hardcodes all shapes; only depends on the runtime at
/opt/trn_rl_repo.
"""

import sys

if "/opt/trn_rl_repo" not in sys.path:
    sys.path.insert(0, "/opt/trn_rl_repo")

import numpy as np
import ml_dtypes

import concourse.bass as bass
import concourse.tile as tile
from concourse import bacc
from concourse import mybir
from concourse.masks import make_identity

# ---- problem constants -----------------------------------------------------
B, S, D = 8, 1024, 1024
H, HD = 16, 64
MLP = 4096
NUM_BUCKETS, MAX_DIST = 32, 128
EPS = 1e-6
NCORES = 8
P = 128
NS = S // P        # 8 token tiles
ND = D // P        # 8 feature tiles
NM = MLP // P      # 32 mlp tiles

F32 = mybir.dt.float32
F32R = mybir.dt.float32r
BF16 = mybir.dt.bfloat16
F8 = mybir.dt.float8e4
BF16NP = ml_dtypes.bfloat16
F8NP = ml_dtypes.float8_e4m3

import os
WI_FP8 = os.environ.get("K_WI_FP8", "1") == "1"   # 2-term fp8 DoubleRow wi
SEG_GPSIMD = os.environ.get("K_SEG_GPSIMD", "1") == "1"
ACT_DMA = os.environ.get("K_ACT_DMA", "1") == "1"
TP_SHIFT = os.environ.get("K_TP_SHIFT", "1") == "1"

AF = mybir.ActivationFunctionType
DR = mybir.MatmulPerfMode.DoubleRow


# ---- host-side relative position bias --------------------------------------
def _rel_pos_bucket_np(rel):
    # mirrors t5x _relative_position_bucket (bidirectional), numpy fp32
    n = -rel
    num_buckets = NUM_BUCKETS // 2          # 16
    ret = (n < 0).astype(np.int32) * num_buckets
    n = np.abs(n)
    max_exact = num_buckets // 2            # 8
    is_small = n < max_exact
    val_if_large = max_exact + (
        np.log(n.astype(np.float32) / max_exact + np.finfo(np.float32).eps)
        / np.log(MAX_DIST / max_exact)
        * (num_buckets - max_exact)
    ).astype(np.int32)
    val_if_large = np.minimum(val_if_large, num_buckets - 1)
    return ret + np.where(is_small, n, val_if_large)


def _bias_data(rel_emb):
    """Compressed exp(bias) data.

    Returns (bias3, cexp):
      bias3 [H, 128, 3, 128] bf16: block d (m = 1-d = k_tile - q_tile) at
        [p, c] = exp(bias[k, q]) for k = k_tile*128 + p, q = q_tile*128 + c,
        i.e. exp(table[k - q]) with k - q = m*128 + p - c.
      cexp  [1, 2H] f32: per head, exp of the two saturated buckets:
        [2h]   = exp(emb[31, h])  (k - q >= 128, q_tile <= k_tile - 2)
        [2h+1] = exp(emb[15, h])  (k - q <= -128, q_tile >= k_tile + 2)
    """
    rel = np.arange(-(S - 1), S, dtype=np.int32)          # k - q in [-1023, 1023]
    buckets = _rel_pos_bucket_np(rel)                     # [2047]
    table = rel_emb[buckets, :].astype(np.float32)        # [2047, H]
    pp = np.arange(P)[:, None, None]
    dd = np.arange(3)[None, :, None]
    cc = np.arange(P)[None, None, :]
    idx = 1023 + (1 - dd) * P + pp - cc                   # [128, 3, 128]
    blocks = np.exp(table[idx])                           # [128, 3, 128, H]
    bias3 = np.ascontiguousarray(blocks.transpose(3, 0, 1, 2)).astype(BF16NP)
    cexp = np.empty((1, 2 * H), np.float32)
    cexp[0, 0::2] = np.exp(rel_emb[31, :].astype(np.float32))
    cexp[0, 1::2] = np.exp(rel_emb[15, :].astype(np.float32))
    return bias3, cexp


# ---- device kernel ---------------------------------------------------------
def build_nc():
    nc = bacc.Bacc(None, target_bir_lowering=False)

    x_d = nc.declare_dram_parameter("x", [S, D], F32R, isOutput=False)
    wq_d = nc.declare_dram_parameter("wq", [D, H * HD], F32R, isOutput=False)
    wk_d = nc.declare_dram_parameter("wk", [D, H * HD], F32R, isOutput=False)
    wv_d = nc.declare_dram_parameter("wv", [D, H * HD], F32R, isOutput=False)
    wo_d = nc.declare_dram_parameter("wo", [H * HD, D], BF16, isOutput=False)
    if WI_FP8:
        wi8_d = nc.declare_dram_parameter("wi8", [D, MLP], F8, isOutput=False)
        wil_d = nc.declare_dram_parameter("wil", [D, MLP], F8, isOutput=False)
    else:
        wi8_d = nc.declare_dram_parameter("wi8", [D, MLP], BF16, isOutput=False)
        wil_d = None
    wm_d = nc.declare_dram_parameter("womlp", [MLP, D], BF16, isOutput=False)
    bias_d = nc.declare_dram_parameter("bias3", [H, P, 3, P], BF16, isOutput=False)
    cexp_d = nc.declare_dram_parameter("cexp", [1, 2 * H], F32, isOutput=False)
    out_d = nc.declare_dram_parameter("out", [S, D], F32, isOutput=True)
    rs_scr = nc.dram_tensor("rs_scr", [1, S], F32)

    wo_t = wo_d.ap().rearrange("(hp p) d -> p hp d", p=P)
    wq_t = wq_d.ap().rearrange("(di p) m -> p di m", p=P)
    wk_t = wk_d.ap().rearrange("(di p) m -> p di m", p=P)
    wv_t = wv_d.ap().rearrange("(di p) m -> p di m", p=P)
    wi8_t = wi8_d.ap().rearrange("(di p) m -> p di m", p=P)
    wil_t = wil_d.ap().rearrange("(di p) m -> p di m", p=P) if WI_FP8 else None
    wm_t = wm_d.ap().rearrange("(ci p) d -> p ci d", p=P)
    bias_t = bias_d.ap().rearrange("h p d c -> p h d c")

    with tile.TileContext(nc) as tc:
        _body(nc, tc, x_d, wq_t, wk_t, wv_t, wo_t, wi8_t, wil_t, wm_t,
              bias_t, cexp_d, out_d, rs_scr)
    nc.finalize()
    return nc


def _rms_factor(nc, nrm, src_ap, sq_tile, eps_t):
    """rstd [P,1] = rsqrt(mean(src^2) + eps); sq_tile is scratch.

    Square+accum on ACT does the elementwise square and the free-axis sum
    in one pass; the rsqrt split (sqrt on ACT, reciprocal on DVE) follows
    the bass guidance (ACT Rsqrt is inaccurate).
    """
    var = nrm.tile([P, 1], F32, tag="var")
    nc.scalar.activation(out=sq_tile, in_=src_ap, func=AF.Square,
                         accum_out=var)
    sd = nrm.tile([P, 1], F32, tag="sd")
    nc.scalar.activation(out=sd, in_=var, func=AF.Sqrt,
                         bias=eps_t[:, :], scale=1.0 / D)
    rstd = nrm.tile([P, 1], F32, tag="rstd")
    nc.vector.reciprocal(out=rstd, in_=sd)
    return rstd


def _body(nc, tc, x_d, wq_t, wk_t, wv_t, wo_t, wi8_t, wil_t, wm_t,
          bias_t, cexp_d, out_d, rs_scr):
    ALU = mybir.AluOpType

    # ---- persistent small data ---------------------------------------------
    singles = tc.alloc_tile_pool(name="singles", bufs=1)
    ident16 = singles.tile([P, P], BF16)
    make_identity(nc, ident16)
    ident32 = singles.tile([P, P], F32)
    make_identity(nc, ident32)
    ident32r = singles.tile([P, P], F32R)
    nc.vector.tensor_copy(out=ident32r, in_=ident32)
    eps_t = singles.tile([P, 1], F32)
    nc.vector.memset(eps_t, EPS)
    cexp_sb = singles.tile([P, 2 * H], F32)
    bias_sb = singles.tile([P, H, 3, P], BF16)
    rstd8f = singles.tile([P, NS], F32)      # per-token rsqrt factors (f32)
    rstd_all = singles.tile([P, S], F32)     # partition-replicated row form

    # activations that live through the attention block
    qkv_act = tc.alloc_tile_pool(name="qkv_act", bufs=1)
    hT = qkv_act.tile([P, ND, S], F32R)      # rmsnorm(x)^T  [d, s]
    qT = qkv_act.tile([P, ND, S], F32R)      # q^T  [m, s] per head-pair tile
    kT = qkv_act.tile([P, ND, S], F32R)
    v_ext = qkv_act.tile([P, NS, H, HD + 1], BF16)  # [tok, stile, h, hd|1]
    nc.vector.memset(v_ext[:, :, :, HD:HD + 1], 1.0)
    # right-side stack: out1 (lives to the end) below attnT (dies after wo)
    out1_pool = tc.alloc_tile_pool(name="out1_pool", bufs=1, side="right")
    out1 = out1_pool.tile([P, NS, D], F32)    # x + attn_out, token-major
    attnT_pool = tc.alloc_tile_pool(name="attnT_pool", bufs=1, side="right")
    attnT = attnT_pool.tile([P, H // 2, S], BF16)   # heads packed 2/tile

    # ---- stage 1 + 2a interleaved: rmsnorm/transpose + V projection --------
    # emission order puts x tiles first on the SP queue, then wv chunks (split
    # per di pair for fast first-use), then bias/cexp (needed only at attn).
    with tc.tile_pool(name="xs1", bufs=2) as xs1, \
         tc.tile_pool(name="sq1", bufs=1) as sq1, \
         tc.tile_pool(name="nrm1", bufs=4) as nrm1, \
         tc.tile_pool(name="wvp", bufs=4) as wvp, \
         tc.tile_pool(name="st1ps", bufs=2, space="PSUM") as st1ps, \
         tc.tile_pool(name="vps", bufs=2, space="PSUM") as vps:
        w_sbs = []

        def emit_wv_dma(quarter):
            # wv columns for heads [4q, 4q+4), split in two [P, 4, 256] tiles
            pair = []
            for dh_ in range(2):
                w_sb = wvp.tile([P, 4, 256], F32R, tag="wv",
                                name=f"wv{quarter}_{dh_}")
                nc.sync.dma_start(
                    out=w_sb,
                    in_=wv_t[:, 4 * dh_:4 * dh_ + 4,
                             quarter * 256:(quarter + 1) * 256])
                pair.append(w_sb)
            w_sbs.append(pair)

        def emit_v(quarter, ci):
            ps = vps.tile([P, 256], F32, space="PSUM", tag="vps")
            for di in range(ND):
                nc.tensor.matmul(
                    ps[:, :],
                    hT[:, di, ci * P:(ci + 1) * P],
                    w_sbs[quarter][di // 4][:, di % 4, :],
                    start=(di == 0), stop=(di == ND - 1),
                )
            nc.scalar.activation(
                out=v_ext[:, ci, quarter * 4:quarter * 4 + 4, 0:HD],
                in_=ps[:, :].rearrange("p (h e) -> p h e", e=HD),
                func=AF.Copy, bias=0.0, scale=rstd8f[:, ci:ci + 1],
            )

        for si in range(NS):
            xt = xs1.tile([P, D], F32R, tag="x")
            nc.sync.dma_start(out=xt, in_=x_d.ap()[si * P:(si + 1) * P, :])
            if si == 2:
                emit_wv_dma(0)
            if si == 4:
                emit_wv_dma(1)
            if si == 6:
                nc.sync.dma_start(out=cexp_sb,
                                  in_=cexp_d.ap()[0:1, :].broadcast_to((P, 2 * H)))
                nc.sync.dma_start(out=bias_sb, in_=bias_t)
            # hT holds raw x^T; the rsqrt factor is folded into the V copy
            # (per-partition scale) and the QK psum drain (rstd_all multiply)
            tp = st1ps.tile([P, ND, P], F32R, space="PSUM", tag="tp1")
            for di in range(ND):
                nc.tensor.transpose(tp[:, di, :], xt[:, di * P:(di + 1) * P],
                                    ident32r[:, :])
            nc.scalar.copy(out=hT[:, :, si * P:(si + 1) * P], in_=tp[:, :, :])
            sq = sq1.tile([P, D], F32R, tag="sq")
            rstd = _rms_factor(nc, nrm1, xt[:, :], sq, eps_t)
            nc.vector.tensor_copy(out=rstd8f[:, si:si + 1], in_=rstd)
            # v for already-transposed token tiles fills PE while rms runs
            if si >= 2:
                emit_v(0, si - 2)
            if si >= 5:
                emit_v(1, si - 5)
        # bounce rstd8f through DRAM into partition-replicated row form
        nc.sync.dma_start(
            out=rs_scr.ap().rearrange("o (s t) -> (o t) s", t=P), in_=rstd8f)
        nc.sync.dma_start(out=rstd_all,
                          in_=rs_scr.ap().broadcast_to((P, S)))
        for ci in range(6, NS):
            emit_v(0, ci)
        for ci in range(3, NS):
            emit_v(1, ci)
        for quarter in range(2, 4):
            emit_wv_dma(quarter)
            for ci in range(NS):
                emit_v(quarter, ci)

    # ---- stage 2b/3: QK projection interleaved with per-head attention -----
    with tc.tile_pool(name="wqkp", bufs=3) as wqkp, \
         tc.tile_pool(name="qkp", bufs=1, space="PSUM") as qkp, \
         tc.tile_pool(name="lgp", bufs=2, space="PSUM") as lgp, \
         tc.tile_pool(name="avp", bufs=1, space="PSUM") as avp, \
         tc.tile_pool(name="tpp", bufs=1, space="PSUM") as tpp, \
         tc.tile_pool(name="exq", bufs=3) as exq, \
         tc.tile_pool(name="wexpp", bufs=4) as wexpp, \
         tc.tile_pool(name="rdp", bufs=4) as rdp, \
         tc.tile_pool(name="avsbp", bufs=1) as avsbp:

        def qk_project(hm):
            for (w_ap, dstT, nm) in ((wq_t, qT, "q"), (wk_t, kT, "k")):
                w_sb = wqkp.tile([P, ND, P], F32R, tag="w", name=f"w{nm}{hm}")
                nc.sync.dma_start(out=w_sb, in_=w_ap[:, :, hm * P:(hm + 1) * P])
                ps = qkp.tile([P, 2, 256], F32, space="PSUM", tag="qk",
                              name=f"qk{nm}{hm}")
                for qt in range(4):
                    sl = ps[:, qt % 2, :]
                    for di in range(ND):
                        nc.tensor.matmul(
                            sl,
                            w_sb[:, di, :],
                            hT[:, di, qt * 256:(qt + 1) * 256],
                            start=(di == 0), stop=(di == ND - 1),
                        )
                    if qt % 2 == 1:
                        nc.vector.tensor_mul(
                            out=dstT[:, hm, (qt - 1) * 256:(qt + 1) * 256]
                                .rearrange("p (a b) -> p a b", a=2),
                            in0=ps[:, :, :],
                            in1=rstd_all[:, (qt - 1) * 256:(qt + 1) * 256]
                                .rearrange("p (a b) -> p a b", a=2))

        def attn_head(h, prev_drain):
            hm, hb = h // 2, HD * (h % 2)
            av = avp.tile([P, NS, P], F32, space="PSUM", tag="av")
            lgs = []

            def emit_lg(ki):
                lg = lgp.tile([P, S], F32, space="PSUM", tag="lg")
                for qh in range(2):
                    nc.tensor.matmul(
                        lg[:, qh * 512:(qh + 1) * 512],
                        kT[hb:hb + HD, hm, ki * P:(ki + 1) * P],
                        qT[hb:hb + HD, hm, qh * 512:(qh + 1) * 512],
                        start=True, stop=True,
                    )
                lgs.append(lg)

            emit_lg(0)
            for ki in range(NS):
                lg = lgs[ki]
                ex = exq.tile([P, S], BF16, tag="ex")
                nc.scalar.activation(out=ex, in_=lg[:, :], func=AF.Exp)
                wexp = wexpp.tile([P, S], BF16, tag="wexp")
                # middle (varying-bias) window: q tiles [ki-1, ki+1]
                q0, q1 = max(ki - 1, 0), min(ki + 2, NS)
                d0 = q0 - (ki - 1)
                nc.vector.tensor_mul(
                    out=wexp[:, q0 * P:q1 * P].rearrange("p (c w) -> p c w", w=P),
                    in0=ex[:, q0 * P:q1 * P].rearrange("p (c w) -> p c w", w=P),
                    in1=bias_sb[:, h, d0:d0 + (q1 - q0), :],
                )
                # constant-bias segments (saturated buckets)
                segs = []
                if q0 > 0:
                    segs.append((0, q0 * P, cexp_sb[:, 2 * h:2 * h + 1]))
                if q1 < NS:
                    segs.append((q1 * P, S, cexp_sb[:, 2 * h + 1:2 * h + 2]))
                segs.sort(key=lambda t: t[1] - t[0])
                for i, (a, b, c_ap) in enumerate(segs):
                    eng = nc.gpsimd if (SEG_GPSIMD and i == len(segs) - 1) else nc.vector
                    eng.tensor_scalar_mul(out=wexp[:, a:b], in0=ex[:, a:b],
                                          scalar1=c_ap)
                # next logits tile goes to PE before av(ki), filling the
                # exp(ki) wait; previous head's transposes fill the first one
                if ki + 1 < NS:
                    emit_lg(ki + 1)
                if ki == 0 and prev_drain is not None:
                    prev_drain()
                for qi in range(NS):
                    # one accumulation group per PSUM bank (4 qi slots/bank):
                    # start pending-zeroes the whole bank, so only the first
                    # matmul in each bank starts and the last stops.
                    nc.tensor.matmul(
                        av[:, qi, 0:HD + 1],
                        wexp[:, qi * P:(qi + 1) * P],
                        v_ext[:, ki, h, :],
                        start=(ki == 0 and qi % 4 == 0),
                        stop=(ki == NS - 1 and qi % 4 == 3),
                    )
            # drain now (DVE, batched): one reciprocal over the 8 denominators
            # and one broadcast-multiply normalize; the PE transposes are
            # deferred into the next head's exp(0) window.
            av_sb = avsbp.tile([P, NS, HD], BF16, tag="avsb")
            for qi in range(NS):
                rden = rdp.tile([P, 1], F32, tag="rd")
                nc.vector.reciprocal(out=rden, in_=av[:, qi, HD:HD + 1])
                nc.vector.tensor_scalar_mul(out=av_sb[:, qi, :],
                                            in0=av[:, qi, 0:HD],
                                            scalar1=rden)

            def drain():
                tp = tpp.tile([P, NS, P], BF16, space="PSUM", tag="tp")
                tb = hb if TP_SHIFT else 0
                for qi in range(NS):
                    nc.tensor.transpose(tp[tb:tb + HD, qi, :], av_sb[:, qi, :],
                                        ident16[:, :])
                nc.vector.tensor_copy(out=attnT[hb:hb + HD, hm, :],
                                      in_=tp[tb:tb + HD, :, :])
            return drain

        pending = None
        for hm in range(ND):
            qk_project(hm)
            pending = attn_head(2 * hm, pending)
            pending = attn_head(2 * hm + 1, pending)
        pending()

    # ---- stage 4/5: attn @ wo + residual, rmsnorm -> h2T --------------------
    qkv_act.release()
    yT_pool = tc.alloc_tile_pool(name="yT_pool", bufs=1)
    yT = yT_pool.tile([P, NM, S], BF16)
    h2T_pool = tc.alloc_tile_pool(name="h2T_pool", bufs=1)
    h2T = h2T_pool.tile([P, ND, S], F8 if WI_FP8 else BF16)

    with tc.tile_pool(name="wop", bufs=1) as wop, \
         tc.tile_pool(name="xs4", bufs=3) as xs4, \
         tc.tile_pool(name="ops", bufs=2, space="PSUM") as ops, \
         tc.tile_pool(name="tp5", bufs=2, space="PSUM") as tp5, \
         tc.tile_pool(name="sq5", bufs=2) as sq5, \
         tc.tile_pool(name="nrm5", bufs=4) as nrm5, \
         tc.tile_pool(name="h2p", bufs=2) as h2p:
        wo_sb = wop.tile([P, H // 2, D], BF16)
        for si in range(NS):
            xt = xs4.tile([P, D], F32R, tag="x")
            nc.sync.dma_start(out=xt, in_=x_d.ap()[si * P:(si + 1) * P, :])
            if si == 0:
                for hq in range(4):
                    nc.sync.dma_start(out=wo_sb[:, 2 * hq:2 * hq + 2, :],
                                      in_=wo_t[:, 2 * hq:2 * hq + 2, :])
            ps = ops.tile([P, D], F32, space="PSUM", tag="wo")
            for hp in range(H // 2):
                for dh in range(2):
                    nc.tensor.matmul(
                        ps[:, dh * 512:(dh + 1) * 512],
                        attnT[:, hp, si * P:(si + 1) * P],
                        wo_sb[:, hp, dh * 512:(dh + 1) * 512],
                        start=(hp == 0), stop=(hp == H // 2 - 1),
                    )
            nc.vector.tensor_add(out=out1[:, si, :], in0=ps[:, :], in1=xt[:, :])
            # stage 5 per si: rmsnorm -> h2 (bf16) -> transpose -> h2T
            sq = sq5.tile([P, D], F32, tag="sq")
            rstd = _rms_factor(nc, nrm5, out1[:, si, :], sq, eps_t)
            h2 = h2p.tile([P, D], BF16, tag="h2")
            nc.vector.tensor_scalar_mul(out=h2, in0=out1[:, si, :],
                                        scalar1=rstd[:, :])
            tp = tp5.tile([P, ND, P], BF16, space="PSUM", tag="tp5")
            for di in range(ND):
                nc.tensor.transpose(tp[:, di, :], h2[:, di * P:(di + 1) * P],
                                    ident16[:, :])
            nc.scalar.copy(out=h2T[:, :, si * P:(si + 1) * P], in_=tp[:, :, :])
    attnT_pool.release()

    # ---- stage 6: y^T = relu(wi^T @ h2^T) ----------------------------------
    with tc.tile_pool(name="wip", bufs=2) as wip, \
         tc.tile_pool(name="psy", bufs=2, space="PSUM") as psy:
        for eighth in range(8):
            sl = slice(eighth * (MLP // 8), (eighth + 1) * (MLP // 8))
            wi_sb = wip.tile([P, ND, MLP // 8], F8 if WI_FP8 else BF16, tag="wi8")
            wi_eng = nc.scalar if ACT_DMA else nc.sync
            wi_eng.dma_start(out=wi_sb, in_=wi8_t[:, :, sl])
            if WI_FP8:
                wil_sb = wip.tile([P, ND, MLP // 8], F8, tag="wil")
                wi_eng.dma_start(out=wil_sb, in_=wil_t[:, :, sl])
            for mj in range(NM // 8):
                m0 = eighth * (NM // 8) + mj
                ps = psy.tile([P, S], F32, space="PSUM", tag="y")
                if WI_FP8:
                    for sh in range(2):
                        for ti, term in enumerate((wi_sb, wil_sb)):
                            for pr in range(4):
                                nc.tensor.matmul(
                                    ps[:, sh * 512:(sh + 1) * 512],
                                    term[:, 2 * pr:2 * pr + 2, mj * P:(mj + 1) * P],
                                    h2T[:, 2 * pr:2 * pr + 2, sh * 512:(sh + 1) * 512],
                                    start=(ti == 0 and pr == 0),
                                    stop=(ti == 1 and pr == 3),
                                    perf_mode=DR,
                                )
                else:
                    for sh in range(2):
                        for di in range(ND):
                            nc.tensor.matmul(
                                ps[:, sh * 512:(sh + 1) * 512],
                                wi_sb[:, di, mj * P:(mj + 1) * P],
                                h2T[:, di, sh * 512:(sh + 1) * 512],
                                start=(di == 0), stop=(di == ND - 1),
                            )
                nc.scalar.activation(out=yT[:, m0, :], in_=ps[:, :], func=AF.Relu)
    h2T_pool.release()

    # ---- stage 7: out = out1 + y^T.T @ womlp -------------------------------
    # 4 groups (dh half of D x sg half of tokens); womlp streamed per group,
    # two groups of PSUM banks so group g+1 computes while g drains.
    with tc.tile_pool(name="wmp", bufs=4) as wmp, \
         tc.tile_pool(name="oop", bufs=4) as oop, \
         tc.tile_pool(name="o2ps", bufs=8, space="PSUM") as o2ps:
        # group list: (dh, si list); last group split so its drain overlaps
        groups = [(0, [0, 1, 2, 3]), (0, [4, 5, 6, 7]),
                  (1, [0, 1, 2, 3]), (1, [4, 5]), (1, [6, 7])]
        for gi, (dh, sis) in enumerate(groups):
            pss = [o2ps.tile([P, 512], F32, space="PSUM", tag="o2",
                             name=f"o2_{gi}_{i}") for i in range(len(sis))]
            for cg in range(NM // 4):
                wmc = wmp.tile([P, 4, 512], BF16, tag="wm")
                dma_eng = nc.scalar if ACT_DMA else nc.sync
                dma_eng.dma_start(
                    out=wmc,
                    in_=wm_t[:, 4 * cg:4 * cg + 4, dh * 512:(dh + 1) * 512])
                for cj in range(4):
                    ci = 4 * cg + cj
                    for i4, si in enumerate(sis):
                        nc.tensor.matmul(
                            pss[i4][:, :],
                            yT[:, ci, si * P:(si + 1) * P],
                            wmc[:, cj, :],
                            start=(ci == 0), stop=(ci == NM - 1),
                        )
            for i4, si in enumerate(sis):
                oo = oop.tile([P, 512], F32, tag="oo")
                nc.vector.tensor_add(out=oo, in0=pss[i4][:, :],
                                     in1=out1[:, si, dh * 512:(dh + 1) * 512])
                nc.sync.dma_start(
                    out=out_d.ap()[si * P:(si + 1) * P, dh * 512:(dh + 1) * 512],
                    in_=oo)

    yT_pool.release()
    out1_pool.release()
    singles.release()


# ---- host wrapper ----------------------------------------------------------
_NC_CACHE = {}


def _get_nc():
    if "nc" not in _NC_CACHE:
        _NC_CACHE["nc"] = build_nc()
    return _NC_CACHE["nc"]


def _get_exec():
    """Compile once: a sharded PJRT executable over the 8 NeuronCores."""
    if "exec" in _NC_CACHE:
        return _NC_CACHE["exec"]
    import jax
    from jax.sharding import Mesh, PartitionSpec, NamedSharding
    from jax.experimental.shard_map import shard_map
    from concourse.bass2jax import (
        _bass_exec_p, install_neuronx_cc_hook, partition_id_tensor,
    )

    nc = _get_nc()
    install_neuronx_cc_hook()
    pname = nc.partition_id_tensor.name if nc.partition_id_tensor else None
    in_names, out_names, out_avals, zero_outs = [], [], [], []
    for alloc in nc.m.functions[0].allocations:
        if not isinstance(alloc, mybir.MemoryLocationSet):
            continue
        name = alloc.memorylocations[0].name
        if alloc.kind == "ExternalInput":
            if name != pname:
                in_names.append(name)
        elif alloc.kind == "ExternalOutput":
            out_names.append(name)
            shape = tuple(alloc.tensor_shape)
            dtype = mybir.dt.np(alloc.dtype)
            out_avals.append(jax.core.ShapedArray(shape, dtype))
            zero_outs.append(np.zeros(shape, dtype))
    n_params = len(in_names)
    all_in_names = in_names + out_names + ([pname] if pname else [])

    def _body(*args):
        operands = list(args)
        if pname is not None:
            operands.append(partition_id_tensor())
        outs = _bass_exec_p.bind(
            *operands,
            out_avals=tuple(out_avals),
            in_names=tuple(all_in_names),
            out_names=tuple(out_names),
            lowering_input_output_aliases=(),
            sim_require_finite=True,
            sim_require_nnan=True,
            nc=nc,
        )
        return tuple(outs)

    n_outs = len(out_avals)
    devices = jax.devices()[:NCORES]
    mesh = Mesh(np.asarray(devices), ("core",))
    sharded = jax.jit(
        shard_map(_body, mesh=mesh,
                  in_specs=(PartitionSpec("core"),) * (n_params + n_outs),
                  out_specs=(PartitionSpec("core"),) * n_outs,
                  check_rep=False),
        donate_argnums=tuple(range(n_params, n_params + n_outs)),
        keep_unused=True,
    )
    sh = NamedSharding(mesh, PartitionSpec("core"))
    _NC_CACHE["exec"] = (sharded, in_names, out_names, zero_outs, sh)
    return _NC_CACHE["exec"]


def _prep_inputs(x, ln1_scale, wq, wk, wv, wo_attn, ln2_scale, wi, wo_mlp, rel_emb):
    x = np.asarray(x, np.float32)
    ln1 = np.asarray(ln1_scale, np.float32)[:, None]
    ln2 = np.asarray(ln2_scale, np.float32)[:, None]
    wq_h = (np.asarray(wq, np.float32) * ln1).astype(np.float32)
    wk_h = (np.asarray(wk, np.float32) * ln1).astype(np.float32)
    wv_h = (np.asarray(wv, np.float32) * ln1).astype(np.float32)
    wo_h = np.asarray(wo_attn, np.float32).astype(BF16NP)
    wi_eff = np.asarray(wi, np.float32) * ln2
    if WI_FP8:
        wi8_h = wi_eff.astype(F8NP)
        wil_h = (wi_eff - wi8_h.astype(np.float32)).astype(F8NP)
    else:
        wi8_h = wi_eff.astype(BF16NP)
        wil_h = None
    wm_h = np.asarray(wo_mlp, np.float32).astype(BF16NP)
    bias3, cexp = _bias_data(np.asarray(rel_emb, np.float32))
    shared = {
        "wq": wq_h, "wk": wk_h, "wv": wv_h, "wo": wo_h,
        "wi8": wi8_h, "womlp": wm_h, "bias3": bias3, "cexp": cexp,
    }
    if WI_FP8:
        shared["wil"] = wil_h
    in_maps = [dict(shared, x=np.ascontiguousarray(x[b])) for b in range(NCORES)]
    return in_maps


def kernel(x, ln1_scale, wq, wk, wv, wo_attn, ln2_scale, wi, wo_mlp, rel_emb):
    import jax
    in_maps = _prep_inputs(x, ln1_scale, wq, wk, wv, wo_attn, ln2_scale,
                           wi, wo_mlp, rel_emb)
    sharded, in_names, out_names, zero_outs, sh = _get_exec()
    concat_in = [
        jax.device_put(
            np.concatenate([in_maps[c][n] for c in range(NCORES)], axis=0), sh)
        for n in in_names
    ]
    czero = [
        jax.device_put(np.zeros((NCORES * z.shape[0], *z.shape[1:]), z.dtype), sh)
        for z in zero_outs
    ]
    outs = sharded(*concat_in, *czero)
    oidx = out_names.index("out")
    full = np.asarray(outs[oidx]).reshape(NCORES, S, D)
    return full.astype(np.float32)


# revision 41
# speedup vs baseline: 1.3196x; 1.0017x over previous
"""T5-style encoder layer (pre-LN, RMSNorm, relative-position bias) on 8 trn2
NeuronCores, data-parallel over the batch dimension (B=8 -> one batch element
per core). Each core runs the full layer for its [S, D] slice; weights and the
relative-bias data are replicated.

Self-contained: hardcodes all shapes; only depends on the runtime at
/opt/trn_rl_repo.
"""

import sys

if "/opt/trn_rl_repo" not in sys.path:
    sys.path.insert(0, "/opt/trn_rl_repo")

import numpy as np
import ml_dtypes

import concourse.bass as bass
import concourse.tile as tile
from concourse import bacc
from concourse import mybir
from concourse.masks import make_identity

# ---- problem constants -----------------------------------------------------
B, S, D = 8, 1024, 1024
H, HD = 16, 64
MLP = 4096
NUM_BUCKETS, MAX_DIST = 32, 128
EPS = 1e-6
NCORES = 8
P = 128
NS = S // P        # 8 token tiles
ND = D // P        # 8 feature tiles
NM = MLP // P      # 32 mlp tiles

F32 = mybir.dt.float32
F32R = mybir.dt.float32r
BF16 = mybir.dt.bfloat16
F8 = mybir.dt.float8e4
BF16NP = ml_dtypes.bfloat16
F8NP = ml_dtypes.float8_e4m3

import os
WI_FP8 = os.environ.get("K_WI_FP8", "1") == "1"   # 2-term fp8 DoubleRow wi
SEG_GPSIMD = os.environ.get("K_SEG_GPSIMD", "1") == "1"
ACT_DMA = os.environ.get("K_ACT_DMA", "1") == "1"
TP_SHIFT = os.environ.get("K_TP_SHIFT", "1") == "1"

AF = mybir.ActivationFunctionType
DR = mybir.MatmulPerfMode.DoubleRow


# ---- host-side relative position bias --------------------------------------
def _rel_pos_bucket_np(rel):
    # mirrors t5x _relative_position_bucket (bidirectional), numpy fp32
    n = -rel
    num_buckets = NUM_BUCKETS // 2          # 16
    ret = (n < 0).astype(np.int32) * num_buckets
    n = np.abs(n)
    max_exact = num_buckets // 2            # 8
    is_small = n < max_exact
    val_if_large = max_exact + (
        np.log(n.astype(np.float32) / max_exact + np.finfo(np.float32).eps)
        / np.log(MAX_DIST / max_exact)
        * (num_buckets - max_exact)
    ).astype(np.int32)
    val_if_large = np.minimum(val_if_large, num_buckets - 1)
    return ret + np.where(is_small, n, val_if_large)


def _bias_data(rel_emb):
    """Compressed exp(bias) data.

    Returns (bias3, cexp):
      bias3 [H, 128, 3, 128] bf16: block d (m = 1-d = k_tile - q_tile) at
        [p, c] = exp(bias[k, q]) for k = k_tile*128 + p, q = q_tile*128 + c,
        i.e. exp(table[k - q]) with k - q = m*128 + p - c.
      cexp  [1, 2H] f32: per head, exp of the two saturated buckets:
        [2h]   = exp(emb[31, h])  (k - q >= 128, q_tile <= k_tile - 2)
        [2h+1] = exp(emb[15, h])  (k - q <= -128, q_tile >= k_tile + 2)
    """
    rel = np.arange(-(S - 1), S, dtype=np.int32)          # k - q in [-1023, 1023]
    buckets = _rel_pos_bucket_np(rel)                     # [2047]
    table = rel_emb[buckets, :].astype(np.float32)        # [2047, H]
    pp = np.arange(P)[:, None, None]
    dd = np.arange(3)[None, :, None]
    cc = np.arange(P)[None, None, :]
    idx = 1023 + (1 - dd) * P + pp - cc                   # [128, 3, 128]
    blocks = np.exp(table[idx])                           # [128, 3, 128, H]
    bias3 = np.ascontiguousarray(blocks.transpose(3, 0, 1, 2)).astype(BF16NP)
    cexp = np.empty((1, 2 * H), np.float32)
    cexp[0, 0::2] = np.exp(rel_emb[31, :].astype(np.float32))
    cexp[0, 1::2] = np.exp(rel_emb[15, :].astype(np.float32))
    return bias3, cexp


# ---- device kernel ---------------------------------------------------------
def build_nc():
    nc = bacc.Bacc(None, target_bir_lowering=False)

    x_d = nc.declare_dram_parameter("x", [S, D], F32R, isOutput=False)
    wq_d = nc.declare_dram_parameter("wq", [D, H * HD], F32R, isOutput=False)
    wk_d = nc.declare_dram_parameter("wk", [D, H * HD], F32R, isOutput=False)
    wv_d = nc.declare_dram_parameter("wv", [D, H * HD], F32R, isOutput=False)
    wo_d = nc.declare_dram_parameter("wo", [H * HD, D], BF16, isOutput=False)
    if WI_FP8:
        wi8_d = nc.declare_dram_parameter("wi8", [D, MLP], F8, isOutput=False)
        wil_d = nc.declare_dram_parameter("wil", [D, MLP], F8, isOutput=False)
    else:
        wi8_d = nc.declare_dram_parameter("wi8", [D, MLP], BF16, isOutput=False)
        wil_d = None
    wm_d = nc.declare_dram_parameter("womlp", [MLP, D], BF16, isOutput=False)
    bias_d = nc.declare_dram_parameter("bias3", [H, P, 3, P], BF16, isOutput=False)
    cexp_d = nc.declare_dram_parameter("cexp", [1, 2 * H], F32, isOutput=False)
    out_d = nc.declare_dram_parameter("out", [S, D], F32, isOutput=True)
    rs_scr = nc.dram_tensor("rs_scr", [1, S], F32)

    wo_t = wo_d.ap().rearrange("(hp p) d -> p hp d", p=P)
    wq_t = wq_d.ap().rearrange("(di p) m -> p di m", p=P)
    wk_t = wk_d.ap().rearrange("(di p) m -> p di m", p=P)
    wv_t = wv_d.ap().rearrange("(di p) m -> p di m", p=P)
    wi8_t = wi8_d.ap().rearrange("(di p) m -> p di m", p=P)
    wil_t = wil_d.ap().rearrange("(di p) m -> p di m", p=P) if WI_FP8 else None
    wm_t = wm_d.ap().rearrange("(ci p) d -> p ci d", p=P)
    bias_t = bias_d.ap().rearrange("h p d c -> p h d c")

    with tile.TileContext(nc) as tc:
        _body(nc, tc, x_d, wq_t, wk_t, wv_t, wo_t, wi8_t, wil_t, wm_t,
              bias_t, cexp_d, out_d, rs_scr)
    nc.finalize()
    return nc


def _rms_factor(nc, nrm, src_ap, sq_tile, eps_t):
    """rstd [P,1] = rsqrt(mean(src^2) + eps); sq_tile is scratch.

    Square+accum on ACT does the elementwise square and the free-axis sum
    in one pass; the rsqrt split (sqrt on ACT, reciprocal on DVE) follows
    the bass guidance (ACT Rsqrt is inaccurate).
    """
    var = nrm.tile([P, 1], F32, tag="var")
    nc.scalar.activation(out=sq_tile, in_=src_ap, func=AF.Square,
                         accum_out=var)
    sd = nrm.tile([P, 1], F32, tag="sd")
    nc.scalar.activation(out=sd, in_=var, func=AF.Sqrt,
                         bias=eps_t[:, :], scale=1.0 / D)
    rstd = nrm.tile([P, 1], F32, tag="rstd")
    nc.vector.reciprocal(out=rstd, in_=sd)
    return rstd


def _body(nc, tc, x_d, wq_t, wk_t, wv_t, wo_t, wi8_t, wil_t, wm_t,
          bias_t, cexp_d, out_d, rs_scr):
    ALU = mybir.AluOpType

    # ---- persistent small data ---------------------------------------------
    singles = tc.alloc_tile_pool(name="singles", bufs=1)
    ident16 = singles.tile([P, P], BF16)
    make_identity(nc, ident16)
    ident32 = singles.tile([P, P], F32)
    make_identity(nc, ident32)
    ident32r = singles.tile([P, P], F32R)
    nc.vector.tensor_copy(out=ident32r, in_=ident32)
    eps_t = singles.tile([P, 1], F32)
    nc.vector.memset(eps_t, EPS)
    cexp_sb = singles.tile([P, 2 * H], F32)
    bias_sb = singles.tile([P, H, 3, P], BF16)
    rstd8f = singles.tile([P, NS], F32)      # per-token rsqrt factors (f32)
    rstd_all = singles.tile([P, S], F32)     # partition-replicated row form

    # activations that live through the attention block
    qkv_act = tc.alloc_tile_pool(name="qkv_act", bufs=1)
    hT = qkv_act.tile([P, ND, S], F32R)      # rmsnorm(x)^T  [d, s]
    qT = qkv_act.tile([P, ND, S], F32R)      # q^T  [m, s] per head-pair tile
    kT = qkv_act.tile([P, ND, S], F32R)
    v_ext = qkv_act.tile([P, NS, H, HD + 1], BF16)  # [tok, stile, h, hd|1]
    nc.vector.memset(v_ext[:, :, :, HD:HD + 1], 1.0)
    # right-side stack: out1 (lives to the end) below attnT (dies after wo)
    out1_pool = tc.alloc_tile_pool(name="out1_pool", bufs=1, side="right")
    out1 = out1_pool.tile([P, NS, D], F32)    # x + attn_out, token-major
    attnT_pool = tc.alloc_tile_pool(name="attnT_pool", bufs=1, side="right")
    attnT = attnT_pool.tile([P, H // 2, S], BF16)   # heads packed 2/tile

    # ---- stage 1 + 2a interleaved: rmsnorm/transpose + V projection --------
    # emission order puts x tiles first on the SP queue, then wv chunks (split
    # per di pair for fast first-use), then bias/cexp (needed only at attn).
    with tc.tile_pool(name="xs1", bufs=2) as xs1, \
         tc.tile_pool(name="sq1", bufs=1) as sq1, \
         tc.tile_pool(name="nrm1", bufs=4) as nrm1, \
         tc.tile_pool(name="wvp", bufs=4) as wvp, \
         tc.tile_pool(name="st1ps", bufs=2, space="PSUM") as st1ps, \
         tc.tile_pool(name="vps", bufs=2, space="PSUM") as vps:
        w_sbs = []

        def emit_wv_dma(quarter):
            # wv columns for heads [4q, 4q+4), split in two [P, 4, 256] tiles
            pair = []
            for dh_ in range(2):
                w_sb = wvp.tile([P, 4, 256], F32R, tag="wv",
                                name=f"wv{quarter}_{dh_}")
                nc.sync.dma_start(
                    out=w_sb,
                    in_=wv_t[:, 4 * dh_:4 * dh_ + 4,
                             quarter * 256:(quarter + 1) * 256])
                pair.append(w_sb)
            w_sbs.append(pair)

        def emit_v(quarter, ci):
            ps = vps.tile([P, 256], F32, space="PSUM", tag="vps")
            for di in range(ND):
                nc.tensor.matmul(
                    ps[:, :],
                    hT[:, di, ci * P:(ci + 1) * P],
                    w_sbs[quarter][di // 4][:, di % 4, :],
                    start=(di == 0), stop=(di == ND - 1),
                )
            nc.scalar.activation(
                out=v_ext[:, ci, quarter * 4:quarter * 4 + 4, 0:HD],
                in_=ps[:, :].rearrange("p (h e) -> p h e", e=HD),
                func=AF.Copy, bias=0.0, scale=rstd8f[:, ci:ci + 1],
            )

        for si in range(NS):
            xt = xs1.tile([P, D], F32R, tag="x")
            nc.sync.dma_start(out=xt, in_=x_d.ap()[si * P:(si + 1) * P, :])
            if si == 1:
                emit_wv_dma(0)
            if si == 3:
                emit_wv_dma(1)
            if si == 6:
                nc.sync.dma_start(out=cexp_sb,
                                  in_=cexp_d.ap()[0:1, :].broadcast_to((P, 2 * H)))
                nc.sync.dma_start(out=bias_sb, in_=bias_t)
            # hT holds raw x^T; the rsqrt factor is folded into the V copy
            # (per-partition scale) and the QK psum drain (rstd_all multiply)
            tp = st1ps.tile([P, ND, P], F32R, space="PSUM", tag="tp1")
            for di in range(ND):
                nc.tensor.transpose(tp[:, di, :], xt[:, di * P:(di + 1) * P],
                                    ident32r[:, :])
            nc.scalar.copy(out=hT[:, :, si * P:(si + 1) * P], in_=tp[:, :, :])
            sq = sq1.tile([P, D], F32R, tag="sq")
            rstd = _rms_factor(nc, nrm1, xt[:, :], sq, eps_t)
            nc.vector.tensor_copy(out=rstd8f[:, si:si + 1], in_=rstd)
            # v for already-transposed token tiles fills PE while rms runs
            if si >= 2:
                emit_v(0, si - 2)
            if si >= 4:
                emit_v(1, si - 4)
        # bounce rstd8f through DRAM into partition-replicated row form
        nc.sync.dma_start(
            out=rs_scr.ap().rearrange("o (s t) -> (o t) s", t=P), in_=rstd8f)
        nc.sync.dma_start(out=rstd_all,
                          in_=rs_scr.ap().broadcast_to((P, S)))
        for ci in range(6, NS):
            emit_v(0, ci)
        emit_wv_dma(2)
        for ci in range(4, NS):
            emit_v(1, ci)
        emit_wv_dma(3)
        for ci in range(NS):
            emit_v(2, ci)
        for ci in range(NS):
            emit_v(3, ci)

    # ---- stage 2b/3: QK projection interleaved with per-head attention -----
    with tc.tile_pool(name="wqkp", bufs=3) as wqkp, \
         tc.tile_pool(name="qkp", bufs=1, space="PSUM") as qkp, \
         tc.tile_pool(name="lgp", bufs=2, space="PSUM") as lgp, \
         tc.tile_pool(name="avp", bufs=1, space="PSUM") as avp, \
         tc.tile_pool(name="tpp", bufs=1, space="PSUM") as tpp, \
         tc.tile_pool(name="exq", bufs=3) as exq, \
         tc.tile_pool(name="wexpp", bufs=4) as wexpp, \
         tc.tile_pool(name="rdp", bufs=4) as rdp, \
         tc.tile_pool(name="avsbp", bufs=1) as avsbp:

        def qk_project(hm):
            for (w_ap, dstT, nm) in ((wq_t, qT, "q"), (wk_t, kT, "k")):
                w_sb = wqkp.tile([P, ND, P], F32R, tag="w", name=f"w{nm}{hm}")
                nc.sync.dma_start(out=w_sb, in_=w_ap[:, :, hm * P:(hm + 1) * P])
                ps = qkp.tile([P, 2, 256], F32, space="PSUM", tag="qk",
                              name=f"qk{nm}{hm}")
                for qt in range(4):
                    sl = ps[:, qt % 2, :]
                    for di in range(ND):
                        nc.tensor.matmul(
                            sl,
                            w_sb[:, di, :],
                            hT[:, di, qt * 256:(qt + 1) * 256],
                            start=(di == 0), stop=(di == ND - 1),
                        )
                    if qt % 2 == 1:
                        nc.vector.tensor_mul(
                            out=dstT[:, hm, (qt - 1) * 256:(qt + 1) * 256]
                                .rearrange("p (a b) -> p a b", a=2),
                            in0=ps[:, :, :],
                            in1=rstd_all[:, (qt - 1) * 256:(qt + 1) * 256]
                                .rearrange("p (a b) -> p a b", a=2))

        def attn_head(h, prev_drain):
            hm, hb = h // 2, HD * (h % 2)
            av = avp.tile([P, NS, P], F32, space="PSUM", tag="av")
            lgs = []

            def emit_lg(ki):
                lg = lgp.tile([P, S], F32, space="PSUM", tag="lg")
                for qh in range(2):
                    nc.tensor.matmul(
                        lg[:, qh * 512:(qh + 1) * 512],
                        kT[hb:hb + HD, hm, ki * P:(ki + 1) * P],
                        qT[hb:hb + HD, hm, qh * 512:(qh + 1) * 512],
                        start=True, stop=True,
                    )
                lgs.append(lg)

            emit_lg(0)
            for ki in range(NS):
                lg = lgs[ki]
                ex = exq.tile([P, S], BF16, tag="ex")
                nc.scalar.activation(out=ex, in_=lg[:, :], func=AF.Exp)
                wexp = wexpp.tile([P, S], BF16, tag="wexp")
                # middle (varying-bias) window: q tiles [ki-1, ki+1]
                q0, q1 = max(ki - 1, 0), min(ki + 2, NS)
                d0 = q0 - (ki - 1)
                nc.vector.tensor_mul(
                    out=wexp[:, q0 * P:q1 * P].rearrange("p (c w) -> p c w", w=P),
                    in0=ex[:, q0 * P:q1 * P].rearrange("p (c w) -> p c w", w=P),
                    in1=bias_sb[:, h, d0:d0 + (q1 - q0), :],
                )
                # constant-bias segments (saturated buckets)
                segs = []
                if q0 > 0:
                    segs.append((0, q0 * P, cexp_sb[:, 2 * h:2 * h + 1]))
                if q1 < NS:
                    segs.append((q1 * P, S, cexp_sb[:, 2 * h + 1:2 * h + 2]))
                segs.sort(key=lambda t: t[1] - t[0])
                for i, (a, b, c_ap) in enumerate(segs):
                    eng = nc.gpsimd if (SEG_GPSIMD and i == len(segs) - 1) else nc.vector
                    eng.tensor_scalar_mul(out=wexp[:, a:b], in0=ex[:, a:b],
                                          scalar1=c_ap)
                # next logits tile goes to PE before av(ki), filling the
                # exp(ki) wait; previous head's transposes fill the first one
                if ki + 1 < NS:
                    emit_lg(ki + 1)
                if ki == 0 and prev_drain is not None:
                    prev_drain()
                for qi in range(NS):
                    # one accumulation group per PSUM bank (4 qi slots/bank):
                    # start pending-zeroes the whole bank, so only the first
                    # matmul in each bank starts and the last stops.
                    nc.tensor.matmul(
                        av[:, qi, 0:HD + 1],
                        wexp[:, qi * P:(qi + 1) * P],
                        v_ext[:, ki, h, :],
                        start=(ki == 0 and qi % 4 == 0),
                        stop=(ki == NS - 1 and qi % 4 == 3),
                    )
            # drain now (DVE, batched): one reciprocal over the 8 denominators
            # and one broadcast-multiply normalize; the PE transposes are
            # deferred into the next head's exp(0) window.
            av_sb = avsbp.tile([P, NS, HD], BF16, tag="avsb")
            for qi in range(NS):
                rden = rdp.tile([P, 1], F32, tag="rd")
                nc.vector.reciprocal(out=rden, in_=av[:, qi, HD:HD + 1])
                nc.vector.tensor_scalar_mul(out=av_sb[:, qi, :],
                                            in0=av[:, qi, 0:HD],
                                            scalar1=rden)

            def drain():
                tp = tpp.tile([P, NS, P], BF16, space="PSUM", tag="tp")
                tb = hb if TP_SHIFT else 0
                for qi in range(NS):
                    nc.tensor.transpose(tp[tb:tb + HD, qi, :], av_sb[:, qi, :],
                                        ident16[:, :])
                nc.vector.tensor_copy(out=attnT[hb:hb + HD, hm, :],
                                      in_=tp[tb:tb + HD, :, :])
            return drain

        pending = None
        for hm in range(ND):
            qk_project(hm)
            pending = attn_head(2 * hm, pending)
            pending = attn_head(2 * hm + 1, pending)
        pending()

    # ---- stage 4/5: attn @ wo + residual, rmsnorm -> h2T --------------------
    qkv_act.release()
    yT_pool = tc.alloc_tile_pool(name="yT_pool", bufs=1)
    yT = yT_pool.tile([P, NM, S], BF16)
    h2T_pool = tc.alloc_tile_pool(name="h2T_pool", bufs=1)
    h2T = h2T_pool.tile([P, ND, S], F8 if WI_FP8 else BF16)

    with tc.tile_pool(name="wop", bufs=1) as wop, \
         tc.tile_pool(name="xs4", bufs=3) as xs4, \
         tc.tile_pool(name="ops", bufs=2, space="PSUM") as ops, \
         tc.tile_pool(name="tp5", bufs=2, space="PSUM") as tp5, \
         tc.tile_pool(name="sq5", bufs=2) as sq5, \
         tc.tile_pool(name="nrm5", bufs=4) as nrm5, \
         tc.tile_pool(name="h2p", bufs=2) as h2p:
        wo_sb = wop.tile([P, H // 2, D], BF16)
        for si in range(NS):
            xt = xs4.tile([P, D], F32R, tag="x")
            nc.sync.dma_start(out=xt, in_=x_d.ap()[si * P:(si + 1) * P, :])
            if si == 0:
                for hq in range(4):
                    nc.sync.dma_start(out=wo_sb[:, 2 * hq:2 * hq + 2, :],
                                      in_=wo_t[:, 2 * hq:2 * hq + 2, :])
            ps = ops.tile([P, D], F32, space="PSUM", tag="wo")
            for hp in range(H // 2):
                for dh in range(2):
                    nc.tensor.matmul(
                        ps[:, dh * 512:(dh + 1) * 512],
                        attnT[:, hp, si * P:(si + 1) * P],
                        wo_sb[:, hp, dh * 512:(dh + 1) * 512],
                        start=(hp == 0), stop=(hp == H // 2 - 1),
                    )
            nc.vector.tensor_add(out=out1[:, si, :], in0=ps[:, :], in1=xt[:, :])
            # stage 5 per si: rmsnorm -> h2 (bf16) -> transpose -> h2T
            sq = sq5.tile([P, D], F32, tag="sq")
            rstd = _rms_factor(nc, nrm5, out1[:, si, :], sq, eps_t)
            h2 = h2p.tile([P, D], BF16, tag="h2")
            nc.vector.tensor_scalar_mul(out=h2, in0=out1[:, si, :],
                                        scalar1=rstd[:, :])
            tp = tp5.tile([P, ND, P], BF16, space="PSUM", tag="tp5")
            for di in range(ND):
                nc.tensor.transpose(tp[:, di, :], h2[:, di * P:(di + 1) * P],
                                    ident16[:, :])
            nc.scalar.copy(out=h2T[:, :, si * P:(si + 1) * P], in_=tp[:, :, :])
    attnT_pool.release()

    # ---- stage 6: y^T = relu(wi^T @ h2^T) ----------------------------------
    with tc.tile_pool(name="wip", bufs=2) as wip, \
         tc.tile_pool(name="psy", bufs=2, space="PSUM") as psy:
        for eighth in range(8):
            sl = slice(eighth * (MLP // 8), (eighth + 1) * (MLP // 8))
            wi_sb = wip.tile([P, ND, MLP // 8], F8 if WI_FP8 else BF16, tag="wi8")
            wi_eng = nc.scalar if ACT_DMA else nc.sync
            wi_eng.dma_start(out=wi_sb, in_=wi8_t[:, :, sl])
            if WI_FP8:
                wil_sb = wip.tile([P, ND, MLP // 8], F8, tag="wil")
                wi_eng.dma_start(out=wil_sb, in_=wil_t[:, :, sl])
            for mj in range(NM // 8):
                m0 = eighth * (NM // 8) + mj
                ps = psy.tile([P, S], F32, space="PSUM", tag="y")
                if WI_FP8:
                    for sh in range(2):
                        for ti, term in enumerate((wi_sb, wil_sb)):
                            for pr in range(4):
                                nc.tensor.matmul(
                                    ps[:, sh * 512:(sh + 1) * 512],
                                    term[:, 2 * pr:2 * pr + 2, mj * P:(mj + 1) * P],
                                    h2T[:, 2 * pr:2 * pr + 2, sh * 512:(sh + 1) * 512],
                                    start=(ti == 0 and pr == 0),
                                    stop=(ti == 1 and pr == 3),
                                    perf_mode=DR,
                                )
                else:
                    for sh in range(2):
                        for di in range(ND):
                            nc.tensor.matmul(
                                ps[:, sh * 512:(sh + 1) * 512],
                                wi_sb[:, di, mj * P:(mj + 1) * P],
                                h2T[:, di, sh * 512:(sh + 1) * 512],
                                start=(di == 0), stop=(di == ND - 1),
                            )
                nc.scalar.activation(out=yT[:, m0, :], in_=ps[:, :], func=AF.Relu)
    h2T_pool.release()

    # ---- stage 7: out = out1 + y^T.T @ womlp -------------------------------
    # 4 groups (dh half of D x sg half of tokens); womlp streamed per group,
    # two groups of PSUM banks so group g+1 computes while g drains.
    with tc.tile_pool(name="wmp", bufs=4) as wmp, \
         tc.tile_pool(name="oop", bufs=4) as oop, \
         tc.tile_pool(name="o2ps", bufs=8, space="PSUM") as o2ps:
        # group list: (dh, si list); last group split so its drain overlaps
        groups = [(0, [0, 1, 2, 3]), (0, [4, 5, 6, 7]),
                  (1, [0, 1, 2, 3]), (1, [4, 5]), (1, [6, 7])]
        for gi, (dh, sis) in enumerate(groups):
            pss = [o2ps.tile([P, 512], F32, space="PSUM", tag="o2",
                             name=f"o2_{gi}_{i}") for i in range(len(sis))]
            for cg in range(NM // 4):
                wmc = wmp.tile([P, 4, 512], BF16, tag="wm")
                dma_eng = nc.scalar if ACT_DMA else nc.sync
                dma_eng.dma_start(
                    out=wmc,
                    in_=wm_t[:, 4 * cg:4 * cg + 4, dh * 512:(dh + 1) * 512])
                for cj in range(4):
                    ci = 4 * cg + cj
                    for i4, si in enumerate(sis):
                        nc.tensor.matmul(
                            pss[i4][:, :],
                            yT[:, ci, si * P:(si + 1) * P],
                            wmc[:, cj, :],
                            start=(ci == 0), stop=(ci == NM - 1),
                        )
            for i4, si in enumerate(sis):
                oo = oop.tile([P, 512], F32, tag="oo")
                nc.vector.tensor_add(out=oo, in0=pss[i4][:, :],
                                     in1=out1[:, si, dh * 512:(dh + 1) * 512])
                nc.sync.dma_start(
                    out=out_d.ap()[si * P:(si + 1) * P, dh * 512:(dh + 1) * 512],
                    in_=oo)

    yT_pool.release()
    out1_pool.release()
    singles.release()


# ---- host wrapper ----------------------------------------------------------
_NC_CACHE = {}


def _get_nc():
    if "nc" not in _NC_CACHE:
        _NC_CACHE["nc"] = build_nc()
    return _NC_CACHE["nc"]


def _get_exec():
    """Compile once: a sharded PJRT executable over the 8 NeuronCores."""
    if "exec" in _NC_CACHE:
        return _NC_CACHE["exec"]
    import jax
    from jax.sharding import Mesh, PartitionSpec, NamedSharding
    from jax.experimental.shard_map import shard_map
    from concourse.bass2jax import (
        _bass_exec_p, install_neuronx_cc_hook, partition_id_tensor,
    )

    nc = _get_nc()
    install_neuronx_cc_hook()
    pname = nc.partition_id_tensor.name if nc.partition_id_tensor else None
    in_names, out_names, out_avals, zero_outs = [], [], [], []
    for alloc in nc.m.functions[0].allocations:
        if not isinstance(alloc, mybir.MemoryLocationSet):
            continue
        name = alloc.memorylocations[0].name
        if alloc.kind == "ExternalInput":
            if name != pname:
                in_names.append(name)
        elif alloc.kind == "ExternalOutput":
            out_names.append(name)
            shape = tuple(alloc.tensor_shape)
            dtype = mybir.dt.np(alloc.dtype)
            out_avals.append(jax.core.ShapedArray(shape, dtype))
            zero_outs.append(np.zeros(shape, dtype))
    n_params = len(in_names)
    all_in_names = in_names + out_names + ([pname] if pname else [])

    def _body(*args):
        operands = list(args)
        if pname is not None:
            operands.append(partition_id_tensor())
        outs = _bass_exec_p.bind(
            *operands,
            out_avals=tuple(out_avals),
            in_names=tuple(all_in_names),
            out_names=tuple(out_names),
            lowering_input_output_aliases=(),
            sim_require_finite=True,
            sim_require_nnan=True,
            nc=nc,
        )
        return tuple(outs)

    n_outs = len(out_avals)
    devices = jax.devices()[:NCORES]
    mesh = Mesh(np.asarray(devices), ("core",))
    sharded = jax.jit(
        shard_map(_body, mesh=mesh,
                  in_specs=(PartitionSpec("core"),) * (n_params + n_outs),
                  out_specs=(PartitionSpec("core"),) * n_outs,
                  check_rep=False),
        donate_argnums=tuple(range(n_params, n_params + n_outs)),
        keep_unused=True,
    )
    sh = NamedSharding(mesh, PartitionSpec("core"))
    _NC_CACHE["exec"] = (sharded, in_names, out_names, zero_outs, sh)
    return _NC_CACHE["exec"]


def _prep_inputs(x, ln1_scale, wq, wk, wv, wo_attn, ln2_scale, wi, wo_mlp, rel_emb):
    x = np.asarray(x, np.float32)
    ln1 = np.asarray(ln1_scale, np.float32)[:, None]
    ln2 = np.asarray(ln2_scale, np.float32)[:, None]
    wq_h = (np.asarray(wq, np.float32) * ln1).astype(np.float32)
    wk_h = (np.asarray(wk, np.float32) * ln1).astype(np.float32)
    wv_h = (np.asarray(wv, np.float32) * ln1).astype(np.float32)
    wo_h = np.asarray(wo_attn, np.float32).astype(BF16NP)
    wi_eff = np.asarray(wi, np.float32) * ln2
    if WI_FP8:
        wi8_h = wi_eff.astype(F8NP)
        wil_h = (wi_eff - wi8_h.astype(np.float32)).astype(F8NP)
    else:
        wi8_h = wi_eff.astype(BF16NP)
        wil_h = None
    wm_h = np.asarray(wo_mlp, np.float32).astype(BF16NP)
    bias3, cexp = _bias_data(np.asarray(rel_emb, np.float32))
    shared = {
        "wq": wq_h, "wk": wk_h, "wv": wv_h, "wo": wo_h,
        "wi8": wi8_h, "womlp": wm_h, "bias3": bias3, "cexp": cexp,
    }
    if WI_FP8:
        shared["wil"] = wil_h
    in_maps = [dict(shared, x=np.ascontiguousarray(x[b])) for b in range(NCORES)]
    return in_maps


def kernel(x, ln1_scale, wq, wk, wv, wo_attn, ln2_scale, wi, wo_mlp, rel_emb):
    import jax
    in_maps = _prep_inputs(x, ln1_scale, wq, wk, wv, wo_attn, ln2_scale,
                           wi, wo_mlp, rel_emb)
    sharded, in_names, out_names, zero_outs, sh = _get_exec()
    concat_in = [
        jax.device_put(
            np.concatenate([in_maps[c][n] for c in range(NCORES)], axis=0), sh)
        for n in in_names
    ]
    czero = [
        jax.device_put(np.zeros((NCORES * z.shape[0], *z.shape[1:]), z.dtype), sh)
        for z in zero_outs
    ]
    outs = sharded(*concat_in, *czero)
    oidx = out_names.index("out")
    full = np.asarray(outs[oidx]).reshape(NCORES, S, D)
    return full.astype(np.float32)
